# revision 1
# baseline (speedup 1.0000x reference)
"""AttentionAugmentation2D Trainium2 kernel (v5).

Shapes (hardcoded): B=8, H=W=32, N=1024, NH=8 heads, dk=dv=32 per head.
inputs [8,32,32,768] = q|k|v (256 each), key_rel_h/w [63,32].

Sharding: data-parallel over batch B across the 8 cores. Each core runs the
full 8-head attention for its batch.

Math per (batch, head), with n=(i,j), m=(i',j') (i = H index):
  logits[n,m] = q[n]@k[m] + q[(j,i)]@rel_h[i'-i+31] + q[(i,j)]@rel_w[i'-i+31]
Both rel terms depend on m only through i', so with
  SWT[u,n] = rel_w[u]@q[(i,j)] + rel_h[u]@q[(j,i)]        (u in [0,63))
  biasT[t,n] = SWT[t+31-i(n), n]                          (shifted windows)
we get  logits^T = K_aug^T.T @ Q_augT  with contraction 64:
  K_aug^T rows: 0:32 = k^T, 32:64 = onehot[t==i'(m)]
  Q_augT rows:  0:32 = q^T, 32:64 = biasT
biasT is computed directly as 32 small matmuls (one per i-block of n),
each using a shifted 32-column window of rel^T as the stationary operand —
no DRAM round-trip. These run in bf16 (a 32-wide free dim would be 4x
slower in f32r on the PE); the bias is small relative to q@k so bf16 is
plenty.  The 1/sqrt(dk) q-scale is folded into the exp activation's scale
operand (logits are linear in q). Softmax without max-subtraction (scaled
logits bounded ~+-8 for randn inputs); row sums come free from a
ones-column appended to V in the attn@V matmul.

Pipeline design (cost-model driven): the Act engine's 64 exps (~66us) are
the floor; everything else is arranged to hide behind them:
 - head-0 critical chain front-loaded and engine-balanced: q-half0 DMA ->
   PE transposes -> {bf16 q^T copy (DVE) || permuted copy (Pool)} -> bias
   matmuls (PE) -> PSUM copies (DVE) -> first logits; dummy PE matmuls at
   t=0 keep the PE p-state ramp warm before the first real transpose;
 - heads 0/1 run their bias matmuls inside the then-unused odd attn@V
   PSUM slot, so the two shared scratch PSUM banks never sit between the
   q/k transposes and the first logits;
 - rel/v/half-1-row DMAs ride the Act-engine DMA queue so the SP queue
   serves nothing but the critical row loads (and later output stores);
 - half-1 input transposes emitted inside head 0's j-loop;
 - attn@V accumulates into a manually double-buffered PSUM region
   (partition slots 0:33 / 64:97);
 - per-head attn^T copy to SBUF at head end; transpose+normalize flush of
   head h interleaved into head h+1's j-loop; outputs shipped as quad
   DMAs (pair DMAs on the final head to shorten the tail).

Toolchain note: walrus codegen only fits ONE semaphore wait in most TPB
instruction structs and does not split excess waits itself (stock kernels
trip this too).  split_multiwaits() below is a BIR post-pass that moves
excess waits onto same-engine InstNoOp carriers placed immediately before
the offending instruction — semantically identical, compiles everywhere.
"""

import numpy as np

import concourse.bass as bass
import concourse.mybir as mybir
import concourse.tile as tile
from concourse import bass_utils
from concourse.masks import make_identity
from concourse.tile import add_dep_helper

F32 = mybir.dt.float32
F32R = mybir.dt.float32r
BF16 = mybir.dt.bfloat16
AF = mybir.ActivationFunctionType

NH = 8
N = 1024
DK = 32
SCALE = float(DK) ** -0.5


def split_multiwaits(nc, dma_limit=1):
    """Move excess semaphore waits onto same-engine nop carriers."""
    n_new = 0
    for f in nc.m.functions:
        for blk in f.blocks:
            newlist = []
            for inst in blk.instructions:
                si = getattr(inst, "sync_info", None)
                is_dma = isinstance(inst, mybir.InstDMACopy)
                limit = dma_limit if is_dma else 1
                if si is not None and len(si.on_wait) > limit:
                    waits = list(si.on_wait)
                    for w in waits[:-1]:
                        n_new += 1
                        newlist.append(mybir.InstNoOp(
                            name=f"I-wc{n_new}",
                            ins=[], outs=[],
                            sync_info=mybir.SyncInfo(on_wait=[w], on_update=[]),
                            bass_nofuse=True,
                            engine=inst.engine,
                        ))
                    inst.sync_info = mybir.SyncInfo(
                        on_wait=waits[-1:], on_update=si.on_update)
                newlist.append(inst)
            blk.instructions = newlist
    return n_new


def kernel_body(tc, outs, ins):
    nc = tc.nc
    x = ins["x"]          # [1024, 768] rows n=(i,j), cols q|k|v
    relh = ins["relh"]    # [63, 32]
    relw = ins["relw"]    # [63, 32]
    out = outs["out"]     # [1024, 256]

    with (
        tc.tile_pool(name="persist", bufs=1) as persist,
        tc.tile_pool(name="expw", bufs=4) as expwp,
        tc.tile_pool(name="qbfp", bufs=2) as qbfp,
        tc.tile_pool(name="av2p", bufs=2) as av2p,
        tc.tile_pool(name="stage", bufs=8) as stagep,
        tc.tile_pool(name="psum_log", bufs=2, space="PSUM") as pslog,
        tc.tile_pool(name="psum_scr", bufs=2, space="PSUM") as psscr,
        tc.tile_pool(name="psum_att", bufs=1, space="PSUM") as psattp,
    ):
        # attn@V accumulator: one 2-bank region, manually double-buffered by
        # partition slot (head h -> rows (h%2)*64 + 0:33). The odd slot also
        # hosts the p-state warm-up dummies and heads 0/1's bias matmuls
        # (its first real use, head 1's attn@V, comes ~20us in).
        ps_att = psattp.tile([128, N], F32, tag="att")

        # ---------------- warm-up + constants ----------------
        dummy_sb = persist.tile([128, 64], F32)
        nc.vector.memset(dummy_sb, 0.0)
        # sized so the warm-up ends as the first row DMA lands (~3.9us):
        # f32 runs 4 cycles/row, so keep the free dim small or the dummies
        # block the real transposes in the in-order PE queue
        for w in range(20):
            nc.tensor.matmul(ps_att[64:96, 0:32], lhsT=dummy_sb[:, 0:32],
                             rhs=dummy_sb[:, 0:32], start=True, stop=True)

        ident = persist.tile([128, 128], F32)
        make_identity(nc, ident)
        # f32r operands must come from f32r-rounding writes (BIR verifier);
        # a DVE copy provides the rounded replica used by all transposes
        identR = persist.tile([128, 128], F32R)
        nc.vector.tensor_copy(identR, ident)
        ident_marker = nc.gpsimd.tensor_copy(ident[0:1, 0:1], ident[0:1, 0:1])

        # rel tables + all non-critical loads on the Act DMA queue; SP
        # carries only the head-0-critical row loads and output stores.
        rel_st = persist.tile([64, 63], F32R)
        nc.scalar.dma_start(out=rel_st[0:32],
                            in_=relw.rearrange("u d -> d u").bitcast(F32R))
        nc.scalar.dma_start(out=rel_st[32:64],
                            in_=relh.rearrange("u d -> d u").bitcast(F32R))

        rowsR = persist.tile([128, 4, 8, 128], F32R)
        CB_COLS = (0, 256, 128, 384)   # q0, k0, q1, k1

        def emit_rows_dma(cb, eng, lo=0, hi=8):
            src = bass.AP(tensor=x.tensor,
                          offset=CB_COLS[cb] + lo * 128 * 768,
                          ap=[[768, 128], [128 * 768, hi - lo], [1, 128]],
                          ).bitcast(F32R)
            eng.dma_start(out=rowsR[:, cb, lo:hi, :], in_=src)

        emit_rows_dma(0, nc.sync, 0, 4)
        emit_rows_dma(0, nc.sync, 4, 8)
        emit_rows_dma(1, nc.sync)
        emit_rows_dma(2, nc.scalar)    # q-half1
        emit_rows_dma(3, nc.scalar)    # k-half1

        # The Tile scheduler orders each engine's queue with its own internal
        # timing model, which disagrees with the device timing enough to
        # shuffle the Pool queue badly (observed: kaug0 scheduled after five
        # unrelated 1.5us Pool copies, gating the first logits by ~8us).
        # Chain every Pool op (and the startup DVE ops) in emission order
        # with same-engine ordering-only deps.
        _chain = {"pool": ident_marker}

        def chained(engine_name, inst):
            prev = _chain.get(engine_name)
            if prev is not None:
                add_dep_helper(inst.ins, prev.ins, sync=False,
                               reason=f"pin {engine_name} queue order")
            _chain[engine_name] = inst
            return inst

        def pool(op, *args, **kwargs):
            return chained("pool", getattr(nc.gpsimd, op)(*args, **kwargs))

        # onehot for K_aug rows 32:64 (Pool; ka[0]'s copy early for head 0,
        # the rest fed into the chain later, outside the critical window)
        oh_st = persist.tile([32, 8, 128], F32)
        pool("memset", oh_st, 0.0)
        oh = oh_st.rearrange("t j (b m) -> t j b m", b=4)
        pool("affine_select",
             out=oh, in_=oh, compare_op=mybir.AluOpType.not_equal,
             fill=1.0, base=0, pattern=[[-4, 8], [-1, 4], [0, 32]],
             channel_multiplier=1)
        ones_st = persist.tile([128, 64], F32)
        pool("memset", ones_st, 1.0)

        rel_bf = persist.tile([64, 63], BF16)

        ka = [persist.tile([64, 8, 128], F32R, tag=f"ka{i}", name=f"ka{i}")
              for i in range(4)]
        pool("tensor_copy", ka[0][32:64], oh_st)
        pool("tensor_copy", rel_bf, rel_st)

        # qT/kT: [p, half, n]; partitions = 32*(h%4)+d within a half
        qT = persist.tile([128, 2, N], F32R)
        kT = persist.tile([128, 2, N], F32R)

        def emit_transposes(cb, dst, half, lo=None, pin=False,
                            pe_anchor=None):
            # 4 transposes into one scratch bank + a single wide copy: the
            # copy's fixed PSUM-access cost is paid once per 4 tiles
            groups = (0, 4) if lo is None else (lo,)
            for g in groups:
                pt = psscr.tile([128, 512], F32R, tag="scr", name=f"pt{cb}_{g}")
                for c in range(4):
                    tr = nc.tensor.transpose(pt[:, c * 128:(c + 1) * 128],
                                             rowsR[:, cb, g + c, :], identR)
                    if pe_anchor is not None:
                        add_dep_helper(tr.ins, pe_anchor.ins, sync=False,
                                       reason="filler after this j's attn@V")
                cp = nc.vector.tensor_copy(
                    dst[:, half, g * 128:(g + 4) * 128], pt)
                if pin:
                    chained("dve", cp)

        # ---------------- per-head q staging + shifted-window bias -------
        qaug_all = persist.tile([64, NH, N], F32R)

        def emit_bias_stage_a(h, pin_dve=False):
            """bf16 staging [q^T ; q^T row-permuted] + f32r q^T for logits."""
            lane = (h % 4) * 32
            qsT = qT[lane:lane + 32, h // 4, :]
            qbf = qbfp.tile([64, N], BF16, tag="qbf", name=f"qbf{h}")
            cv = nc.vector.tensor_copy(qbf[0:32, :], qsT)
            if pin_dve:
                chained("dve", cv)
            pool("tensor_copy",
                 qbf[32:64, :].rearrange("d (i j) -> d i j", i=32),
                 qsT.rearrange("d (i j) -> d j i", i=32, j=32))
            pool("tensor_copy", qaug_all[0:32, h, :], qsT)
            return qbf

        def emit_bias_stage_b(h, qbf, ps_bs, pin_dve=False,
                              pe_anchor=None):
            """biasT[t, (i,j)] = SWT[t+31-i, (i,j)]: per i-block matmul with
            a shifted rel^T window as the stationary operand."""
            for half in range(2):
                ps_b = ps_bs[half]
                for ib in range(16):
                    i = half * 16 + ib
                    mm = nc.tensor.matmul(
                        ps_b[:, ib * 32:(ib + 1) * 32],
                        lhsT=rel_bf[:, 31 - i:63 - i],
                        rhs=qbf[:, i * 32:(i + 1) * 32],
                        start=True, stop=True)
                    if pe_anchor is not None:
                        add_dep_helper(mm.ins, pe_anchor.ins, sync=False,
                                       reason="filler after this j's attn@V")
                cp = nc.vector.tensor_copy(
                    qaug_all[32:64, h, half * 512:(half + 1) * 512], ps_b)
                if pin_dve:
                    chained("dve", cp)

        def emit_bias(h, pin_dve=True, pe_anchor=None):
            qbf = emit_bias_stage_a(h, pin_dve)
            ps_bs = [psscr.tile([32, 512], F32, tag="scr", name=f"ps_b{h}_{x}")
                     for x in range(2)]
            emit_bias_stage_b(h, qbf, ps_bs, pin_dve, pe_anchor)

        def emit_kaug(h, split=False):
            lane = (h % 4) * 32
            ksT = kT[lane:lane + 32, h // 4, :]
            if split:
                # j0/j1 slice on the (faster) DVE chain so the first logits
                # unblock early; the rest follows on Pool
                chained("dve", nc.vector.tensor_copy(
                    ka[h % 4][0:32, 0:2, :].rearrange("d j m -> d (j m)"),
                    ksT[:, 0:256]))
                pool("tensor_copy",
                     ka[h % 4][0:32, 2:8, :].rearrange("d j m -> d (j m)"),
                     ksT[:, 256:1024])
            else:
                pool("tensor_copy",
                     ka[h % 4][0:32].rearrange("d j m -> d (j m)"), ksT)

        def emit_onehot(i):
            cp = nc.vector.tensor_copy(ka[i][32:64], oh_st)
            tail = _chain.get("dve_startup_tail")
            if tail is not None:
                add_dep_helper(cp.ins, tail.ins, sync=False,
                               reason="onehots after startup DVE chain")

        # ---------------- startup emission order ----------------
        # Strictly head-0-critical work before the loop; head 0's bias runs
        # in the then-unused odd attn@V PSUM slot so the two scratch banks
        # stay free for the q0/k0 transposes. Everything else (bias/kaug/
        # onehot for later heads) is fed into the j-loop hooks.
        emit_transposes(0, qT, 0, pin=True)      # q-half0
        qbf0 = emit_bias_stage_a(0, pin_dve=True)
        # k-half0: head 0's K_aug rows are copied straight from the
        # transpose PSUM tiles (rows 0:32 = lane 0), so the first logits
        # do not wait for the full kT staging chain; the kT copies for
        # heads 1-3 trail at the chain tail.
        k0pts = []
        for g in (0, 4):
            pt = psscr.tile([128, 512], F32R, tag="scr", name=f"ptk0_{g}")
            for c in range(4):
                nc.tensor.transpose(pt[:, c * 128:(c + 1) * 128],
                                    rowsR[:, 1, g + c, :], identR)
            k0pts.append(pt)
        chained("dve", nc.vector.tensor_copy(
            ka[0][0:32, 0:4, :].rearrange("d j m -> d (j m)"), k0pts[0][0:32]))
        emit_bias_stage_b(0, qbf0, [ps_att[64:96, 0:512],
                                    ps_att[64:96, 512:1024]], pin_dve=True)
        chained("dve", nc.vector.tensor_copy(
            ka[0][0:32, 4:8, :].rearrange("d j m -> d (j m)"), k0pts[1][0:32]))
        for g in (0, 1):
            chained("dve", nc.vector.tensor_copy(
                kT[:, 0, g * 512:(g + 1) * 512], k0pts[g]))
        _chain["dve_startup_tail"] = _chain["dve"]

        # v chunks staged in f32 (contiguous DMAs on the Act queue) and
        # converted to bf16 on Pool: the attn@V matmul accumulates into a
        # partition-offset PSUM slot, which the ISA allows for bf16 but not
        # f32r operands. bf16 exp-weights/v cost ~1e-3 relative error.
        v_st = persist.tile([128, 8, 256], F32)
        v_aug = persist.tile([128, NH, 8, 33], BF16)
        for j in range(8):
            nc.scalar.dma_start(
                out=v_st[:, j, :], in_=x[j * 128:(j + 1) * 128, 512:768])

        def emit_vconv(j):
            cp = nc.vector.tensor_copy(
                v_aug[:, :, j, 0:32],
                v_st[:, j, :].rearrange("p (h d) -> p h d", h=NH))
            tail = _chain.get("dve_startup_tail")
            if tail is not None:
                add_dep_helper(cp.ins, tail.ins, sync=False,
                               reason="v staging after startup DVE chain")

        emit_vconv(0)
        nc.vector.tensor_copy(
            v_aug[:, :, :, 32:33].rearrange("p h j o -> p (h j o)"), ones_st)
        emit_vconv(1)
        emit_vconv(2)

        # ---------------- output staging / flush ----------------
        out_sb = persist.tile([128, 8, 256], F32)

        def emit_av2_copy(h, av2t, csl=slice(0, N)):
            s = (h % 2) * 64
            nc.vector.tensor_copy(av2t[0:33, csl], ps_att[s:s + 33, csl])

        def flush_head(h, av2t, nts, tail=False):
            groups = {3: 4, 7: 4} if tail else {7: 8}
            for nt in nts:
                csl = slice(nt * 128, (nt + 1) * 128)
                # on the tail, borrow the (by then idle) logits PSUM banks so
                # four transposes can be in flight instead of two
                pool_ = pslog if (tail and nt % 2) else psscr
                tag = "log" if (tail and nt % 2) else "scr"
                ps_t = pool_.tile([128, 33], F32, tag=tag, name=f"ps_t{h}_{nt}")
                nc.tensor.transpose(ps_t, av2t[0:33, csl], ident[0:33, 0:33])
                rec = stagep.tile([128, 1], F32, tag="rec")
                nc.vector.reciprocal(rec, ps_t[:, 32:33])
                nc.vector.tensor_scalar_mul(
                    out_sb[:, nt, h * 32:(h + 1) * 32], ps_t[:, 0:32], rec)
                if nt in groups:
                    group = groups[nt]
                    g = nt - group + 1
                    dstap = bass.AP(
                        tensor=out.tensor,
                        offset=g * 128 * 256 + h * 32,
                        ap=[[256, 128], [128 * 256, group], [1, 32]])
                    nc.sync.dma_start(
                        out=dstap, in_=out_sb[:, g:nt + 1, h * 32:(h + 1) * 32])

        # ---------------- per-head pipeline ----------------
        # early-head hooks: later heads' staging spread across the j-loops
        # so nothing competes with the first heads' critical chains
        HOOKS = {
            (0, 2): lambda a: (emit_vconv(3), emit_vconv(4),
                               emit_transposes(2, qT, 1, 0, pin=True,
                                               pe_anchor=a)),
            (0, 3): lambda a: (emit_bias(1, pin_dve=True, pe_anchor=a),
                               emit_transposes(2, qT, 1, 4, pin=True,
                                               pe_anchor=a)),
            (0, 4): lambda a: (emit_vconv(5), emit_vconv(6),
                               emit_vconv(7), emit_kaug(1), emit_onehot(1)),
            (0, 5): lambda a: (emit_bias(2, pe_anchor=a),
                               emit_transposes(3, kT, 1, 0, pin=True,
                                               pe_anchor=a)),
            (0, 6): lambda a: emit_transposes(3, kT, 1, 4, pin=True,
                                              pe_anchor=a),
            (1, 1): lambda a: (emit_kaug(2), emit_onehot(2)),
            (1, 3): lambda a: (emit_kaug(3), emit_onehot(3), emit_kaug(4)),
            (2, 1): lambda a: emit_kaug(5),
            (3, 1): lambda a: emit_kaug(6),
            (4, 1): lambda a: emit_kaug(7),
        }
        pending = None    # (head, av2t) awaiting transpose+normalize

        def emit_logits(h, j):
            qaug = qaug_all[:, h, :]
            ps_l = pslog.tile([128, N], F32, tag="log", name=f"ps_l{h}_{j}")
            for half in range(2):
                sl = slice(half * 512, (half + 1) * 512)
                nc.tensor.matmul(
                    ps_l[:, sl], lhsT=ka[h % 4][:, j, :],
                    rhs=qaug[:, sl], start=True, stop=True)
            return ps_l

        for h in range(NH):
            s = (h % 2) * 64
            for j in range(8):
                # logits for j are emitted at the end of iteration j-1 so
                # hook fillers can never precede them in the engine queues
                if j == 0:
                    ps_l = emit_logits(h, 0)
                ew = expwp.tile([128, N], BF16, tag="ew", name=f"ew{h}_{j}")
                nc.scalar.activation(ew, ps_l, AF.Exp, scale=SCALE)
                for half in range(2):
                    sl = slice(half * 512, (half + 1) * 512)
                    a_last = nc.tensor.matmul(
                        ps_att[s:s + 33, sl], lhsT=v_aug[:, h, j, :],
                        rhs=ew[:, sl], start=(j == 0), stop=(j == 7))
                if j < 7:
                    ps_l = emit_logits(h, j + 1)
                elif h + 1 < NH:
                    ps_l = None   # next head's j0 emitted at its loop start
                if (h, j) in HOOKS:
                    HOOKS[(h, j)](a_last)
                if j == 1 and pending is not None:
                    flush_head(pending[0], pending[1], range(8))
                    pending = None
                if j == 6 and h >= 1 and h + 2 < NH:
                    emit_bias(h + 2)

            av2t = av2p.tile([33, N], F32, tag="av2", name=f"av2_{h}")
            if h < NH - 1:
                emit_av2_copy(h, av2t)
            else:
                # tail: quarter-split the last copy so the flush transposes
                # start as soon as the first columns land
                for qq in range(4):
                    emit_av2_copy(h, av2t, slice(qq * 256, (qq + 1) * 256))
            pending = (h, av2t)

        # tail flush of the last head
        flush_head(pending[0], pending[1], range(4), tail=True)
        flush_head(pending[0], pending[1], range(4, 8), tail=True)


_NC_CACHE = {}


def _build():
    if "nc" in _NC_CACHE:
        return _NC_CACHE["nc"]
    nc = bass.Bass("TRN2", target_bir_lowering=False, debug=False,
                   enable_asserts=True, num_devices=8)
    ins = {
        "x": nc.dram_tensor("x", [N, 768], F32, kind="ExternalInput").ap(),
        "relh": nc.dram_tensor("relh", [63, 32], F32, kind="ExternalInput").ap(),
        "relw": nc.dram_tensor("relw", [63, 32], F32, kind="ExternalInput").ap(),
    }
    outs = {
        "out": nc.dram_tensor("out", [N, 256], F32, kind="ExternalOutput").ap(),
    }
    with tile.TileContext(nc) as tc:
        kernel_body(tc, outs, ins)
    split_multiwaits(nc)
    _NC_CACHE["nc"] = nc
    return nc


def kernel(inputs, key_rel_h, key_rel_w, _trace=False):
    nc = _build()
    x = np.ascontiguousarray(np.asarray(inputs, dtype=np.float32).reshape(8, N, 768))
    rh = np.ascontiguousarray(np.asarray(key_rel_h, dtype=np.float32))
    rw = np.ascontiguousarray(np.asarray(key_rel_w, dtype=np.float32))
    in_maps = [{"x": x[c], "relh": rh, "relw": rw} for c in range(8)]
    res = bass_utils.run_bass_kernel_spmd(
        nc, in_maps, core_ids=list(range(8)), trace=_trace)
    outp = np.stack([r["out"] for r in res.results])
    if _trace:
        kernel.last_results = res
    return outp.reshape(8, 32, 32, 256)



# revision 38
# speedup vs baseline: 1.0297x; 1.0297x over previous
"""AttentionAugmentation2D Trainium2 kernel (v6).

Shapes (hardcoded): B=8, H=W=32, N=1024, NH=8 heads, dk=dv=32 per head.
inputs [8,32,32,768] = q|k|v (256 each), key_rel_h/w [63,32].
Sharding: data-parallel over batch B across the 8 cores.

Math per (batch, head), n=(i,j), m=(i',j') (i = H index):
  logits[n,m] = q[n]@k[m] + q[(j,i)]@rel_h[i'-i+31] + q[(i,j)]@rel_w[i'-i+31]
Both rel terms depend on m only through i', so with
  SWT[u,n] = rel_w[u]@q[(i,j)] + rel_h[u]@q[(j,i)]        (u in [0,63))
  biasT[t,n] = SWT[t+31-i(n), n]                          (shifted windows)
we get  logits^T = K_aug^T.T @ Q_augT  with contraction 64:
  K_aug rows: 0:32 = k^T, 32:64 = onehot[t==i'(m)] ;  Q_aug: [q^T; biasT].

v6 redesign vs v5 (cost-model driven):
 - All PE operands are bf16 (error budget 2e-2 is ~10x away): packed
   bf16 SBUF-to-SBUF DVE copies run in 4x perf mode.
 - attn@V swaps operand roles: the exp-weight chunk ew[:, nt*128:...]
   is the *stationary* matmul operand and v (32 cols + a ones column
   for the softmax row-sums) streams, so each matmul charges only 33
   rows instead of 512.  The output lands n-major, which kills v5's 64
   flush transposes and attn^T staging copies; accumulation uses 8
   sub-bank [128,33] regions spaced 64 cols apart in ONE psum bank,
   single-buffered across heads (each head's epilogue completes before
   the next head's first accumulating matmul needs the regions).
 - biasT is built by TWO accumulating matmuls per i-block (relw-window
   x a q^T block, then relh-window x a stride-32 column view of q^T
   that realizes the (i,j)->(j,i) permutation for free), so no
   permuted-q replica and no separate bias-rhs tile exist; both rhs
   reads come straight from qaug rows 0:32.
 - The softmax exp is split between the Act engine (activation Exp)
   and the Pool engine: gpsimd supports elementwise pow, so
   exp(s*x) = pow(e^s, x) with a stride-0 broadcast base.  gpsimd
   cannot read PSUM, so DVE stages the offloaded logit tiles
   PSUM->SBUF; the offload count balances Act against DVE+Pool.
 - Per-head epilogue: DVE reciprocal over the 8 strided ones-sums,
   one broadcast tensor_tensor multiply normalizing all 8 regions into
   the n-major staging tile, one strided DMA per head.
 - PSUM map: banks 0-5 = one [128,6,512] region manually slotted in
   512-col halves: pair (0,1) serves the Pool-offloaded logit tiles
   plus the startup/half-1 transposes; pairs (2,3)/(4,5) alternate
   (continuously across heads) for the Act-exp'd tiles.  Bank 6 =
   attn regions; bank 7 = bias scratch (halves at partitions 0:32 /
   32:64, warm-up dummies at 64:96).
 - rel tables are DMA'd in natural [63,32] layout (a transposed DMA
   would cost ~2000 descriptors on the single shared DMA stream, ahead
   of the critical q/k row loads) and transposed on the PE instead.

Toolchain note: walrus codegen only fits ONE semaphore wait in most
TPB instruction structs; split_multiwaits() moves excess waits onto
same-engine InstNoOp carriers (same workaround as v5).
"""

import numpy as np

import concourse.bass as bass
import concourse.mybir as mybir
import concourse.tile as tile
from concourse import bass_utils
from concourse.masks import make_identity
from concourse.tile import add_dep_helper

F32 = mybir.dt.float32
F32R = mybir.dt.float32r
BF16 = mybir.dt.bfloat16
AF = mybir.ActivationFunctionType

NH = 8
N = 1024
DK = 32
SCALE = float(DK) ** -0.5
BASE = float(np.exp(SCALE))

# Per-head tuple of js whose exp runs on Pool (via DVE PSUM->SBUF stage).
# Must be a subset of {0, 3} (those js own psum slot pair (0,1)); their
# attn@V is deferred to head end.
OFFLOAD = {h: (0,) for h in range(NH)}

KMARKS = []   # (inst_name, label) for trace debugging


def split_multiwaits(nc, dma_limit=1):
    """Move excess semaphore waits onto same-engine nop carriers."""
    n_new = 0
    for f in nc.m.functions:
        for blk in f.blocks:
            newlist = []
            for inst in blk.instructions:
                si = getattr(inst, "sync_info", None)
                is_dma = isinstance(inst, mybir.InstDMACopy)
                limit = dma_limit if is_dma else 1
                if si is not None and len(si.on_wait) > limit:
                    waits = list(si.on_wait)
                    for w in waits[:-1]:
                        n_new += 1
                        newlist.append(mybir.InstNoOp(
                            name=f"I-wc{n_new}",
                            ins=[], outs=[],
                            sync_info=mybir.SyncInfo(on_wait=[w], on_update=[]),
                            bass_nofuse=True,
                            engine=inst.engine,
                        ))
                    inst.sync_info = mybir.SyncInfo(
                        on_wait=waits[-1:], on_update=si.on_update)
                newlist.append(inst)
            blk.instructions = newlist
    return n_new


def kernel_body(tc, outs, ins):
    nc = tc.nc
    x = ins["x"]          # [1024, 768] rows n=(i,j), cols q|k|v
    relh = ins["relh"]    # [63, 32]
    relw = ins["relw"]    # [63, 32]
    out = outs["out"]     # [1024, 256]

    with (
        tc.tile_pool(name="persist", bufs=1) as persist,
        tc.tile_pool(name="expw", bufs=10) as expwp,
        tc.tile_pool(name="qaug", bufs=3) as qaugp,
        tc.tile_pool(name="lstg", bufs=3) as lstgp,
        tc.tile_pool(name="psmain", bufs=1, space="PSUM") as psmain,
    ):
        # ---- PSUM map ----
        ps_all = psmain.tile([128, 8, 512], F32, tag="ps")
        ps_flat = ps_all.rearrange("p s c -> p (s c)")

        def slot(s, n=1):
            return ps_flat[:, s * 512:(s + n) * 512]
        ps_att = slot(6)
        ps_bias = slot(7)

        # ---------------- DMAs (the DMA stream is serial in practice:
        # critical q rows first, tiny rel loads sandwiched, v last) ----
        rowsR = persist.tile([128, 4, 8, 128], F32R)
        CB_COLS = (0, 256, 128, 384)   # q0, k0, q1, k1

        def emit_rows_dma(cb, eng, lo=0, hi=8):
            src = bass.AP(tensor=x.tensor,
                          offset=CB_COLS[cb] + lo * 128 * 768,
                          ap=[[768, 128], [128 * 768, hi - lo], [1, 128]],
                          ).bitcast(F32R)
            eng.dma_start(out=rowsR[:, cb, lo:hi, :], in_=src)

        rel_nat = persist.tile([64, 64], F32R)
        nc.vector.memset(rel_nat.bitcast(F32), 0.0)
        v_st = persist.tile([128, 8, 256], F32)

        def emit_v_dma(j):
            nc.scalar.dma_start(
                out=v_st[:, j, :], in_=x[j * 128:(j + 1) * 128, 512:768])

        # the HWDGE serves the SP and Act queues round-robin: interleave
        # so the critical loads (q, rel, k, early v) land in order
        emit_rows_dma(0, nc.sync, 0, 4)          # q half0 lo
        nc.scalar.dma_start(out=rel_nat[0:63, 0:32], in_=relw.bitcast(F32R))
        emit_rows_dma(0, nc.sync, 4, 8)          # q half0 hi
        nc.scalar.dma_start(out=rel_nat[0:63, 32:64], in_=relh.bitcast(F32R))
        emit_rows_dma(1, nc.sync, 0, 4)          # k half0 lo
        emit_v_dma(0)
        emit_rows_dma(1, nc.sync, 4, 8)          # k half0 hi
        emit_v_dma(1)
        emit_v_dma(2)
        emit_rows_dma(2, nc.scalar)              # q half1
        emit_rows_dma(3, nc.scalar)              # k half1
        for j in range(3, 8):
            emit_v_dma(j)

        # ---------------- warm-up + constants ----------------
        _chain = {}
        dummy_sb = persist.tile([128, 64], F32)
        nc.vector.memset(dummy_sb, 0.0)
        for w in range(20):
            _dm = nc.tensor.matmul(ps_bias[64:96, 0:32],
                                   lhsT=dummy_sb[:, 0:32],
                                   rhs=dummy_sb[:, 0:32],
                                   start=True, stop=True)
            _chain.setdefault("pe", _dm)
            if _chain["pe"] is not _dm:
                add_dep_helper(_dm.ins, _chain["pe"].ins, sync=False,
                               reason="pin pe queue order")
            _chain["pe"] = _dm

        zeros_bf = persist.tile([128, 128], BF16)
        nc.vector.memset(zeros_bf, 0.0)
        ident = persist.tile([128, 128], F32)
        make_identity(nc, ident)
        identR = persist.tile([128, 128], F32R)
        nc.vector.tensor_copy(identR, ident)
        ident_marker = nc.gpsimd.tensor_copy(ident[0:1, 0:1], ident[0:1, 0:1])

        # pow base for the Pool exp share: pow(e^s, x) = exp(s*x)
        base_t = persist.tile([128, 1], F32)
        nc.vector.memset(base_t, BASE)

        def base_bcast(cols):
            return bass.AP(tensor=base_t.tensor, offset=base_t.offset,
                           ap=[list(base_t.ap[0]), [0, cols]])

        # Pin Pool and startup-DVE queue order (the Tile scheduler's
        # internal timing model reorders engine queues badly otherwise).
        _chain["pool"] = ident_marker

        def chained(engine_name, inst):
            prev = _chain.get(engine_name)
            if prev is not None:
                add_dep_helper(inst.ins, prev.ins, sync=False,
                               reason=f"pin {engine_name} queue order")
            _chain[engine_name] = inst
            return inst

        def pool(op, *args, **kwargs):
            return chained("pool", getattr(nc.gpsimd, op)(*args, **kwargs))

        def pe(inst):
            return chained("pe", inst)

        # onehot rows for K_aug (bf16) -- no input deps, head of Pool chain
        oh_st = persist.tile([32, 8, 128], F32)
        pool("memset", oh_st, 0.0)
        oh = oh_st.rearrange("t j (b m) -> t j b m", b=4)
        pool("affine_select",
             out=oh, in_=oh, compare_op=mybir.AluOpType.not_equal,
             fill=1.0, base=0, pattern=[[-4, 8], [-1, 4], [0, 32]],
             channel_multiplier=1)
        oh_bf = persist.tile([32, 8, 128], BF16)
        pool("tensor_copy", oh_bf, oh_st)

        # bf16 transposed replicas of q and k: [p=32*(h%4)+d, half, n]
        qT = persist.tile([128, 2, N], BF16)
        kT = persist.tile([128, 2, N], BF16)

        ka = [persist.tile([64, 8, 128], BF16, tag=f"ka{i}", name=f"ka{i}")
              for i in range(4)]

        # rel^T: [32, 2, 63] at partitions 0:32 (PE operands must share a
        # base partition): [:,0,:] = relw^T, [:,1,:] = relh^T
        rel_bf = persist.tile([32, 2, 63], BF16)

        def emit_transposes(cb, dst, half, lo, s, pin=False, pe_anchor=None):
            """4 transposes into one psum slot + one wide copy into the
            bf16 replica."""
            pt = slot(s).bitcast(F32R)
            for c in range(4):
                tr = pe(nc.tensor.transpose(pt[:, c * 128:(c + 1) * 128],
                                            rowsR[:, cb, lo + c, :], identR))
            cp = nc.vector.tensor_copy(
                dst[:, half, lo * 128:(lo + 4) * 128], slot(s))
            if pin:
                chained("dve", cp)

        # ---------------- per-head staging ----------------
        def emit_qstage(h, pin=False):
            """qaug rows 0:32 = q^T for head h (bf16 4x copy)."""
            lane = (h % 4) * 32
            qsT = qT[lane:lane + 32, h // 4, :]
            qaug = qaugp.tile([64, N], BF16, tag="qaug", name=f"qaug{h}")
            c1 = nc.vector.tensor_copy(qaug[0:32, :], qsT)
            if pin:
                chained("dve", c1)
            return qaug

        def emit_bias(h, qaug, halves=(0, 1), pin=False, pe_anchor=None):
            """biasT[t,(i,j)] = SWT[t+31-i,(i,j)]: per i-block, two
            accumulating matmuls with shifted rel windows; the relh term
            reads q^T through a stride-32 column view (the (i,j)->(j,i)
            permutation).  psum bank 7, halves at partitions 0:32/32:64."""
            for half in halves:
                ps_b = ps_bias[half * 32:half * 32 + 32, :]
                for ib in range(16):
                    i = half * 16 + ib
                    pe(nc.tensor.matmul(
                        ps_b[:, ib * 32:(ib + 1) * 32],
                        lhsT=rel_bf[:, 0, 31 - i:63 - i],
                        rhs=qaug[0:32, i * 32:(i + 1) * 32],
                        start=True, stop=False))
                    perm_rhs = bass.AP(
                        tensor=qaug.tensor, offset=qaug.offset + i,
                        ap=[[qaug.ap[0][0], 32], [32, 32]])
                    pe(nc.tensor.matmul(
                        ps_b[:, ib * 32:(ib + 1) * 32],
                        lhsT=rel_bf[:, 1, 31 - i:63 - i],
                        rhs=perm_rhs,
                        start=False, stop=True))
                cp = nc.vector.tensor_copy(
                    qaug[32:64, half * 512:(half + 1) * 512], ps_b)
                if pin:
                    chained("dve", cp)

        def emit_kaug(h, pin=False, los=(0, 8)):
            lane = (h % 4) * 32
            lo, hi = los
            ksT = kT[lane:lane + 32, h // 4, lo * 128:hi * 128]
            cp = nc.vector.tensor_copy(
                ka[h % 4][0:32, lo:hi].rearrange("d j m -> d (j m)"), ksT)
            if pin:
                chained("dve", cp)

        def emit_kaug_oh(h, pin=False):
            if h < 4:
                co = nc.vector.tensor_copy(ka[h % 4][32:64], oh_bf)
                if pin:
                    chained("dve", co)

        # ---------------- v staging ----------------
        v_aug = persist.tile([128, NH, 8, 33], BF16)
        ones_st = persist.tile([128, 64], F32)
        nc.vector.memset(ones_st, 1.0)

        def emit_vconv(j):
            pool("tensor_copy",
                 v_aug[:, :, j, 0:32],
                 v_st[:, j, :].rearrange("p (h d) -> p h d", h=NH))

        nc.vector.tensor_copy(
            v_aug[:, :, :, 32:33].rearrange("p h j o -> p (h j o)"), ones_st)

        # ---------------- epilogue ----------------
        out_sb = persist.tile([128, 8, 256], F32)
        rec_t = persist.tile([128, NH, 8], F32)

        def emit_epilogue(h):
            """reciprocal of the 8 ones-sums + one broadcast normalize of
            the 8 [128,33] regions into out_sb; one strided DMA."""
            rec = rec_t[:, h, :]
            sums_ap = bass.AP(tensor=ps_att.tensor, offset=ps_att.offset + 32,
                              ap=[list(ps_att.ap[0]), [64, 8]])
            chained("dve", nc.vector.reciprocal(rec, sums_ap))
            in0 = bass.AP(tensor=ps_att.tensor, offset=ps_att.offset,
                          ap=[list(ps_att.ap[0]), [64, 8], [1, 32]])
            in1 = bass.AP(tensor=rec.tensor, offset=rec.offset,
                          ap=[list(rec.ap[0]), [1, 8], [0, 32]])
            out_ap = bass.AP(tensor=out_sb.tensor,
                             offset=out_sb.offset + h * 32,
                             ap=[list(out_sb.ap[0]), [256, 8], [1, 32]])
            chained("dve", nc.vector.tensor_tensor(
                out=out_ap, in0=in0, in1=in1, op=mybir.AluOpType.mult))
            groups = ((0, 8),)
            for glo, ghi in groups:
                dstap = bass.AP(
                    tensor=out.tensor,
                    offset=glo * 128 * 256 + h * 32,
                    ap=[[256, 128], [128 * 256, ghi - glo], [1, 32]])
                nc.sync.dma_start(out=dstap,
                                  in_=out_sb[:, glo:ghi, h * 32:(h + 1) * 32])

        # ---------------- startup ----------------
        # PE order: dummies, rel-w transpose, q transposes, rel-h
        # transpose, k transposes, bias mms, logits.  The DVE chain IS
        # the head-0 critical path: qaug0 and ka0 rows are copied
        # straight out of the transpose psum slots (lane 0); the qT
        # replica copies (for later heads) trail behind and delay only
        # head 0's j0, whose exp is Pool-offloaded and slack-tolerant.
        # Slot use: q-lo->0, q-hi->1, rel->4 (cols 0:127), k-lo->2,
        # k-hi->5; slot 3 stays free for j1's logits.
        relT = slot(4).bitcast(F32R)
        pe(nc.tensor.transpose(relT[0:32, 0:64],
                               rel_nat[:, 0:32], identR[0:64, 0:64]))

        def transp4(cb, lo, s):
            pt = slot(s).bitcast(F32R)
            for c in range(4):
                pe(nc.tensor.transpose(pt[:, c * 128:(c + 1) * 128],
                                       rowsR[:, cb, lo + c, :], identR))

        transp4(0, 0, 0)                                # q half0 lo
        pe(nc.tensor.transpose(relT[0:32, 64:128],
                               rel_nat[:, 32:64], identR[0:64, 0:64]))
        transp4(0, 4, 1)                                # q half0 hi
        transp4(1, 0, 2)                                # k half0 lo

        chained("dve", nc.vector.tensor_copy(
            rel_bf.rearrange("p a u -> p (a u)"),
            bass.AP(tensor=ps_flat.tensor,
                    offset=ps_flat.offset + 4 * 512,
                    ap=[[ps_flat.ap[0][0], 32], [64, 2], [1, 63]])))
        qaug_h = {0: qaugp.tile([64, N], BF16, tag="qaug", name="qaug0")}
        chained("dve", nc.vector.tensor_copy(
            qaug_h[0][0:32, 0:512], slot(0)[0:32, :]))
        chained("dve", nc.vector.tensor_copy(
            qaug_h[0][0:32, 512:1024], slot(1)[0:32, :]))
        chained("dve", nc.vector.tensor_copy(
            ka[0][0:32, 0:4].rearrange("d j m -> d (j m)"),
            slot(2)[0:32, :]))
        emit_kaug_oh(0, pin=True)
        emit_bias(0, qaug_h[0], halves=(0,), pin=True)
        emit_bias(0, qaug_h[0], halves=(1,), pin=True)
        transp4(1, 4, 5)                                # k half0 hi
        chained("dve", nc.vector.tensor_copy(
            ka[0][0:32, 4:8].rearrange("d j m -> d (j m)"),
            slot(5)[0:32, :]))
        # trailing (delays only head 0's slack-tolerant j0): qT replica
        chained("dve", nc.vector.tensor_copy(qT[:, 0, 0:512], slot(0)))
        chained("dve", nc.vector.tensor_copy(qT[:, 0, 512:1024], slot(1)))
        for j in range(8):
            emit_vconv(j)

        # ---------------- main pipeline ----------------
        ACT_PAIRS = ((2, 3), (4, 5))
        act_rot = [0]   # continuous pair rotation across heads

        def emit_logits(h, j, lo_slot):
            qaug = qaug_h[h]
            for half in range(2):
                mm = pe(nc.tensor.matmul(
                    slot(lo_slot + half), lhsT=ka[h % 4][:, j, :],
                    rhs=qaug[:, half * 512:(half + 1) * 512],
                    start=True, stop=True))
                KMARKS.append((mm.ins.name, f"logits{h}_{j}_h{half}_s{lo_slot+half}"))

        def hook(h, j, a):
            if h == 0:
                if j == 2:
                    emit_transposes(2, qT, 1, 0, s=0, pin=True)
                elif j == 3:
                    emit_transposes(2, qT, 1, 4, s=1, pin=True)
                    qaug_h[1] = emit_qstage(1, pin=True)
                    emit_bias(1, qaug_h[1], halves=(0,), pin=True)
                elif j == 4:
                    # kT half0 replica: re-transpose (startup slots were
                    # drained into ka0 directly); ka[1] rows come straight
                    # from the re-transpose psum (lane 1 = partitions
                    # 32:64; DVE copies may shift partitions)
                    emit_transposes(1, kT, 0, 0, s=0, pin=True)
                    chained("dve", nc.vector.tensor_copy(
                        ka[1][0:32, 0:4].rearrange("d j m -> d (j m)"),
                        slot(0)[32:64, :]))
                    emit_bias(1, qaug_h[1], halves=(1,), pin=True)
                elif j == 5:
                    emit_transposes(1, kT, 0, 4, s=1, pin=True)
                    chained("dve", nc.vector.tensor_copy(
                        ka[1][0:32, 4:8].rearrange("d j m -> d (j m)"),
                        slot(1)[32:64, :]))
                    emit_kaug_oh(1, pin=True)
                elif j == 6:
                    qaug_h[2] = emit_qstage(2, pin=True)
                    emit_bias(2, qaug_h[2], halves=(0,), pin=True)
                elif j == 7:
                    emit_bias(2, qaug_h[2], halves=(1,), pin=True)
            elif h == 1:
                if j == 1:
                    emit_transposes(3, kT, 1, 0, s=0, pin=True)
                    emit_kaug(2, pin=True)
                    emit_kaug_oh(2, pin=True)
                elif j == 2:
                    qaug_h[3] = emit_qstage(3, pin=True)
                elif j == 4:
                    emit_bias(3, qaug_h[3], halves=(0,), pin=True)
                elif j == 5:
                    emit_transposes(3, kT, 1, 4, s=1, pin=True)
                    emit_bias(3, qaug_h[3], halves=(1,), pin=True)
            else:
                if j == 1 and h + 1 < NH:
                    emit_kaug(h + 1, pin=True)
                    emit_kaug_oh(h + 1, pin=True)
                elif j == 2 and h + 2 < NH:
                    qaug_h[h + 2] = emit_qstage(h + 2, pin=True)
                elif j == 4 and h + 2 < NH:
                    emit_bias(h + 2, qaug_h[h + 2], halves=(0,), pin=True)
                elif j == 5 and h + 2 < NH:
                    emit_bias(h + 2, qaug_h[h + 2], halves=(1,), pin=True)

        def emit_attnv_zero():
            # one full-width start=True matmul resets the attn bank;
            # interleaved per-region start writes clobber each other on HW
            pe(nc.tensor.matmul(ps_att, lhsT=zeros_bf,
                                rhs=qT[:, 0, 0:512],
                                start=True, stop=False))

        def emit_attnv(h, j, ew, start, stop):
            a_last = None
            for nt in range(8):
                a_last = pe(nc.tensor.matmul(
                    ps_att[:, nt * 64:nt * 64 + 33],
                    lhsT=ew[:, nt * 128:(nt + 1) * 128],
                    rhs=v_aug[:, h, j, :],
                    start=start, stop=stop))
            return a_last

        # pending deferred work from head h-1, emitted inside head h's j1
        # iteration (gives the slow offload pipeline extra time before its
        # attn@V could block the in-order PE queue):
        pending = None

        # pair assignment per tile, rotation continuous across heads
        pair_of = {}
        rot = 0
        for h in range(NH):
            for j in range(8):
                if j in OFFLOAD[h]:
                    pair_of[(h, j)] = (0, 1)
                else:
                    pair_of[(h, j)] = ACT_PAIRS[rot % 2]
                    rot += 1

        emitted_logits = set()

        def next_tile(h, j):
            if j < 7:
                return (h, j + 1)
            return (h + 1, 0) if h + 1 < NH else None

        def emit_logits_once(t):
            if t is not None and t not in emitted_logits:
                emitted_logits.add(t)
                emit_logits(t[0], t[1], pair_of[t][0])

        # head-0 priming: j1/j2 logits first (they gate Act); j0 last --
        # its slots are released only by the trailing qT replica copies,
        # and its Pool-exp pipeline has most of the head as slack
        for t in ((0, 1), (0, 2), (0, 0)):
            emit_logits_once(t)
        for h in range(NH):
            off_js = OFFLOAD[h]
            act_js = [j for j in range(8) if j not in off_js]
            last = NH - 1
            first_j = act_js[0]
            last_j = 7 if h == last else 0
            ews = {}
            for j in range(8):
                # this tile's logits were emitted one iteration ago; emit
                # the NEXT tile's logits before this tile's attn@V so the
                # pinned PE queue never waits an exp to issue logits
                if h == last and j == 7:
                    emit_attnv(h, 0, ews[0], False, False)
                if j in off_js:
                    ls = lstgp.tile([128, N], F32, tag="ls",
                                    name=f"ls{h}_{j}")
                    KMARKS.append((chained("dve", nc.vector.tensor_copy(
                        ls, slot(0, 2))).ins.name, f"stage{h}_{j}"))
                    ew = expwp.tile([128, N], BF16, tag="ew",
                                    name=f"ew{h}_{j}")
                    chained("pool", nc.gpsimd.tensor_tensor(
                        out=ew, in0=base_bcast(N), in1=ls,
                        op=mybir.AluOpType.pow))
                    ews[j] = ew
                    emit_logits_once(next_tile(h, j))
                    nt2 = next_tile(h, j)
                    if nt2 is not None:
                        emit_logits_once(next_tile(*nt2))
                else:
                    ew = expwp.tile([128, N], BF16, tag="ew",
                                    name=f"ew{h}_{j}")
                    KMARKS.append((nc.scalar.activation(
                        ew, slot(pair_of[(h, j)][0], 2),
                        AF.Exp, scale=SCALE).ins.name, f"exp{h}_{j}"))
                    ews[j] = ew
                    # two tiles ahead: logits(t+2) only WAR-waits this
                    # exp's pair, giving the chain ~1us of margin
                    emit_logits_once(next_tile(h, j))
                    nt2 = next_tile(h, j)
                    if nt2 is not None:
                        emit_logits_once(next_tile(*nt2))
                    if j == 1 and pending is not None:
                        ph, defs = pending
                        for idx, (pj, pew) in enumerate(defs):
                            emit_attnv(ph, pj, pew, False,
                                       idx == len(defs) - 1)
                        emit_epilogue(ph)
                        pending = None
                    if h == last and j == 7 and 3 in off_js:
                        emit_attnv(h, 3, ews[3], False, False)
                    if j == first_j:
                        emit_attnv_zero()
                    a_last = emit_attnv(h, j, ew, False, j == last_j)
                    hook(h, j, a_last)
            if h != last:
                # all deferred attn@V for Pool-exp'd js punts into head
                # h+1's j1 (so a pow still in flight can never block the
                # in-order PE queue at the head boundary)
                pending = (h, [(j, ews[j]) for j in off_js])

        emit_epilogue(NH - 1)


_NC_CACHE = {}


def _build():
    if "nc" in _NC_CACHE:
        return _NC_CACHE["nc"]
    nc = bass.Bass("TRN2", target_bir_lowering=False, debug=False,
                   enable_asserts=True, num_devices=8)
    ins = {
        "x": nc.dram_tensor("x", [N, 768], F32, kind="ExternalInput").ap(),
        "relh": nc.dram_tensor("relh", [63, 32], F32, kind="ExternalInput").ap(),
        "relw": nc.dram_tensor("relw", [63, 32], F32, kind="ExternalInput").ap(),
    }
    outs = {
        "out": nc.dram_tensor("out", [N, 256], F32, kind="ExternalOutput").ap(),
    }
    with tile.TileContext(nc) as tc:
        kernel_body(tc, outs, ins)
    split_multiwaits(nc)
    _NC_CACHE["nc"] = nc
    return nc


def kernel(inputs, key_rel_h, key_rel_w, _trace=False):
    nc = _build()
    x = np.ascontiguousarray(np.asarray(inputs, dtype=np.float32).reshape(8, N, 768))
    rh = np.ascontiguousarray(np.asarray(key_rel_h, dtype=np.float32))
    rw = np.ascontiguousarray(np.asarray(key_rel_w, dtype=np.float32))
    in_maps = [{"x": x[c], "relh": rh, "relw": rw} for c in range(8)]
    res = bass_utils.run_bass_kernel_spmd(
        nc, in_maps, core_ids=list(range(8)), trace=_trace)
    outp = np.stack([r["out"] for r in res.results])
    if _trace:
        kernel.last_results = res
    return outp.reshape(8, 32, 32, 256)


# revision 43
# speedup vs baseline: 1.0953x; 1.0637x over previous
"""AttentionAugmentation2D Trainium2 kernel (v6).

Shapes (hardcoded): B=8, H=W=32, N=1024, NH=8 heads, dk=dv=32 per head.
inputs [8,32,32,768] = q|k|v (256 each), key_rel_h/w [63,32].
Sharding: data-parallel over batch B across the 8 cores.

Math per (batch, head), n=(i,j), m=(i',j') (i = H index):
  logits[n,m] = q[n]@k[m] + q[(j,i)]@rel_h[i'-i+31] + q[(i,j)]@rel_w[i'-i+31]
Both rel terms depend on m only through i', so with
  SWT[u,n] = rel_w[u]@q[(i,j)] + rel_h[u]@q[(j,i)]        (u in [0,63))
  biasT[t,n] = SWT[t+31-i(n), n]                          (shifted windows)
we get  logits^T = K_aug^T.T @ Q_augT  with contraction 64:
  K_aug rows: 0:32 = k^T, 32:64 = onehot[t==i'(m)] ;  Q_aug: [q^T; biasT].

v6 redesign vs v5 (cost-model driven):
 - All PE operands are bf16 (error budget 2e-2 is ~10x away): packed
   bf16 SBUF-to-SBUF DVE copies run in 4x perf mode.
 - attn@V swaps operand roles: the exp-weight chunk ew[:, nt*128:...]
   is the *stationary* matmul operand and v (32 cols + a ones column
   for the softmax row-sums) streams, so each matmul charges only 33
   rows instead of 512.  The output lands n-major, which kills v5's 64
   flush transposes and attn^T staging copies; accumulation uses 8
   sub-bank [128,33] regions spaced 64 cols apart in ONE psum bank,
   single-buffered across heads (each head's epilogue completes before
   the next head's first accumulating matmul needs the regions).
 - biasT is built by TWO accumulating matmuls per i-block (relw-window
   x a q^T block, then relh-window x a stride-32 column view of q^T
   that realizes the (i,j)->(j,i) permutation for free), so no
   permuted-q replica and no separate bias-rhs tile exist; both rhs
   reads come straight from qaug rows 0:32.
 - The softmax exp is split between the Act engine (activation Exp)
   and the Pool engine: gpsimd supports elementwise pow, so
   exp(s*x) = pow(e^s, x) with a stride-0 broadcast base.  gpsimd
   cannot read PSUM, so DVE stages the offloaded logit tiles
   PSUM->SBUF; the offload count balances Act against DVE+Pool.
 - Per-head epilogue: DVE reciprocal over the 8 strided ones-sums,
   one broadcast tensor_tensor multiply normalizing all 8 regions into
   the n-major staging tile, one strided DMA per head.
 - PSUM map: banks 0-5 = one [128,8,512] region manually slotted in
   512-col halves: pair (0,1) serves the Pool-offloaded logit tiles,
   the startup/half-1 transposes, and (heads >= 3) the bias scratch;
   pairs (2,3)/(4,5) alternate (continuously across heads) for the
   Act-exp'd tiles.  Banks 6/7 double-buffer the attn regions across
   heads (each head's bank is reset by ONE full-width start=True
   matmul -- interleaved per-region start writes clobber each other on
   real HW -- and all attn@V matmuls accumulate with start=False);
   bank 7 also hosts heads 0-2's bias scratch and the warm-up dummies
   before head 1's accumulation begins.
 - Engine queue discipline: the Tile scheduler reorders engine streams
   with its internal timing model, so ALL PE/DVE/Pool instructions are
   pinned in emission order with ordering-only deps; logit matmuls are
   emitted two tiles ahead so they only WAR-wait the exp that frees
   their psum pair (~1us of margin); deferred attn@V and the epilogue
   of head h-1 are emitted inside head h's j1 iteration.
 - rel tables are DMA'd in natural [63,32] layout (a transposed DMA
   would cost ~2000 descriptors on the single shared DMA stream, ahead
   of the critical q/k row loads) and transposed on the PE instead.

Toolchain note: walrus codegen only fits ONE semaphore wait in most
TPB instruction structs; split_multiwaits() moves excess waits onto
same-engine InstNoOp carriers (same workaround as v5).
"""

import numpy as np

import concourse.bass as bass
import concourse.mybir as mybir
import concourse.tile as tile
from concourse import bass_utils
from concourse.masks import make_identity
from concourse.tile import add_dep_helper

F32 = mybir.dt.float32
F32R = mybir.dt.float32r
BF16 = mybir.dt.bfloat16
AF = mybir.ActivationFunctionType

NH = 8
N = 1024
DK = 32
SCALE = float(DK) ** -0.5
BASE = float(np.exp(SCALE))

# Per-head tuple of js whose exp runs on Pool (via DVE PSUM->SBUF stage).
# Must be a subset of {0, 3} (those js own psum slot pair (0,1)); their
# attn@V is deferred to head end.
OFFLOAD = {h: (0,) for h in range(NH)}

KMARKS = []   # (inst_name, label) for trace debugging


def split_multiwaits(nc, dma_limit=1):
    """Move excess semaphore waits onto same-engine nop carriers."""
    n_new = 0
    for f in nc.m.functions:
        for blk in f.blocks:
            newlist = []
            for inst in blk.instructions:
                si = getattr(inst, "sync_info", None)
                is_dma = isinstance(inst, mybir.InstDMACopy)
                limit = dma_limit if is_dma else 1
                if si is not None and len(si.on_wait) > limit:
                    waits = list(si.on_wait)
                    for w in waits[:-1]:
                        n_new += 1
                        newlist.append(mybir.InstNoOp(
                            name=f"I-wc{n_new}",
                            ins=[], outs=[],
                            sync_info=mybir.SyncInfo(on_wait=[w], on_update=[]),
                            bass_nofuse=True,
                            engine=inst.engine,
                        ))
                    inst.sync_info = mybir.SyncInfo(
                        on_wait=waits[-1:], on_update=si.on_update)
                newlist.append(inst)
            blk.instructions = newlist
    return n_new


def kernel_body(tc, outs, ins):
    nc = tc.nc
    x = ins["x"]          # [1024, 768] rows n=(i,j), cols q|k|v
    relh = ins["relh"]    # [63, 32]
    relw = ins["relw"]    # [63, 32]
    out = outs["out"]     # [1024, 256]

    with (
        tc.tile_pool(name="persist", bufs=1) as persist,
        tc.tile_pool(name="expw", bufs=10) as expwp,
        tc.tile_pool(name="qaug", bufs=3) as qaugp,
        tc.tile_pool(name="lstg", bufs=3) as lstgp,
        tc.tile_pool(name="psmain", bufs=1, space="PSUM") as psmain,
    ):
        # ---- PSUM map ----
        ps_all = psmain.tile([128, 8, 512], F32, tag="ps")
        ps_flat = ps_all.rearrange("p s c -> p (s c)")

        def slot(s, n=1):
            return ps_flat[:, s * 512:(s + n) * 512]
        def ps_att_of(h):
            return slot(6 + h % 2)
        ps_bias = slot(7)

        # ---------------- DMAs (the DMA stream is serial in practice:
        # critical q rows first, tiny rel loads sandwiched, v last) ----
        rowsR = persist.tile([128, 4, 8, 128], F32R)
        CB_COLS = (0, 256, 128, 384)   # q0, k0, q1, k1

        def emit_rows_dma(cb, eng, lo=0, hi=8):
            src = bass.AP(tensor=x.tensor,
                          offset=CB_COLS[cb] + lo * 128 * 768,
                          ap=[[768, 128], [128 * 768, hi - lo], [1, 128]],
                          ).bitcast(F32R)
            eng.dma_start(out=rowsR[:, cb, lo:hi, :], in_=src)

        rel_nat = persist.tile([64, 64], F32R)
        nc.vector.memset(rel_nat.bitcast(F32), 0.0)
        v_st = persist.tile([128, 8, 256], F32)

        def emit_v_dma(j):
            nc.scalar.dma_start(
                out=v_st[:, j, :], in_=x[j * 128:(j + 1) * 128, 512:768])

        # the HWDGE serves the SP and Act queues round-robin: interleave
        # so the critical loads (q, rel, k, early v) land in order
        emit_rows_dma(0, nc.sync, 0, 4)          # q half0 lo
        nc.scalar.dma_start(out=rel_nat[0:63, 0:32], in_=relw.bitcast(F32R))
        emit_rows_dma(0, nc.sync, 4, 8)          # q half0 hi
        nc.scalar.dma_start(out=rel_nat[0:63, 32:64], in_=relh.bitcast(F32R))
        emit_rows_dma(1, nc.sync, 0, 4)          # k half0 lo
        emit_v_dma(0)
        emit_rows_dma(1, nc.sync, 4, 8)          # k half0 hi
        emit_v_dma(1)
        emit_v_dma(2)
        emit_rows_dma(2, nc.scalar)              # q half1
        emit_rows_dma(3, nc.scalar)              # k half1
        for j in range(3, 8):
            emit_v_dma(j)

        # ---------------- warm-up + constants ----------------
        _chain = {}
        dummy_sb = persist.tile([128, 64], F32)
        nc.vector.memset(dummy_sb, 0.0)
        for w in range(20):
            _dm = nc.tensor.matmul(ps_bias[64:96, 0:32],
                                   lhsT=dummy_sb[:, 0:32],
                                   rhs=dummy_sb[:, 0:32],
                                   start=True, stop=True)
            _chain.setdefault("pe", _dm)
            if _chain["pe"] is not _dm:
                add_dep_helper(_dm.ins, _chain["pe"].ins, sync=False,
                               reason="pin pe queue order")
            _chain["pe"] = _dm

        zeros_bf = persist.tile([128, 128], BF16)
        nc.vector.memset(zeros_bf, 0.0)
        ident = persist.tile([128, 128], F32)
        make_identity(nc, ident)
        identR = persist.tile([128, 128], F32R)
        nc.vector.tensor_copy(identR, ident)
        ident_marker = nc.gpsimd.tensor_copy(ident[0:1, 0:1], ident[0:1, 0:1])

        # pow base for the Pool exp share: pow(e^s, x) = exp(s*x)
        base_t = persist.tile([128, 1], F32)
        nc.vector.memset(base_t, BASE)

        def base_bcast(cols):
            return bass.AP(tensor=base_t.tensor, offset=base_t.offset,
                           ap=[list(base_t.ap[0]), [0, cols]])

        # Pin Pool and startup-DVE queue order (the Tile scheduler's
        # internal timing model reorders engine queues badly otherwise).
        _chain["pool"] = ident_marker

        def chained(engine_name, inst):
            prev = _chain.get(engine_name)
            if prev is not None:
                add_dep_helper(inst.ins, prev.ins, sync=False,
                               reason=f"pin {engine_name} queue order")
            _chain[engine_name] = inst
            return inst

        def pool(op, *args, **kwargs):
            return chained("pool", getattr(nc.gpsimd, op)(*args, **kwargs))

        def pe(inst):
            return chained("pe", inst)

        # onehot rows for K_aug (bf16) -- no input deps, head of Pool chain
        oh_st = persist.tile([32, 8, 128], F32)
        pool("memset", oh_st, 0.0)
        oh = oh_st.rearrange("t j (b m) -> t j b m", b=4)
        pool("affine_select",
             out=oh, in_=oh, compare_op=mybir.AluOpType.not_equal,
             fill=1.0, base=0, pattern=[[-4, 8], [-1, 4], [0, 32]],
             channel_multiplier=1)
        oh_bf = persist.tile([32, 8, 128], BF16)
        pool("tensor_copy", oh_bf, oh_st)

        # bf16 transposed replicas of q and k: [p=32*(h%4)+d, half, n]
        qT = persist.tile([128, 2, N], BF16)
        kT = persist.tile([128, 2, N], BF16)

        ka = [persist.tile([64, 8, 128], BF16, tag=f"ka{i}", name=f"ka{i}")
              for i in range(4)]

        # rel^T: [32, 2, 63] at partitions 0:32 (PE operands must share a
        # base partition): [:,0,:] = relw^T, [:,1,:] = relh^T
        rel_bf = persist.tile([32, 2, 63], BF16)

        def emit_transposes(cb, dst, half, lo, s, pin=False, pe_anchor=None):
            """4 transposes into one psum slot + one wide copy into the
            bf16 replica."""
            pt = slot(s).bitcast(F32R)
            for c in range(4):
                tr = pe(nc.tensor.transpose(pt[:, c * 128:(c + 1) * 128],
                                            rowsR[:, cb, lo + c, :], identR))
            cp = nc.vector.tensor_copy(
                dst[:, half, lo * 128:(lo + 4) * 128], slot(s))
            if pin:
                chained("dve", cp)

        # ---------------- per-head staging ----------------
        def emit_qstage(h, pin=False):
            """qaug rows 0:32 = q^T for head h (bf16 4x copy)."""
            lane = (h % 4) * 32
            qsT = qT[lane:lane + 32, h // 4, :]
            qaug = qaugp.tile([64, N], BF16, tag="qaug", name=f"qaug{h}")
            c1 = nc.vector.tensor_copy(qaug[0:32, :], qsT)
            if pin:
                chained("dve", c1)
            return qaug

        def emit_bias(h, qaug, halves=(0, 1), pin=False, pe_anchor=None):
            """biasT[t,(i,j)] = SWT[t+31-i,(i,j)]: per i-block, two
            accumulating matmuls with shifted rel windows; the relh term
            reads q^T through a stride-32 column view (the (i,j)->(j,i)
            permutation).  Scratch: heads 0-2 use bank 7 halves at
            partitions 0:32/32:64 (bank 7 becomes attn-B from head 1's
            accumulation on); later heads use the idle offload slots."""
            for half in halves:
                if h <= 2:
                    ps_b = ps_bias[half * 32:half * 32 + 32, :]
                else:
                    ps_b = slot(half)[0:32, :]
                for ib in range(16):
                    i = half * 16 + ib
                    pe(nc.tensor.matmul(
                        ps_b[:, ib * 32:(ib + 1) * 32],
                        lhsT=rel_bf[:, 0, 31 - i:63 - i],
                        rhs=qaug[0:32, i * 32:(i + 1) * 32],
                        start=True, stop=False))
                    perm_rhs = bass.AP(
                        tensor=qaug.tensor, offset=qaug.offset + i,
                        ap=[[qaug.ap[0][0], 32], [32, 32]])
                    pe(nc.tensor.matmul(
                        ps_b[:, ib * 32:(ib + 1) * 32],
                        lhsT=rel_bf[:, 1, 31 - i:63 - i],
                        rhs=perm_rhs,
                        start=False, stop=True))
                cp = nc.vector.tensor_copy(
                    qaug[32:64, half * 512:(half + 1) * 512], ps_b)
                if pin:
                    chained("dve", cp)

        def emit_kaug(h, pin=False, los=(0, 8)):
            lane = (h % 4) * 32
            lo, hi = los
            ksT = kT[lane:lane + 32, h // 4, lo * 128:hi * 128]
            cp = nc.vector.tensor_copy(
                ka[h % 4][0:32, lo:hi].rearrange("d j m -> d (j m)"), ksT)
            if pin:
                chained("dve", cp)

        def emit_kaug_oh(h, pin=False):
            if h < 4:
                co = nc.vector.tensor_copy(ka[h % 4][32:64], oh_bf)
                if pin:
                    chained("dve", co)

        # ---------------- v staging ----------------
        v_aug = persist.tile([128, NH, 8, 33], BF16)
        ones_st = persist.tile([128, 64], F32)
        nc.vector.memset(ones_st, 1.0)

        def emit_vconv(j):
            pool("tensor_copy",
                 v_aug[:, :, j, 0:32],
                 v_st[:, j, :].rearrange("p (h d) -> p h d", h=NH))

        nc.vector.tensor_copy(
            v_aug[:, :, :, 32:33].rearrange("p h j o -> p (h j o)"), ones_st)

        # ---------------- epilogue ----------------
        out_sb = persist.tile([128, 8, 256], F32)
        rec_t = persist.tile([128, NH, 8], F32)

        def emit_epilogue(h):
            """reciprocal of the 8 ones-sums + one broadcast normalize of
            the 8 [128,33] regions into out_sb; one strided DMA."""
            ps_att = ps_att_of(h)
            rec = rec_t[:, h, :]
            sums_ap = bass.AP(tensor=ps_att.tensor, offset=ps_att.offset + 32,
                              ap=[list(ps_att.ap[0]), [64, 8]])
            chained("dve", nc.vector.reciprocal(rec, sums_ap))
            in0 = bass.AP(tensor=ps_att.tensor, offset=ps_att.offset,
                          ap=[list(ps_att.ap[0]), [64, 8], [1, 32]])
            in1 = bass.AP(tensor=rec.tensor, offset=rec.offset,
                          ap=[list(rec.ap[0]), [1, 8], [0, 32]])
            out_ap = bass.AP(tensor=out_sb.tensor,
                             offset=out_sb.offset + h * 32,
                             ap=[list(out_sb.ap[0]), [256, 8], [1, 32]])
            chained("dve", nc.vector.tensor_tensor(
                out=out_ap, in0=in0, in1=in1, op=mybir.AluOpType.mult))
            groups = ((0, 8),)
            for glo, ghi in groups:
                dstap = bass.AP(
                    tensor=out.tensor,
                    offset=glo * 128 * 256 + h * 32,
                    ap=[[256, 128], [128 * 256, ghi - glo], [1, 32]])
                nc.sync.dma_start(out=dstap,
                                  in_=out_sb[:, glo:ghi, h * 32:(h + 1) * 32])

        # ---------------- startup ----------------
        # PE order: dummies, rel-w transpose, q transposes, rel-h
        # transpose, k transposes, bias mms, logits.  The DVE chain IS
        # the head-0 critical path: qaug0 and ka0 rows are copied
        # straight out of the transpose psum slots (lane 0); the qT
        # replica copies (for later heads) trail behind and delay only
        # head 0's j0, whose exp is Pool-offloaded and slack-tolerant.
        # Slot use: q-lo->0, q-hi->1, rel->4 (cols 0:127), k-lo->2,
        # k-hi->5; slot 3 stays free for j1's logits.
        relT = slot(4).bitcast(F32R)
        pe(nc.tensor.transpose(relT[0:32, 0:64],
                               rel_nat[:, 0:32], identR[0:64, 0:64]))

        def transp4(cb, lo, s):
            pt = slot(s).bitcast(F32R)
            for c in range(4):
                pe(nc.tensor.transpose(pt[:, c * 128:(c + 1) * 128],
                                       rowsR[:, cb, lo + c, :], identR))

        transp4(0, 0, 0)                                # q half0 lo
        pe(nc.tensor.transpose(relT[0:32, 64:128],
                               rel_nat[:, 32:64], identR[0:64, 0:64]))
        transp4(0, 4, 1)                                # q half0 hi
        transp4(1, 0, 2)                                # k half0 lo

        chained("dve", nc.vector.tensor_copy(
            rel_bf.rearrange("p a u -> p (a u)"),
            bass.AP(tensor=ps_flat.tensor,
                    offset=ps_flat.offset + 4 * 512,
                    ap=[[ps_flat.ap[0][0], 32], [64, 2], [1, 63]])))
        qaug_h = {0: qaugp.tile([64, N], BF16, tag="qaug", name="qaug0")}
        chained("dve", nc.vector.tensor_copy(
            qaug_h[0][0:32, 0:512], slot(0)[0:32, :]))
        chained("dve", nc.vector.tensor_copy(
            qaug_h[0][0:32, 512:1024], slot(1)[0:32, :]))
        chained("dve", nc.vector.tensor_copy(
            ka[0][0:32, 0:4].rearrange("d j m -> d (j m)"),
            slot(2)[0:32, :]))
        # ka[1] rows straight off the same transpose slots (lane 1 =
        # psum partitions 32:64; DVE copies may shift partitions)
        chained("dve", nc.vector.tensor_copy(
            ka[1][0:32, 0:4].rearrange("d j m -> d (j m)"),
            slot(2)[32:64, :]))
        emit_kaug_oh(0, pin=True)
        emit_bias(0, qaug_h[0], halves=(0,), pin=True)
        emit_bias(0, qaug_h[0], halves=(1,), pin=True)
        transp4(1, 4, 5)                                # k half0 hi
        chained("dve", nc.vector.tensor_copy(
            ka[0][0:32, 4:8].rearrange("d j m -> d (j m)"),
            slot(5)[0:32, :]))
        chained("dve", nc.vector.tensor_copy(
            ka[1][0:32, 4:8].rearrange("d j m -> d (j m)"),
            slot(5)[32:64, :]))
        pool("tensor_copy", ka[1][32:64], oh_bf)
        # trailing (delays only head 0's slack-tolerant j0): qT replica
        chained("dve", nc.vector.tensor_copy(qT[:, 0, 0:512], slot(0)))
        chained("dve", nc.vector.tensor_copy(qT[:, 0, 512:1024], slot(1)))
        for j in range(8):
            emit_vconv(j)

        # ---------------- main pipeline ----------------
        ACT_PAIRS = ((2, 3), (4, 5))
        act_rot = [0]   # continuous pair rotation across heads

        def emit_logits(h, j, lo_slot):
            qaug = qaug_h[h]
            for half in range(2):
                mm = pe(nc.tensor.matmul(
                    slot(lo_slot + half), lhsT=ka[h % 4][:, j, :],
                    rhs=qaug[:, half * 512:(half + 1) * 512],
                    start=True, stop=True))
                KMARKS.append((mm.ins.name, f"logits{h}_{j}_h{half}_s{lo_slot+half}"))

        def hook(h, j, a):
            if h == 0:
                if j == 2:
                    emit_transposes(2, qT, 1, 0, s=0, pin=True)
                elif j == 3:
                    emit_transposes(2, qT, 1, 4, s=1, pin=True)
                    qaug_h[1] = emit_qstage(1, pin=True)
                    emit_bias(1, qaug_h[1], halves=(0,), pin=True)
                elif j == 4:
                    # kT half0 replica: re-transpose (startup slots were
                    # drained into ka0/ka1 directly)
                    emit_transposes(1, kT, 0, 0, s=0, pin=True)
                    emit_bias(1, qaug_h[1], halves=(1,), pin=True)
                elif j == 5:
                    emit_transposes(1, kT, 0, 4, s=1, pin=True)
                elif j == 6:
                    qaug_h[2] = emit_qstage(2, pin=True)
                    emit_bias(2, qaug_h[2], halves=(0,), pin=True)
                elif j == 7:
                    emit_bias(2, qaug_h[2], halves=(1,), pin=True)
            elif h == 1:
                if j == 1:
                    emit_transposes(3, kT, 1, 0, s=0, pin=True)
                    emit_kaug(2, pin=True)
                    emit_kaug_oh(2, pin=True)
                elif j == 2:
                    qaug_h[3] = emit_qstage(3, pin=True)
                elif j == 3:
                    emit_bias(3, qaug_h[3], halves=(0,), pin=True)
                elif j == 4:
                    emit_bias(3, qaug_h[3], halves=(1,), pin=True)
                elif j == 5:
                    emit_transposes(3, kT, 1, 4, s=1, pin=True)
            else:
                if j == 1 and h + 1 < NH:
                    emit_kaug(h + 1, pin=True)
                    emit_kaug_oh(h + 1, pin=True)
                elif j == 2 and h + 2 < NH:
                    qaug_h[h + 2] = emit_qstage(h + 2, pin=True)
                elif j == 3 and h + 2 < NH:
                    emit_bias(h + 2, qaug_h[h + 2], halves=(0,), pin=True)
                elif j == 4 and h + 2 < NH:
                    emit_bias(h + 2, qaug_h[h + 2], halves=(1,), pin=True)

        def emit_attnv_zero(h):
            # one full-width start=True matmul resets the attn bank;
            # interleaved per-region start writes clobber each other on HW
            pe(nc.tensor.matmul(ps_att_of(h), lhsT=zeros_bf,
                                rhs=qT[:, 0, 0:512],
                                start=True, stop=False))

        def emit_attnv(h, j, ew, start, stop):
            ps_att = ps_att_of(h)
            a_last = None
            for nt in range(8):
                a_last = pe(nc.tensor.matmul(
                    ps_att[:, nt * 64:nt * 64 + 33],
                    lhsT=ew[:, nt * 128:(nt + 1) * 128],
                    rhs=v_aug[:, h, j, :],
                    start=start, stop=stop))
            return a_last

        # pending deferred work from head h-1, emitted inside head h's j1
        # iteration (gives the slow offload pipeline extra time before its
        # attn@V could block the in-order PE queue):
        pending = None

        # pair assignment per tile, rotation continuous across heads
        pair_of = {}
        rot = 0
        for h in range(NH):
            for j in range(8):
                if j in OFFLOAD[h]:
                    pair_of[(h, j)] = (0, 1)
                else:
                    pair_of[(h, j)] = ACT_PAIRS[rot % 2]
                    rot += 1

        emitted_logits = set()

        def next_tile(h, j):
            if j < 7:
                return (h, j + 1)
            return (h + 1, 0) if h + 1 < NH else None

        def emit_logits_once(t):
            if t is not None and t not in emitted_logits:
                emitted_logits.add(t)
                emit_logits(t[0], t[1], pair_of[t][0])

        # head-0 priming: j1/j2 logits first (they gate Act); j0 last --
        # its slots are released only by the trailing qT replica copies,
        # and its Pool-exp pipeline has most of the head as slack
        for t in ((0, 1), (0, 2), (0, 0)):
            emit_logits_once(t)
        for h in range(NH):
            off_js = OFFLOAD[h]
            act_js = [j for j in range(8) if j not in off_js]
            last = NH - 1
            first_j = act_js[0]
            last_j = 7 if h == last else 0
            ews = {}
            for j in range(8):
                # this tile's logits were emitted one iteration ago; emit
                # the NEXT tile's logits before this tile's attn@V so the
                # pinned PE queue never waits an exp to issue logits
                if h == last and j == 7:
                    emit_attnv(h, 0, ews[0], False, False)
                if j in off_js:
                    if j == 0:
                        emit_attnv_zero(h)
                    ls = lstgp.tile([128, N], F32, tag="ls",
                                    name=f"ls{h}_{j}")
                    KMARKS.append((chained("dve", nc.vector.tensor_copy(
                        ls, slot(0, 2))).ins.name, f"stage{h}_{j}"))
                    ew = expwp.tile([128, N], BF16, tag="ew",
                                    name=f"ew{h}_{j}")
                    chained("pool", nc.gpsimd.tensor_tensor(
                        out=ew, in0=base_bcast(N), in1=ls,
                        op=mybir.AluOpType.pow))
                    ews[j] = ew
                    emit_logits_once(next_tile(h, j))
                    nt2 = next_tile(h, j)
                    if nt2 is not None:
                        emit_logits_once(next_tile(*nt2))
                else:
                    ew = expwp.tile([128, N], BF16, tag="ew",
                                    name=f"ew{h}_{j}")
                    if (h, j) == (0, 1):
                        # two half-exps: half0 unblocks on bias-half-a,
                        # pulling the whole Act stream ~1us earlier
                        for hf in range(2):
                            KMARKS.append((nc.scalar.activation(
                                ew[:, hf * 512:(hf + 1) * 512],
                                slot(pair_of[(h, j)][0] + hf),
                                AF.Exp, scale=SCALE).ins.name,
                                f"exp{h}_{j}h{hf}"))
                    else:
                        KMARKS.append((nc.scalar.activation(
                            ew, slot(pair_of[(h, j)][0], 2),
                            AF.Exp, scale=SCALE).ins.name, f"exp{h}_{j}"))
                    ews[j] = ew
                    # two tiles ahead: logits(t+2) only WAR-waits this
                    # exp's pair, giving the chain ~1us of margin
                    emit_logits_once(next_tile(h, j))
                    nt2 = next_tile(h, j)
                    if nt2 is not None:
                        emit_logits_once(next_tile(*nt2))
                    if j == 1 and pending is not None:
                        ph, defs = pending
                        for idx, (pj, pew) in enumerate(defs):
                            emit_attnv(ph, pj, pew, False,
                                       idx == len(defs) - 1)
                        emit_epilogue(ph)
                        pending = None
                    if h == last and j == 7 and 3 in off_js:
                        emit_attnv(h, 3, ews[3], False, False)
                    a_last = emit_attnv(h, j, ew, False, j == last_j)
                    hook(h, j, a_last)
            if h != last:
                # all deferred attn@V for Pool-exp'd js punts into head
                # h+1's j1 (so a pow still in flight can never block the
                # in-order PE queue at the head boundary)
                pending = (h, [(j, ews[j]) for j in off_js])

        emit_epilogue(NH - 1)


_NC_CACHE = {}


def _build():
    if "nc" in _NC_CACHE:
        return _NC_CACHE["nc"]
    nc = bass.Bass("TRN2", target_bir_lowering=False, debug=False,
                   enable_asserts=True, num_devices=8)
    ins = {
        "x": nc.dram_tensor("x", [N, 768], F32, kind="ExternalInput").ap(),
        "relh": nc.dram_tensor("relh", [63, 32], F32, kind="ExternalInput").ap(),
        "relw": nc.dram_tensor("relw", [63, 32], F32, kind="ExternalInput").ap(),
    }
    outs = {
        "out": nc.dram_tensor("out", [N, 256], F32, kind="ExternalOutput").ap(),
    }
    with tile.TileContext(nc) as tc:
        kernel_body(tc, outs, ins)
    split_multiwaits(nc)
    _NC_CACHE["nc"] = nc
    return nc


def kernel(inputs, key_rel_h, key_rel_w, _trace=False):
    nc = _build()
    x = np.ascontiguousarray(np.asarray(inputs, dtype=np.float32).reshape(8, N, 768))
    rh = np.ascontiguousarray(np.asarray(key_rel_h, dtype=np.float32))
    rw = np.ascontiguousarray(np.asarray(key_rel_w, dtype=np.float32))
    in_maps = [{"x": x[c], "relh": rh, "relw": rw} for c in range(8)]
    res = bass_utils.run_bass_kernel_spmd(
        nc, in_maps, core_ids=list(range(8)), trace=_trace)
    outp = np.stack([r["out"] for r in res.results])
    if _trace:
        kernel.last_results = res
    return outp.reshape(8, 32, 32, 256)


# revision 47
# speedup vs baseline: 1.0988x; 1.0032x over previous
"""AttentionAugmentation2D Trainium2 kernel (v6).

Shapes (hardcoded): B=8, H=W=32, N=1024, NH=8 heads, dk=dv=32 per head.
inputs [8,32,32,768] = q|k|v (256 each), key_rel_h/w [63,32].
Sharding: data-parallel over batch B across the 8 cores.

Math per (batch, head), n=(i,j), m=(i',j') (i = H index):
  logits[n,m] = q[n]@k[m] + q[(j,i)]@rel_h[i'-i+31] + q[(i,j)]@rel_w[i'-i+31]
Both rel terms depend on m only through i', so with
  SWT[u,n] = rel_w[u]@q[(i,j)] + rel_h[u]@q[(j,i)]        (u in [0,63))
  biasT[t,n] = SWT[t+31-i(n), n]                          (shifted windows)
we get  logits^T = K_aug^T.T @ Q_augT  with contraction 64:
  K_aug rows: 0:32 = k^T, 32:64 = onehot[t==i'(m)] ;  Q_aug: [q^T; biasT].

v6 redesign vs v5 (cost-model driven):
 - All PE operands are bf16 (error budget 2e-2 is ~10x away): packed
   bf16 SBUF-to-SBUF DVE copies run in 4x perf mode.
 - attn@V swaps operand roles: the exp-weight chunk ew[:, nt*128:...]
   is the *stationary* matmul operand and v (32 cols + a ones column
   for the softmax row-sums) streams, so each matmul charges only 33
   rows instead of 512.  The output lands n-major, which kills v5's 64
   flush transposes and attn^T staging copies; accumulation uses 8
   sub-bank [128,33] regions spaced 64 cols apart in ONE psum bank,
   single-buffered across heads (each head's epilogue completes before
   the next head's first accumulating matmul needs the regions).
 - biasT is built by TWO accumulating matmuls per i-block (relw-window
   x a q^T block, then relh-window x a stride-32 column view of q^T
   that realizes the (i,j)->(j,i) permutation for free), so no
   permuted-q replica and no separate bias-rhs tile exist; both rhs
   reads come straight from qaug rows 0:32.
 - The softmax exp is split between the Act engine (activation Exp)
   and the Pool engine: gpsimd supports elementwise pow, so
   exp(s*x) = pow(e^s, x) with a stride-0 broadcast base.  gpsimd
   cannot read PSUM, so DVE stages the offloaded logit tiles
   PSUM->SBUF; the offload count balances Act against DVE+Pool.
 - Per-head epilogue: DVE reciprocal over the 8 strided ones-sums,
   one broadcast tensor_tensor multiply normalizing all 8 regions into
   the n-major staging tile, one strided DMA per head.
 - PSUM map: banks 0-5 = one [128,8,512] region manually slotted in
   512-col halves: pair (0,1) serves the Pool-offloaded logit tiles,
   the startup/half-1 transposes, and (heads >= 3) the bias scratch;
   pairs (2,3)/(4,5) alternate (continuously across heads) for the
   Act-exp'd tiles.  Banks 6/7 double-buffer the attn regions across
   heads (each head's bank is reset by ONE full-width start=True
   matmul -- interleaved per-region start writes clobber each other on
   real HW -- and all attn@V matmuls accumulate with start=False);
   bank 7 also hosts heads 0-2's bias scratch and the warm-up dummies
   before head 1's accumulation begins.
 - Engine queue discipline: the Tile scheduler reorders engine streams
   with its internal timing model, so ALL PE/DVE/Pool instructions are
   pinned in emission order with ordering-only deps; logit matmuls are
   emitted two tiles ahead so they only WAR-wait the exp that frees
   their psum pair (~1us of margin); deferred attn@V and the epilogue
   of head h-1 are emitted inside head h's j1 iteration.
 - rel tables are DMA'd in natural [63,32] layout (a transposed DMA
   would cost ~2000 descriptors on the single shared DMA stream, ahead
   of the critical q/k row loads) and transposed on the PE instead.

Toolchain note: walrus codegen only fits ONE semaphore wait in most
TPB instruction structs; split_multiwaits() moves excess waits onto
same-engine InstNoOp carriers (same workaround as v5).
"""

import numpy as np

import concourse.bass as bass
import concourse.mybir as mybir
import concourse.tile as tile
from concourse import bass_utils
from concourse.masks import make_identity
from concourse.tile import add_dep_helper

F32 = mybir.dt.float32
F32R = mybir.dt.float32r
BF16 = mybir.dt.bfloat16
AF = mybir.ActivationFunctionType

NH = 8
N = 1024
DK = 32
SCALE = float(DK) ** -0.5
BASE = float(np.exp(SCALE))

# Per-head tuple of js whose exp runs on Pool (via DVE PSUM->SBUF stage).
# Must be a subset of {0, 3} (those js own psum slot pair (0,1)); their
# attn@V is deferred to head end.
OFFLOAD = {h: (0,) for h in range(NH)}

KMARKS = []   # (inst_name, label) for trace debugging


def split_multiwaits(nc, dma_limit=1):
    """Move excess semaphore waits onto same-engine nop carriers."""
    n_new = 0
    for f in nc.m.functions:
        for blk in f.blocks:
            newlist = []
            for inst in blk.instructions:
                si = getattr(inst, "sync_info", None)
                is_dma = isinstance(inst, mybir.InstDMACopy)
                limit = dma_limit if is_dma else 1
                if si is not None and len(si.on_wait) > limit:
                    waits = list(si.on_wait)
                    for w in waits[:-1]:
                        n_new += 1
                        newlist.append(mybir.InstNoOp(
                            name=f"I-wc{n_new}",
                            ins=[], outs=[],
                            sync_info=mybir.SyncInfo(on_wait=[w], on_update=[]),
                            bass_nofuse=True,
                            engine=inst.engine,
                        ))
                    inst.sync_info = mybir.SyncInfo(
                        on_wait=waits[-1:], on_update=si.on_update)
                newlist.append(inst)
            blk.instructions = newlist
    return n_new


def kernel_body(tc, outs, ins):
    nc = tc.nc
    x = ins["x"]          # [1024, 768] rows n=(i,j), cols q|k|v
    relh = ins["relh"]    # [63, 32]
    relw = ins["relw"]    # [63, 32]
    out = outs["out"]     # [1024, 256]

    with (
        tc.tile_pool(name="persist", bufs=1) as persist,
        tc.tile_pool(name="expw", bufs=10) as expwp,
        tc.tile_pool(name="qaug", bufs=3) as qaugp,
        tc.tile_pool(name="lstg", bufs=3) as lstgp,
        tc.tile_pool(name="psmain", bufs=1, space="PSUM") as psmain,
    ):
        # ---- PSUM map ----
        ps_all = psmain.tile([128, 8, 512], F32, tag="ps")
        ps_flat = ps_all.rearrange("p s c -> p (s c)")

        def slot(s, n=1):
            return ps_flat[:, s * 512:(s + n) * 512]
        def ps_att_of(h):
            return slot(6 + h % 2)
        ps_bias = slot(7)

        # ---------------- DMAs (the DMA stream is serial in practice:
        # critical q rows first, tiny rel loads sandwiched, v last) ----
        rowsR = persist.tile([128, 4, 8, 128], F32R)
        CB_COLS = (0, 256, 128, 384)   # q0, k0, q1, k1

        def emit_rows_dma(cb, eng, lo=0, hi=8):
            src = bass.AP(tensor=x.tensor,
                          offset=CB_COLS[cb] + lo * 128 * 768,
                          ap=[[768, 128], [128 * 768, hi - lo], [1, 128]],
                          ).bitcast(F32R)
            eng.dma_start(out=rowsR[:, cb, lo:hi, :], in_=src)

        rel_nat = persist.tile([64, 64], F32R)
        nc.vector.memset(rel_nat.bitcast(F32), 0.0)
        v_st = persist.tile([128, 8, 256], F32)

        def emit_v_dma(j):
            nc.scalar.dma_start(
                out=v_st[:, j, :], in_=x[j * 128:(j + 1) * 128, 512:768])

        # the HWDGE serves the SP and Act queues round-robin: interleave
        # so the critical loads (q, rel, k, early v) land in order
        emit_rows_dma(0, nc.sync, 0, 4)          # q half0 lo
        nc.scalar.dma_start(out=rel_nat[0:63, 0:32], in_=relw.bitcast(F32R))
        emit_rows_dma(0, nc.sync, 4, 8)          # q half0 hi
        nc.scalar.dma_start(out=rel_nat[0:63, 32:64], in_=relh.bitcast(F32R))
        emit_rows_dma(1, nc.sync, 0, 4)          # k half0 lo
        emit_v_dma(0)
        emit_rows_dma(1, nc.sync, 4, 8)          # k half0 hi
        emit_v_dma(1)
        emit_v_dma(2)
        emit_rows_dma(2, nc.scalar)              # q half1
        emit_rows_dma(3, nc.scalar)              # k half1
        for j in range(3, 8):
            emit_v_dma(j)

        # ---------------- warm-up + constants ----------------
        _chain = {}
        dummy_sb = persist.tile([128, 64], F32)
        nc.vector.memset(dummy_sb, 0.0)
        for w in range(20):
            _dm = nc.tensor.matmul(ps_bias[64:96, 0:32],
                                   lhsT=dummy_sb[:, 0:32],
                                   rhs=dummy_sb[:, 0:32],
                                   start=True, stop=True)
            _chain.setdefault("pe", _dm)
            if _chain["pe"] is not _dm:
                add_dep_helper(_dm.ins, _chain["pe"].ins, sync=False,
                               reason="pin pe queue order")
            _chain["pe"] = _dm

        zeros_bf = persist.tile([128, 128], BF16)
        nc.vector.memset(zeros_bf, 0.0)
        ident = persist.tile([128, 128], F32)
        make_identity(nc, ident)
        identR = persist.tile([128, 128], F32R)
        nc.vector.tensor_copy(identR, ident)
        ident_marker = nc.gpsimd.tensor_copy(ident[0:1, 0:1], ident[0:1, 0:1])

        # pow base for the Pool exp share: pow(e^s, x) = exp(s*x)
        base_t = persist.tile([128, 1], F32)
        nc.vector.memset(base_t, BASE)

        def base_bcast(cols):
            return bass.AP(tensor=base_t.tensor, offset=base_t.offset,
                           ap=[list(base_t.ap[0]), [0, cols]])

        # Pin Pool and startup-DVE queue order (the Tile scheduler's
        # internal timing model reorders engine queues badly otherwise).
        _chain["pool"] = ident_marker

        def chained(engine_name, inst):
            prev = _chain.get(engine_name)
            if prev is not None:
                add_dep_helper(inst.ins, prev.ins, sync=False,
                               reason=f"pin {engine_name} queue order")
            _chain[engine_name] = inst
            return inst

        def pool(op, *args, **kwargs):
            return chained("pool", getattr(nc.gpsimd, op)(*args, **kwargs))

        def pe(inst):
            return chained("pe", inst)

        # onehot rows for K_aug (bf16) -- no input deps, head of Pool chain
        oh_st = persist.tile([32, 8, 128], F32)
        pool("memset", oh_st, 0.0)
        oh = oh_st.rearrange("t j (b m) -> t j b m", b=4)
        pool("affine_select",
             out=oh, in_=oh, compare_op=mybir.AluOpType.not_equal,
             fill=1.0, base=0, pattern=[[-4, 8], [-1, 4], [0, 32]],
             channel_multiplier=1)
        oh_bf = persist.tile([32, 8, 128], BF16)
        pool("tensor_copy", oh_bf, oh_st)

        # bf16 transposed replicas of q and k: [p=32*(h%4)+d, half, n]
        qT = persist.tile([128, 2, N], BF16)
        kT = persist.tile([128, 2, N], BF16)

        ka = [persist.tile([64, 8, 128], BF16, tag=f"ka{i}", name=f"ka{i}")
              for i in range(4)]

        # rel^T: [32, 2, 63] at partitions 0:32 (PE operands must share a
        # base partition): [:,0,:] = relw^T, [:,1,:] = relh^T
        rel_bf = persist.tile([32, 2, 63], BF16)

        def emit_transposes(cb, dst, half, lo, s, pin=False, pe_anchor=None):
            """4 transposes into one psum slot + one wide copy into the
            bf16 replica."""
            pt = slot(s).bitcast(F32R)
            for c in range(4):
                tr = pe(nc.tensor.transpose(pt[:, c * 128:(c + 1) * 128],
                                            rowsR[:, cb, lo + c, :], identR))
            cp = nc.vector.tensor_copy(
                dst[:, half, lo * 128:(lo + 4) * 128], slot(s))
            if pin:
                chained("dve", cp)

        # ---------------- per-head staging ----------------
        def emit_qstage(h, pin=False):
            """qaug rows 0:32 = q^T for head h (bf16 4x copy)."""
            lane = (h % 4) * 32
            qsT = qT[lane:lane + 32, h // 4, :]
            qaug = qaugp.tile([64, N], BF16, tag="qaug", name=f"qaug{h}")
            c1 = nc.vector.tensor_copy(qaug[0:32, :], qsT)
            if pin:
                chained("dve", c1)
            return qaug

        def emit_bias(h, qaug, halves=(0, 1), pin=False, pe_anchor=None):
            """biasT[t,(i,j)] = SWT[t+31-i,(i,j)]: per i-block, two
            accumulating matmuls with shifted rel windows; the relh term
            reads q^T through a stride-32 column view (the (i,j)->(j,i)
            permutation).  Scratch: heads 0-2 use bank 7 halves at
            partitions 0:32/32:64 (bank 7 becomes attn-B from head 1's
            accumulation on); later heads use the idle offload slots."""
            for half in halves:
                if h <= 2:
                    ps_b = ps_bias[half * 32:half * 32 + 32, :]
                else:
                    ps_b = slot(half)[0:32, :]
                for ib in range(16):
                    i = half * 16 + ib
                    pe(nc.tensor.matmul(
                        ps_b[:, ib * 32:(ib + 1) * 32],
                        lhsT=rel_bf[:, 0, 31 - i:63 - i],
                        rhs=qaug[0:32, i * 32:(i + 1) * 32],
                        start=True, stop=False))
                    perm_rhs = bass.AP(
                        tensor=qaug.tensor, offset=qaug.offset + i,
                        ap=[[qaug.ap[0][0], 32], [32, 32]])
                    pe(nc.tensor.matmul(
                        ps_b[:, ib * 32:(ib + 1) * 32],
                        lhsT=rel_bf[:, 1, 31 - i:63 - i],
                        rhs=perm_rhs,
                        start=False, stop=True))
                cp = nc.vector.tensor_copy(
                    qaug[32:64, half * 512:(half + 1) * 512], ps_b)
                if pin:
                    chained("dve", cp)

        def emit_kaug(h, pin=False, los=(0, 8)):
            lane = (h % 4) * 32
            lo, hi = los
            ksT = kT[lane:lane + 32, h // 4, lo * 128:hi * 128]
            cp = nc.vector.tensor_copy(
                ka[h % 4][0:32, lo:hi].rearrange("d j m -> d (j m)"), ksT)
            if pin:
                chained("dve", cp)

        def emit_kaug_oh(h, pin=False):
            if h < 4:
                co = nc.vector.tensor_copy(ka[h % 4][32:64], oh_bf)
                if pin:
                    chained("dve", co)

        # ---------------- v staging ----------------
        v_aug = persist.tile([128, NH, 8, 33], BF16)
        ones_st = persist.tile([128, 64], F32)
        nc.vector.memset(ones_st, 1.0)

        def emit_vconv(j):
            pool("tensor_copy",
                 v_aug[:, :, j, 0:32],
                 v_st[:, j, :].rearrange("p (h d) -> p h d", h=NH))

        nc.vector.tensor_copy(
            v_aug[:, :, :, 32:33].rearrange("p h j o -> p (h j o)"), ones_st)

        # ---------------- epilogue ----------------
        out_sb = persist.tile([128, 8, 256], F32)
        rec_t = persist.tile([128, NH, 8], F32)

        def emit_epilogue(h):
            """reciprocal of the 8 ones-sums + one broadcast normalize of
            the 8 [128,33] regions into out_sb; one strided DMA."""
            ps_att = ps_att_of(h)
            rec = rec_t[:, h, :]
            sums_ap = bass.AP(tensor=ps_att.tensor, offset=ps_att.offset + 32,
                              ap=[list(ps_att.ap[0]), [64, 8]])
            chained("dve", nc.vector.reciprocal(rec, sums_ap))
            in0 = bass.AP(tensor=ps_att.tensor, offset=ps_att.offset,
                          ap=[list(ps_att.ap[0]), [64, 8], [1, 32]])
            in1 = bass.AP(tensor=rec.tensor, offset=rec.offset,
                          ap=[list(rec.ap[0]), [1, 8], [0, 32]])
            out_ap = bass.AP(tensor=out_sb.tensor,
                             offset=out_sb.offset + h * 32,
                             ap=[list(out_sb.ap[0]), [256, 8], [1, 32]])
            chained("dve", nc.vector.tensor_tensor(
                out=out_ap, in0=in0, in1=in1, op=mybir.AluOpType.mult))
            groups = ((0, 8),)
            for glo, ghi in groups:
                dstap = bass.AP(
                    tensor=out.tensor,
                    offset=glo * 128 * 256 + h * 32,
                    ap=[[256, 128], [128 * 256, ghi - glo], [1, 32]])
                nc.sync.dma_start(out=dstap,
                                  in_=out_sb[:, glo:ghi, h * 32:(h + 1) * 32])

        # ---------------- startup ----------------
        # PE order: dummies, rel-w transpose, q transposes, rel-h
        # transpose, k transposes, bias mms, logits.  The DVE chain IS
        # the head-0 critical path: qaug0 and ka0 rows are copied
        # straight out of the transpose psum slots (lane 0); the qT
        # replica copies (for later heads) trail behind and delay only
        # head 0's j0, whose exp is Pool-offloaded and slack-tolerant.
        # Slot use: q-lo->0, q-hi->1, rel->4 (cols 0:127), k-lo->2,
        # k-hi->5; slot 3 stays free for j1's logits.
        relT = slot(4).bitcast(F32R)
        pe(nc.tensor.transpose(relT[0:32, 0:64],
                               rel_nat[:, 0:32], identR[0:64, 0:64]))

        def transp4(cb, lo, s):
            pt = slot(s).bitcast(F32R)
            for c in range(4):
                pe(nc.tensor.transpose(pt[:, c * 128:(c + 1) * 128],
                                       rowsR[:, cb, lo + c, :], identR))

        transp4(0, 0, 0)                                # q half0 lo
        pe(nc.tensor.transpose(relT[0:32, 64:128],
                               rel_nat[:, 32:64], identR[0:64, 0:64]))
        transp4(0, 4, 1)                                # q half0 hi
        transp4(1, 0, 2)                                # k half0 lo

        chained("dve", nc.vector.tensor_copy(
            rel_bf.rearrange("p a u -> p (a u)"),
            bass.AP(tensor=ps_flat.tensor,
                    offset=ps_flat.offset + 4 * 512,
                    ap=[[ps_flat.ap[0][0], 32], [64, 2], [1, 63]])))
        qaug_h = {0: qaugp.tile([64, N], BF16, tag="qaug", name="qaug0")}
        chained("dve", nc.vector.tensor_copy(
            qaug_h[0][0:32, 0:512], slot(0)[0:32, :]))
        chained("dve", nc.vector.tensor_copy(
            qaug_h[0][0:32, 512:1024], slot(1)[0:32, :]))
        chained("dve", nc.vector.tensor_copy(
            ka[0][0:32, 0:4].rearrange("d j m -> d (j m)"),
            slot(2)[0:32, :]))
        # ka[1] rows straight off the same transpose slots (lane 1 =
        # psum partitions 32:64; DVE copies may shift partitions)
        chained("dve", nc.vector.tensor_copy(
            ka[1][0:32, 0:4].rearrange("d j m -> d (j m)"),
            slot(2)[32:64, :]))
        emit_kaug_oh(0, pin=True)
        emit_bias(0, qaug_h[0], halves=(0,), pin=True)
        emit_bias(0, qaug_h[0], halves=(1,), pin=True)
        transp4(1, 4, 5)                                # k half0 hi
        chained("dve", nc.vector.tensor_copy(
            ka[0][0:32, 4:8].rearrange("d j m -> d (j m)"),
            slot(5)[0:32, :]))
        chained("dve", nc.vector.tensor_copy(
            ka[1][0:32, 4:8].rearrange("d j m -> d (j m)"),
            slot(5)[32:64, :]))
        pool("tensor_copy", ka[1][32:64], oh_bf)
        # trailing (delays only head 0's slack-tolerant j0): qT replica
        chained("dve", nc.vector.tensor_copy(qT[:, 0, 0:512], slot(0)))
        chained("dve", nc.vector.tensor_copy(qT[:, 0, 512:1024], slot(1)))
        for j in range(8):
            emit_vconv(j)

        # ---------------- main pipeline ----------------
        ACT_PAIRS = ((2, 3), (4, 5))
        act_rot = [0]   # continuous pair rotation across heads

        def emit_logits(h, j, lo_slot):
            qaug = qaug_h[h]
            for half in range(2):
                mm = pe(nc.tensor.matmul(
                    slot(lo_slot + half), lhsT=ka[h % 4][:, j, :],
                    rhs=qaug[:, half * 512:(half + 1) * 512],
                    start=True, stop=True))
                KMARKS.append((mm.ins.name, f"logits{h}_{j}_h{half}_s{lo_slot+half}"))

        def hook(h, j, a):
            if h == 0:
                if j == 2:
                    emit_transposes(2, qT, 1, 0, s=0, pin=True)
                elif j == 3:
                    emit_transposes(2, qT, 1, 4, s=1, pin=True)
                    qaug_h[1] = emit_qstage(1, pin=True)
                    emit_bias(1, qaug_h[1], halves=(0,), pin=True)
                elif j == 4:
                    # kT half0 replica: re-transpose (startup slots were
                    # drained into ka0/ka1 directly)
                    emit_transposes(1, kT, 0, 0, s=0, pin=True)
                    emit_bias(1, qaug_h[1], halves=(1,), pin=True)
                elif j == 5:
                    emit_transposes(1, kT, 0, 4, s=0, pin=True)
                elif j == 6:
                    qaug_h[2] = emit_qstage(2, pin=True)
                    emit_bias(2, qaug_h[2], halves=(0,), pin=True)
                elif j == 7:
                    emit_bias(2, qaug_h[2], halves=(1,), pin=True)
            elif h == 1:
                if j == 1:
                    emit_transposes(3, kT, 1, 0, s=0, pin=True)
                    emit_kaug(2, pin=True)
                    emit_kaug_oh(2, pin=True)
                elif j == 2:
                    qaug_h[3] = emit_qstage(3, pin=True)
                elif j == 3:
                    emit_bias(3, qaug_h[3], halves=(0,), pin=True)
                elif j == 4:
                    emit_bias(3, qaug_h[3], halves=(1,), pin=True)
                elif j == 5:
                    emit_transposes(3, kT, 1, 4, s=1, pin=True)
            else:
                if j == 1 and h + 1 < NH:
                    emit_kaug(h + 1, pin=True)
                    emit_kaug_oh(h + 1, pin=True)
                elif j == 2 and h + 2 < NH:
                    qaug_h[h + 2] = emit_qstage(h + 2, pin=True)
                elif j == 3 and h + 2 < NH:
                    emit_bias(h + 2, qaug_h[h + 2], halves=(0,), pin=True)
                elif j == 4 and h + 2 < NH:
                    emit_bias(h + 2, qaug_h[h + 2], halves=(1,), pin=True)

        def emit_offload(h, j):
            emit_logits_once((h, j))
            ls = lstgp.tile([128, N], F32, tag="ls", name=f"ls{h}_{j}")
            KMARKS.append((chained("dve", nc.vector.tensor_copy(
                ls, slot(0, 2))).ins.name, f"stage{h}_{j}"))
            ew = expwp.tile([128, N], BF16, tag="ew", name=f"ew{h}_{j}")
            chained("pool", nc.gpsimd.tensor_tensor(
                out=ew, in0=base_bcast(N), in1=ls,
                op=mybir.AluOpType.pow))
            return ew

        def emit_attnv_zero(h):
            # one full-width start=True matmul resets the attn bank;
            # interleaved per-region start writes clobber each other on HW
            pe(nc.tensor.matmul(ps_att_of(h), lhsT=zeros_bf,
                                rhs=qT[:, 0, 0:512],
                                start=True, stop=False))

        def emit_attnv(h, j, ew, start, stop):
            ps_att = ps_att_of(h)
            a_last = None
            for nt in range(8):
                a_last = pe(nc.tensor.matmul(
                    ps_att[:, nt * 64:nt * 64 + 33],
                    lhsT=ew[:, nt * 128:(nt + 1) * 128],
                    rhs=v_aug[:, h, j, :],
                    start=start, stop=stop))
            return a_last

        # pending deferred work from head h-1, emitted inside head h's j1
        # iteration (gives the slow offload pipeline extra time before its
        # attn@V could block the in-order PE queue):
        pending = None

        # pair assignment per tile, rotation continuous across heads
        pair_of = {}
        rot = 0
        for h in range(NH):
            for j in range(8):
                if j in OFFLOAD[h]:
                    pair_of[(h, j)] = (0, 1)
                else:
                    pair_of[(h, j)] = ACT_PAIRS[rot % 2]
                    rot += 1

        emitted_logits = set()

        def next_tile(h, j):
            if j < 7:
                return (h, j + 1)
            return (h + 1, 0) if h + 1 < NH else None

        def emit_logits_once(t):
            if t is not None and t not in emitted_logits:
                emitted_logits.add(t)
                emit_logits(t[0], t[1], pair_of[t][0])

        # head-0 priming: j1/j2 logits first (they gate Act); j0 last --
        # its slots are released only by the trailing qT replica copies,
        # and its Pool-exp pipeline has most of the head as slack
        for t in ((0, 1), (0, 2)):
            emit_logits_once(t)
        for h in range(NH):
            off_js = OFFLOAD[h]
            act_js = [j for j in range(8) if j not in off_js]
            last = NH - 1
            first_j = act_js[0]
            last_j = 7 if h == last else 0
            ews = {}
            for j in range(8):
                # this tile's logits were emitted one iteration ago; emit
                # the NEXT tile's logits before this tile's attn@V so the
                # pinned PE queue never waits an exp to issue logits
                if h == last and j == 7:
                    emit_attnv(h, 0, ews[0], False, False)
                if j in off_js:
                    if j == 0 and h != 1:
                        # h=1's bank (7) drains bias(2) late; its zero is
                        # deferred to (1, j1) so it can't block the chain
                        emit_attnv_zero(h)
                    ews[j] = emit_offload(h, j)
                    emit_logits_once(next_tile(h, j))
                    nt2 = next_tile(h, j)
                    if nt2 is not None:
                        emit_logits_once(next_tile(*nt2))
                else:
                    ew = expwp.tile([128, N], BF16, tag="ew",
                                    name=f"ew{h}_{j}")
                    if (h, j) in ((0, 1), (0, 2)):
                        # two half-exps: half0 unblocks on bias-half-a,
                        # pulling the whole Act stream ~1us earlier
                        for hf in range(2):
                            KMARKS.append((nc.scalar.activation(
                                ew[:, hf * 512:(hf + 1) * 512],
                                slot(pair_of[(h, j)][0] + hf),
                                AF.Exp, scale=SCALE).ins.name,
                                f"exp{h}_{j}h{hf}"))
                    else:
                        KMARKS.append((nc.scalar.activation(
                            ew, slot(pair_of[(h, j)][0], 2),
                            AF.Exp, scale=SCALE).ins.name, f"exp{h}_{j}"))
                    ews[j] = ew
                    # two tiles ahead: logits(t+2) only WAR-waits this
                    # exp's pair, giving the chain ~1us of margin
                    emit_logits_once(next_tile(h, j))
                    nt2 = next_tile(h, j)
                    if nt2 is not None:
                        emit_logits_once(next_tile(*nt2))
                    if j == 1 and pending is not None:
                        ph, defs = pending
                        for idx, (pj, pew) in enumerate(defs):
                            emit_attnv(ph, pj, pew, False,
                                       idx == len(defs) - 1)
                        emit_epilogue(ph)
                        pending = None
                    if h == last and j == 7 and 3 in off_js:
                        emit_attnv(h, 3, ews[3], False, False)
                    if h == 1 and j == 1:
                        emit_attnv_zero(1)
                    a_last = emit_attnv(h, j, ew, False, j == last_j)
                    hook(h, j, a_last)
            if h != last:
                # all deferred attn@V for Pool-exp'd js punts into head
                # h+1's j1 (so a pow still in flight can never block the
                # in-order PE queue at the head boundary)
                pending = (h, [(j, ews[j]) for j in off_js])

        emit_epilogue(NH - 1)


_NC_CACHE = {}


def _build():
    if "nc" in _NC_CACHE:
        return _NC_CACHE["nc"]
    nc = bass.Bass("TRN2", target_bir_lowering=False, debug=False,
                   enable_asserts=True, num_devices=8)
    ins = {
        "x": nc.dram_tensor("x", [N, 768], F32, kind="ExternalInput").ap(),
        "relh": nc.dram_tensor("relh", [63, 32], F32, kind="ExternalInput").ap(),
        "relw": nc.dram_tensor("relw", [63, 32], F32, kind="ExternalInput").ap(),
    }
    outs = {
        "out": nc.dram_tensor("out", [N, 256], F32, kind="ExternalOutput").ap(),
    }
    with tile.TileContext(nc) as tc:
        kernel_body(tc, outs, ins)
    split_multiwaits(nc)
    _NC_CACHE["nc"] = nc
    return nc


def kernel(inputs, key_rel_h, key_rel_w, _trace=False):
    nc = _build()
    x = np.ascontiguousarray(np.asarray(inputs, dtype=np.float32).reshape(8, N, 768))
    rh = np.ascontiguousarray(np.asarray(key_rel_h, dtype=np.float32))
    rw = np.ascontiguousarray(np.asarray(key_rel_w, dtype=np.float32))
    in_maps = [{"x": x[c], "relh": rh, "relw": rw} for c in range(8)]
    res = bass_utils.run_bass_kernel_spmd(
        nc, in_maps, core_ids=list(range(8)), trace=_trace)
    outp = np.stack([r["out"] for r in res.results])
    if _trace:
        kernel.last_results = res
    return outp.reshape(8, 32, 32, 256)


# revision 49
# speedup vs baseline: 1.1155x; 1.0153x over previous
"""AttentionAugmentation2D Trainium2 kernel (v6).

Shapes (hardcoded): B=8, H=W=32, N=1024, NH=8 heads, dk=dv=32 per head.
inputs [8,32,32,768] = q|k|v (256 each), key_rel_h/w [63,32].
Sharding: data-parallel over batch B across the 8 cores.

Math per (batch, head), n=(i,j), m=(i',j') (i = H index):
  logits[n,m] = q[n]@k[m] + q[(j,i)]@rel_h[i'-i+31] + q[(i,j)]@rel_w[i'-i+31]
Both rel terms depend on m only through i', so with
  SWT[u,n] = rel_w[u]@q[(i,j)] + rel_h[u]@q[(j,i)]        (u in [0,63))
  biasT[t,n] = SWT[t+31-i(n), n]                          (shifted windows)
we get  logits^T = K_aug^T.T @ Q_augT  with contraction 64:
  K_aug rows: 0:32 = k^T, 32:64 = onehot[t==i'(m)] ;  Q_aug: [q^T; biasT].

v6 redesign vs v5 (cost-model driven):
 - All PE operands are bf16 (error budget 2e-2 is ~10x away): packed
   bf16 SBUF-to-SBUF DVE copies run in 4x perf mode.
 - attn@V swaps operand roles: the exp-weight chunk ew[:, nt*128:...]
   is the *stationary* matmul operand and v (32 cols + a ones column
   for the softmax row-sums) streams, so each matmul charges only 33
   rows instead of 512.  The output lands n-major, which kills v5's 64
   flush transposes and attn^T staging copies; accumulation uses 8
   sub-bank [128,33] regions spaced 64 cols apart in ONE psum bank,
   single-buffered across heads (each head's epilogue completes before
   the next head's first accumulating matmul needs the regions).
 - biasT is built by TWO accumulating matmuls per i-block (relw-window
   x a q^T block, then relh-window x a stride-32 column view of q^T
   that realizes the (i,j)->(j,i) permutation for free), so no
   permuted-q replica and no separate bias-rhs tile exist; both rhs
   reads come straight from qaug rows 0:32.
 - The softmax exp is split between the Act engine (activation Exp)
   and the Pool engine: gpsimd supports elementwise pow, so
   exp(s*x) = pow(e^s, x) with a stride-0 broadcast base.  gpsimd
   cannot read PSUM, so DVE stages the offloaded logit tiles
   PSUM->SBUF; the offload count balances Act against DVE+Pool.
 - Per-head epilogue: DVE reciprocal over the 8 strided ones-sums,
   one broadcast tensor_tensor multiply normalizing all 8 regions into
   the n-major staging tile, one strided DMA per head.
 - PSUM map: banks 0-5 = one [128,8,512] region manually slotted in
   512-col halves: pair (0,1) serves the Pool-offloaded logit tiles,
   the startup/half-1 transposes, and (heads >= 3) the bias scratch;
   pairs (2,3)/(4,5) alternate (continuously across heads) for the
   Act-exp'd tiles.  Banks 6/7 double-buffer the attn regions across
   heads (each head's bank is reset by ONE full-width start=True
   matmul -- interleaved per-region start writes clobber each other on
   real HW -- and all attn@V matmuls accumulate with start=False);
   bank 7 also hosts heads 0-2's bias scratch and the warm-up dummies
   before head 1's accumulation begins.
 - Engine queue discipline: the Tile scheduler reorders engine streams
   with its internal timing model, so ALL PE/DVE/Pool instructions are
   pinned in emission order with ordering-only deps; logit matmuls are
   emitted two tiles ahead so they only WAR-wait the exp that frees
   their psum pair (~1us of margin); deferred attn@V and the epilogue
   of head h-1 are emitted inside head h's j1 iteration.
 - rel tables are DMA'd in natural [63,32] layout (a transposed DMA
   would cost ~2000 descriptors on the single shared DMA stream, ahead
   of the critical q/k row loads) and transposed on the PE instead.

Toolchain note: walrus codegen only fits ONE semaphore wait in most
TPB instruction structs; split_multiwaits() moves excess waits onto
same-engine InstNoOp carriers (same workaround as v5).
"""

import numpy as np

import concourse.bass as bass
import concourse.mybir as mybir
import concourse.tile as tile
from concourse import bass_utils
from concourse.masks import make_identity
from concourse.tile import add_dep_helper

F32 = mybir.dt.float32
F32R = mybir.dt.float32r
BF16 = mybir.dt.bfloat16
AF = mybir.ActivationFunctionType

NH = 8
N = 1024
DK = 32
SCALE = float(DK) ** -0.5
BASE = float(np.exp(SCALE))

# Per-head tuple of js whose exp runs on Pool (via DVE PSUM->SBUF stage).
# Must be a subset of {0, 3} (those js own psum slot pair (0,1)); their
# attn@V is deferred to head end.
OFFLOAD = {h: (0,) for h in range(NH)}

KMARKS = []   # (inst_name, label) for trace debugging


def split_multiwaits(nc, dma_limit=1):
    """Move excess semaphore waits onto same-engine nop carriers."""
    n_new = 0
    for f in nc.m.functions:
        for blk in f.blocks:
            newlist = []
            for inst in blk.instructions:
                si = getattr(inst, "sync_info", None)
                is_dma = isinstance(inst, mybir.InstDMACopy)
                limit = dma_limit if is_dma else 1
                if si is not None and len(si.on_wait) > limit:
                    waits = list(si.on_wait)
                    for w in waits[:-1]:
                        n_new += 1
                        newlist.append(mybir.InstNoOp(
                            name=f"I-wc{n_new}",
                            ins=[], outs=[],
                            sync_info=mybir.SyncInfo(on_wait=[w], on_update=[]),
                            bass_nofuse=True,
                            engine=inst.engine,
                        ))
                    inst.sync_info = mybir.SyncInfo(
                        on_wait=waits[-1:], on_update=si.on_update)
                newlist.append(inst)
            blk.instructions = newlist
    return n_new


def kernel_body(tc, outs, ins):
    nc = tc.nc
    x = ins["x"]          # [1024, 768] rows n=(i,j), cols q|k|v
    relh = ins["relh"]    # [63, 32]
    relw = ins["relw"]    # [63, 32]
    out = outs["out"]     # [1024, 256]

    with (
        tc.tile_pool(name="persist", bufs=1) as persist,
        tc.tile_pool(name="expw", bufs=10) as expwp,
        tc.tile_pool(name="qaug", bufs=3) as qaugp,
        tc.tile_pool(name="lstg", bufs=3) as lstgp,
        tc.tile_pool(name="psmain", bufs=1, space="PSUM") as psmain,
    ):
        # ---- PSUM map ----
        ps_all = psmain.tile([128, 8, 512], F32, tag="ps")
        ps_flat = ps_all.rearrange("p s c -> p (s c)")

        def slot(s, n=1):
            return ps_flat[:, s * 512:(s + n) * 512]
        def ps_att_of(h):
            return slot(6 + h % 2)
        ps_bias = slot(7)

        # ---------------- DMAs (the DMA stream is serial in practice:
        # critical q rows first, tiny rel loads sandwiched, v last) ----
        rowsR = persist.tile([128, 4, 8, 128], F32R)
        CB_COLS = (0, 256, 128, 384)   # q0, k0, q1, k1

        def emit_rows_dma(cb, eng, lo=0, hi=8):
            src = bass.AP(tensor=x.tensor,
                          offset=CB_COLS[cb] + lo * 128 * 768,
                          ap=[[768, 128], [128 * 768, hi - lo], [1, 128]],
                          ).bitcast(F32R)
            eng.dma_start(out=rowsR[:, cb, lo:hi, :], in_=src)

        rel_nat = persist.tile([64, 64], F32R)
        nc.vector.memset(rel_nat.bitcast(F32), 0.0)
        v_st = persist.tile([128, 8, 256], F32)

        def emit_v_dma(j):
            nc.sync.dma_start(
                out=v_st[:, j, :], in_=x[j * 128:(j + 1) * 128, 512:768])

        # ALL input DMAs ride the SP queue: a dma_start costs ~667ns of
        # SEQUENCER time on the issuing engine, and the Act sequencer must
        # stay free to issue exps; a single queue also makes the serial
        # DMA stream follow emission order exactly
        emit_rows_dma(0, nc.sync, 0, 4)          # q half0 lo
        nc.sync.dma_start(out=rel_nat[0:63, 0:32], in_=relw.bitcast(F32R))
        emit_rows_dma(0, nc.sync, 4, 8)          # q half0 hi
        nc.sync.dma_start(out=rel_nat[0:63, 32:64], in_=relh.bitcast(F32R))
        emit_rows_dma(1, nc.sync, 0, 4)          # k half0 lo
        emit_v_dma(0)
        emit_rows_dma(1, nc.sync, 4, 8)          # k half0 hi
        emit_v_dma(1)
        emit_v_dma(2)
        emit_rows_dma(2, nc.sync)                # q half1
        emit_rows_dma(3, nc.sync)                # k half1
        for j in range(3, 8):
            emit_v_dma(j)

        # ---------------- warm-up + constants ----------------
        _chain = {}
        dummy_sb = persist.tile([128, 64], F32)
        nc.vector.memset(dummy_sb, 0.0)
        for w in range(20):
            _dm = nc.tensor.matmul(ps_bias[64:96, 0:32],
                                   lhsT=dummy_sb[:, 0:32],
                                   rhs=dummy_sb[:, 0:32],
                                   start=True, stop=True)
            _chain.setdefault("pe", _dm)
            if _chain["pe"] is not _dm:
                add_dep_helper(_dm.ins, _chain["pe"].ins, sync=False,
                               reason="pin pe queue order")
            _chain["pe"] = _dm

        zeros_bf = persist.tile([128, 128], BF16)
        nc.vector.memset(zeros_bf, 0.0)
        # pre-warm the Act engine's Exp table (the first activation pays
        # a ~1.3us table load otherwise -- on the critical startup path)
        act_warm = persist.tile([128, 1], F32)
        nc.scalar.activation(act_warm, dummy_sb[:, 0:1], AF.Exp, scale=SCALE)

        ident = persist.tile([128, 128], F32)
        make_identity(nc, ident)
        identR = persist.tile([128, 128], F32R)
        nc.vector.tensor_copy(identR, ident)
        ident_marker = nc.gpsimd.tensor_copy(ident[0:1, 0:1], ident[0:1, 0:1])

        # pow base for the Pool exp share: pow(e^s, x) = exp(s*x)
        base_t = persist.tile([128, 1], F32)
        nc.vector.memset(base_t, BASE)

        def base_bcast(cols):
            return bass.AP(tensor=base_t.tensor, offset=base_t.offset,
                           ap=[list(base_t.ap[0]), [0, cols]])

        # Pin Pool and startup-DVE queue order (the Tile scheduler's
        # internal timing model reorders engine queues badly otherwise).
        _chain["pool"] = ident_marker

        def chained(engine_name, inst):
            prev = _chain.get(engine_name)
            if prev is not None:
                add_dep_helper(inst.ins, prev.ins, sync=False,
                               reason=f"pin {engine_name} queue order")
            _chain[engine_name] = inst
            return inst

        def pool(op, *args, **kwargs):
            return chained("pool", getattr(nc.gpsimd, op)(*args, **kwargs))

        def pe(inst):
            return chained("pe", inst)

        # onehot rows for K_aug (bf16) -- no input deps, head of Pool chain
        oh_st = persist.tile([32, 8, 128], F32)
        pool("memset", oh_st, 0.0)
        oh = oh_st.rearrange("t j (b m) -> t j b m", b=4)
        pool("affine_select",
             out=oh, in_=oh, compare_op=mybir.AluOpType.not_equal,
             fill=1.0, base=0, pattern=[[-4, 8], [-1, 4], [0, 32]],
             channel_multiplier=1)
        oh_bf = persist.tile([32, 8, 128], BF16)
        pool("tensor_copy", oh_bf, oh_st)

        # bf16 transposed replicas of q and k: [p=32*(h%4)+d, half, n]
        qT = persist.tile([128, 2, N], BF16)
        kT = persist.tile([128, 2, N], BF16)

        ka = [persist.tile([64, 8, 128], BF16, tag=f"ka{i}", name=f"ka{i}")
              for i in range(4)]

        # rel^T: [32, 2, 63] at partitions 0:32 (PE operands must share a
        # base partition): [:,0,:] = relw^T, [:,1,:] = relh^T
        rel_bf = persist.tile([32, 2, 63], BF16)

        def emit_transposes(cb, dst, half, lo, s, pin=False, pe_anchor=None):
            """4 transposes into one psum slot + one wide copy into the
            bf16 replica."""
            pt = slot(s).bitcast(F32R)
            for c in range(4):
                tr = pe(nc.tensor.transpose(pt[:, c * 128:(c + 1) * 128],
                                            rowsR[:, cb, lo + c, :], identR))
            cp = nc.vector.tensor_copy(
                dst[:, half, lo * 128:(lo + 4) * 128], slot(s))
            if pin:
                chained("dve", cp)

        # ---------------- per-head staging ----------------
        def emit_qstage(h, pin=False):
            """qaug rows 0:32 = q^T for head h (bf16 4x copy)."""
            lane = (h % 4) * 32
            qsT = qT[lane:lane + 32, h // 4, :]
            qaug = qaugp.tile([64, N], BF16, tag="qaug", name=f"qaug{h}")
            c1 = nc.vector.tensor_copy(qaug[0:32, :], qsT)
            if pin:
                chained("dve", c1)
            return qaug

        def emit_bias(h, qaug, halves=(0, 1), pin=False, pe_anchor=None):
            """biasT[t,(i,j)] = SWT[t+31-i,(i,j)]: per i-block, two
            accumulating matmuls with shifted rel windows; the relh term
            reads q^T through a stride-32 column view (the (i,j)->(j,i)
            permutation).  Scratch: heads 0-2 use bank 7 halves at
            partitions 0:32/32:64 (bank 7 becomes attn-B from head 1's
            accumulation on); later heads use the idle offload slots."""
            for half in halves:
                if h <= 2:
                    ps_b = ps_bias[half * 32:half * 32 + 32, :]
                else:
                    ps_b = slot(half)[0:32, :]
                for ib in range(16):
                    i = half * 16 + ib
                    pe(nc.tensor.matmul(
                        ps_b[:, ib * 32:(ib + 1) * 32],
                        lhsT=rel_bf[:, 0, 31 - i:63 - i],
                        rhs=qaug[0:32, i * 32:(i + 1) * 32],
                        start=True, stop=False))
                    perm_rhs = bass.AP(
                        tensor=qaug.tensor, offset=qaug.offset + i,
                        ap=[[qaug.ap[0][0], 32], [32, 32]])
                    pe(nc.tensor.matmul(
                        ps_b[:, ib * 32:(ib + 1) * 32],
                        lhsT=rel_bf[:, 1, 31 - i:63 - i],
                        rhs=perm_rhs,
                        start=False, stop=True))
                cp = nc.vector.tensor_copy(
                    qaug[32:64, half * 512:(half + 1) * 512], ps_b)
                if pin:
                    chained("dve", cp)

        def emit_kaug(h, pin=False, los=(0, 8)):
            lane = (h % 4) * 32
            lo, hi = los
            ksT = kT[lane:lane + 32, h // 4, lo * 128:hi * 128]
            cp = nc.vector.tensor_copy(
                ka[h % 4][0:32, lo:hi].rearrange("d j m -> d (j m)"), ksT)
            if pin:
                chained("dve", cp)

        def emit_kaug_oh(h, pin=False):
            if h < 4:
                co = nc.vector.tensor_copy(ka[h % 4][32:64], oh_bf)
                if pin:
                    chained("dve", co)

        # ---------------- v staging ----------------
        v_aug = persist.tile([128, NH, 8, 33], BF16)
        ones_st = persist.tile([128, 64], F32)
        nc.vector.memset(ones_st, 1.0)

        def emit_vconv(j):
            pool("tensor_copy",
                 v_aug[:, :, j, 0:32],
                 v_st[:, j, :].rearrange("p (h d) -> p h d", h=NH))

        nc.vector.tensor_copy(
            v_aug[:, :, :, 32:33].rearrange("p h j o -> p (h j o)"), ones_st)

        # ---------------- epilogue ----------------
        out_sb = persist.tile([128, 8, 256], F32)
        rec_t = persist.tile([128, NH, 8], F32)

        def emit_epilogue(h):
            """reciprocal of the 8 ones-sums + one broadcast normalize of
            the 8 [128,33] regions into out_sb; one strided DMA."""
            ps_att = ps_att_of(h)
            rec = rec_t[:, h, :]
            sums_ap = bass.AP(tensor=ps_att.tensor, offset=ps_att.offset + 32,
                              ap=[list(ps_att.ap[0]), [64, 8]])
            chained("dve", nc.vector.reciprocal(rec, sums_ap))
            in0 = bass.AP(tensor=ps_att.tensor, offset=ps_att.offset,
                          ap=[list(ps_att.ap[0]), [64, 8], [1, 32]])
            in1 = bass.AP(tensor=rec.tensor, offset=rec.offset,
                          ap=[list(rec.ap[0]), [1, 8], [0, 32]])
            out_ap = bass.AP(tensor=out_sb.tensor,
                             offset=out_sb.offset + h * 32,
                             ap=[list(out_sb.ap[0]), [256, 8], [1, 32]])
            chained("dve", nc.vector.tensor_tensor(
                out=out_ap, in0=in0, in1=in1, op=mybir.AluOpType.mult))
            groups = ((0, 8),)
            for glo, ghi in groups:
                dstap = bass.AP(
                    tensor=out.tensor,
                    offset=glo * 128 * 256 + h * 32,
                    ap=[[256, 128], [128 * 256, ghi - glo], [1, 32]])
                nc.sync.dma_start(out=dstap,
                                  in_=out_sb[:, glo:ghi, h * 32:(h + 1) * 32])

        # ---------------- startup ----------------
        # PE order: dummies, rel-w transpose, q transposes, rel-h
        # transpose, k transposes, bias mms, logits.  The DVE chain IS
        # the head-0 critical path: qaug0 and ka0 rows are copied
        # straight out of the transpose psum slots (lane 0); the qT
        # replica copies (for later heads) trail behind and delay only
        # head 0's j0, whose exp is Pool-offloaded and slack-tolerant.
        # Slot use: q-lo->0, q-hi->1, rel->4 (cols 0:127), k-lo->2,
        # k-hi->5; slot 3 stays free for j1's logits.
        relT = slot(4).bitcast(F32R)
        pe(nc.tensor.transpose(relT[0:32, 0:64],
                               rel_nat[:, 0:32], identR[0:64, 0:64]))

        def transp4(cb, lo, s):
            pt = slot(s).bitcast(F32R)
            for c in range(4):
                pe(nc.tensor.transpose(pt[:, c * 128:(c + 1) * 128],
                                       rowsR[:, cb, lo + c, :], identR))

        transp4(0, 0, 0)                                # q half0 lo
        pe(nc.tensor.transpose(relT[0:32, 64:128],
                               rel_nat[:, 32:64], identR[0:64, 0:64]))
        transp4(0, 4, 1)                                # q half0 hi
        transp4(1, 0, 2)                                # k half0 lo

        chained("dve", nc.vector.tensor_copy(
            rel_bf.rearrange("p a u -> p (a u)"),
            bass.AP(tensor=ps_flat.tensor,
                    offset=ps_flat.offset + 4 * 512,
                    ap=[[ps_flat.ap[0][0], 32], [64, 2], [1, 63]])))
        qaug_h = {0: qaugp.tile([64, N], BF16, tag="qaug", name="qaug0")}
        chained("dve", nc.vector.tensor_copy(
            qaug_h[0][0:32, 0:512], slot(0)[0:32, :]))
        chained("dve", nc.vector.tensor_copy(
            qaug_h[0][0:32, 512:1024], slot(1)[0:32, :]))
        chained("dve", nc.vector.tensor_copy(
            ka[0][0:32, 0:4].rearrange("d j m -> d (j m)"),
            slot(2)[0:32, :]))
        # ka[1] rows straight off the same transpose slots (lane 1 =
        # psum partitions 32:64; DVE copies may shift partitions)
        chained("dve", nc.vector.tensor_copy(
            ka[1][0:32, 0:4].rearrange("d j m -> d (j m)"),
            slot(2)[32:64, :]))
        emit_kaug_oh(0, pin=True)
        emit_bias(0, qaug_h[0], halves=(0,), pin=True)
        emit_bias(0, qaug_h[0], halves=(1,), pin=True)
        transp4(1, 4, 5)                                # k half0 hi
        chained("dve", nc.vector.tensor_copy(
            ka[0][0:32, 4:8].rearrange("d j m -> d (j m)"),
            slot(5)[0:32, :]))
        chained("dve", nc.vector.tensor_copy(
            ka[1][0:32, 4:8].rearrange("d j m -> d (j m)"),
            slot(5)[32:64, :]))
        pool("tensor_copy", ka[1][32:64], oh_bf)
        # trailing (delays only head 0's slack-tolerant j0): qT replica
        chained("dve", nc.vector.tensor_copy(qT[:, 0, 0:512], slot(0)))
        chained("dve", nc.vector.tensor_copy(qT[:, 0, 512:1024], slot(1)))
        for j in range(8):
            emit_vconv(j)

        # ---------------- main pipeline ----------------
        ACT_PAIRS = ((2, 3), (4, 5))
        act_rot = [0]   # continuous pair rotation across heads

        def emit_logits(h, j, lo_slot):
            qaug = qaug_h[h]
            for half in range(2):
                mm = pe(nc.tensor.matmul(
                    slot(lo_slot + half), lhsT=ka[h % 4][:, j, :],
                    rhs=qaug[:, half * 512:(half + 1) * 512],
                    start=True, stop=True))
                KMARKS.append((mm.ins.name, f"logits{h}_{j}_h{half}_s{lo_slot+half}"))

        def hook(h, j, a):
            if h == 0:
                if j == 2:
                    emit_transposes(2, qT, 1, 0, s=0, pin=True)
                elif j == 3:
                    emit_transposes(2, qT, 1, 4, s=1, pin=True)
                    qaug_h[1] = emit_qstage(1, pin=True)
                    emit_bias(1, qaug_h[1], halves=(0,), pin=True)
                elif j == 4:
                    # kT half0 replica: re-transpose (startup slots were
                    # drained into ka0/ka1 directly)
                    emit_transposes(1, kT, 0, 0, s=0, pin=True)
                    emit_bias(1, qaug_h[1], halves=(1,), pin=True)
                elif j == 5:
                    emit_transposes(1, kT, 0, 4, s=0, pin=True)
                elif j == 6:
                    qaug_h[2] = emit_qstage(2, pin=True)
                    emit_bias(2, qaug_h[2], halves=(0,), pin=True)
                elif j == 7:
                    emit_bias(2, qaug_h[2], halves=(1,), pin=True)
            elif h == 1:
                if j == 1:
                    emit_transposes(3, kT, 1, 0, s=0, pin=True)
                    emit_kaug(2, pin=True)
                    emit_kaug_oh(2, pin=True)
                elif j == 2:
                    qaug_h[3] = emit_qstage(3, pin=True)
                elif j == 3:
                    emit_bias(3, qaug_h[3], halves=(0,), pin=True)
                elif j == 4:
                    emit_bias(3, qaug_h[3], halves=(1,), pin=True)
                elif j == 5:
                    emit_transposes(3, kT, 1, 4, s=1, pin=True)
            else:
                if j == 1 and h + 1 < NH:
                    emit_kaug(h + 1, pin=True)
                    emit_kaug_oh(h + 1, pin=True)
                elif j == 2 and h + 2 < NH:
                    qaug_h[h + 2] = emit_qstage(h + 2, pin=True)
                elif j == 3 and h + 2 < NH:
                    emit_bias(h + 2, qaug_h[h + 2], halves=(0,), pin=True)
                elif j == 4 and h + 2 < NH:
                    emit_bias(h + 2, qaug_h[h + 2], halves=(1,), pin=True)

        def emit_offload(h, j):
            emit_logits_once((h, j))
            ls = lstgp.tile([128, N], F32, tag="ls", name=f"ls{h}_{j}")
            KMARKS.append((chained("dve", nc.vector.tensor_copy(
                ls, slot(0, 2))).ins.name, f"stage{h}_{j}"))
            ew = expwp.tile([128, N], BF16, tag="ew", name=f"ew{h}_{j}")
            chained("pool", nc.gpsimd.tensor_tensor(
                out=ew, in0=base_bcast(N), in1=ls,
                op=mybir.AluOpType.pow))
            return ew

        def emit_attnv_zero(h):
            # one full-width start=True matmul resets the attn bank;
            # interleaved per-region start writes clobber each other on HW
            pe(nc.tensor.matmul(ps_att_of(h), lhsT=zeros_bf,
                                rhs=qT[:, 0, 0:512],
                                start=True, stop=False))

        def emit_attnv(h, j, ew, start, stop):
            ps_att = ps_att_of(h)
            a_last = None
            for nt in range(8):
                a_last = pe(nc.tensor.matmul(
                    ps_att[:, nt * 64:nt * 64 + 33],
                    lhsT=ew[:, nt * 128:(nt + 1) * 128],
                    rhs=v_aug[:, h, j, :],
                    start=start, stop=stop))
            return a_last

        # pending deferred work from head h-1, emitted inside head h's j1
        # iteration (gives the slow offload pipeline extra time before its
        # attn@V could block the in-order PE queue):
        pending = None

        # pair assignment per tile, rotation continuous across heads
        pair_of = {}
        rot = 0
        for h in range(NH):
            for j in range(8):
                if j in OFFLOAD[h]:
                    pair_of[(h, j)] = (0, 1)
                else:
                    pair_of[(h, j)] = ACT_PAIRS[rot % 2]
                    rot += 1

        emitted_logits = set()

        def next_tile(h, j):
            if j < 7:
                return (h, j + 1)
            return (h + 1, 0) if h + 1 < NH else None

        def emit_logits_once(t):
            if t is not None and t not in emitted_logits:
                emitted_logits.add(t)
                emit_logits(t[0], t[1], pair_of[t][0])

        # head-0 priming: j1/j2 logits first (they gate Act); j0 last --
        # its slots are released only by the trailing qT replica copies,
        # and its Pool-exp pipeline has most of the head as slack
        for t in ((0, 1), (0, 2)):
            emit_logits_once(t)
        for h in range(NH):
            off_js = OFFLOAD[h]
            act_js = [j for j in range(8) if j not in off_js]
            last = NH - 1
            first_j = act_js[0]
            last_j = 7 if h == last else 0
            ews = {}
            for j in range(8):
                # this tile's logits were emitted one iteration ago; emit
                # the NEXT tile's logits before this tile's attn@V so the
                # pinned PE queue never waits an exp to issue logits
                if h == last and j == 7:
                    emit_attnv(h, 0, ews[0], False, False)
                if j in off_js:
                    if j == 0 and h != 1:
                        # h=1's bank (7) drains bias(2) late; its zero is
                        # deferred to (1, j1) so it can't block the chain
                        emit_attnv_zero(h)
                    ews[j] = emit_offload(h, j)
                    emit_logits_once(next_tile(h, j))
                    nt2 = next_tile(h, j)
                    if nt2 is not None:
                        emit_logits_once(next_tile(*nt2))
                else:
                    ew = expwp.tile([128, N], BF16, tag="ew",
                                    name=f"ew{h}_{j}")
                    if (h, j) in ((0, 1), (0, 2)):
                        # two half-exps: half0 unblocks on bias-half-a,
                        # pulling the whole Act stream ~1us earlier
                        for hf in range(2):
                            KMARKS.append((nc.scalar.activation(
                                ew[:, hf * 512:(hf + 1) * 512],
                                slot(pair_of[(h, j)][0] + hf),
                                AF.Exp, scale=SCALE).ins.name,
                                f"exp{h}_{j}h{hf}"))
                    else:
                        KMARKS.append((nc.scalar.activation(
                            ew, slot(pair_of[(h, j)][0], 2),
                            AF.Exp, scale=SCALE).ins.name, f"exp{h}_{j}"))
                    ews[j] = ew
                    # two tiles ahead: logits(t+2) only WAR-waits this
                    # exp's pair, giving the chain ~1us of margin
                    emit_logits_once(next_tile(h, j))
                    nt2 = next_tile(h, j)
                    if nt2 is not None:
                        emit_logits_once(next_tile(*nt2))
                    if j == 1 and pending is not None:
                        ph, defs = pending
                        for idx, (pj, pew) in enumerate(defs):
                            emit_attnv(ph, pj, pew, False,
                                       idx == len(defs) - 1)
                        emit_epilogue(ph)
                        pending = None
                    if h == last and j == 7 and 3 in off_js:
                        emit_attnv(h, 3, ews[3], False, False)
                    if h == 1 and j == 1:
                        emit_attnv_zero(1)
                    a_last = emit_attnv(h, j, ew, False, j == last_j)
                    hook(h, j, a_last)
            if h != last:
                # all deferred attn@V for Pool-exp'd js punts into head
                # h+1's j1 (so a pow still in flight can never block the
                # in-order PE queue at the head boundary)
                pending = (h, [(j, ews[j]) for j in off_js])

        emit_epilogue(NH - 1)


_NC_CACHE = {}


def _build():
    if "nc" in _NC_CACHE:
        return _NC_CACHE["nc"]
    nc = bass.Bass("TRN2", target_bir_lowering=False, debug=False,
                   enable_asserts=True, num_devices=8)
    ins = {
        "x": nc.dram_tensor("x", [N, 768], F32, kind="ExternalInput").ap(),
        "relh": nc.dram_tensor("relh", [63, 32], F32, kind="ExternalInput").ap(),
        "relw": nc.dram_tensor("relw", [63, 32], F32, kind="ExternalInput").ap(),
    }
    outs = {
        "out": nc.dram_tensor("out", [N, 256], F32, kind="ExternalOutput").ap(),
    }
    with tile.TileContext(nc) as tc:
        kernel_body(tc, outs, ins)
    split_multiwaits(nc)
    _NC_CACHE["nc"] = nc
    return nc


def kernel(inputs, key_rel_h, key_rel_w, _trace=False):
    nc = _build()
    x = np.ascontiguousarray(np.asarray(inputs, dtype=np.float32).reshape(8, N, 768))
    rh = np.ascontiguousarray(np.asarray(key_rel_h, dtype=np.float32))
    rw = np.ascontiguousarray(np.asarray(key_rel_w, dtype=np.float32))
    in_maps = [{"x": x[c], "relh": rh, "relw": rw} for c in range(8)]
    res = bass_utils.run_bass_kernel_spmd(
        nc, in_maps, core_ids=list(range(8)), trace=_trace)
    outp = np.stack([r["out"] for r in res.results])
    if _trace:
        kernel.last_results = res
    return outp.reshape(8, 32, 32, 256)


# revision 50
# speedup vs baseline: 1.1195x; 1.0035x over previous
"""AttentionAugmentation2D Trainium2 kernel (v6).

Shapes (hardcoded): B=8, H=W=32, N=1024, NH=8 heads, dk=dv=32 per head.
inputs [8,32,32,768] = q|k|v (256 each), key_rel_h/w [63,32].
Sharding: data-parallel over batch B across the 8 cores.

Math per (batch, head), n=(i,j), m=(i',j') (i = H index):
  logits[n,m] = q[n]@k[m] + q[(j,i)]@rel_h[i'-i+31] + q[(i,j)]@rel_w[i'-i+31]
Both rel terms depend on m only through i', so with
  SWT[u,n] = rel_w[u]@q[(i,j)] + rel_h[u]@q[(j,i)]        (u in [0,63))
  biasT[t,n] = SWT[t+31-i(n), n]                          (shifted windows)
we get  logits^T = K_aug^T.T @ Q_augT  with contraction 64:
  K_aug rows: 0:32 = k^T, 32:64 = onehot[t==i'(m)] ;  Q_aug: [q^T; biasT].

v6 redesign vs v5 (cost-model driven):
 - All PE operands are bf16 (error budget 2e-2 is ~10x away): packed
   bf16 SBUF-to-SBUF DVE copies run in 4x perf mode.
 - attn@V swaps operand roles: the exp-weight chunk ew[:, nt*128:...]
   is the *stationary* matmul operand and v (32 cols + a ones column
   for the softmax row-sums) streams, so each matmul charges only 33
   rows instead of 512.  The output lands n-major, which kills v5's 64
   flush transposes and attn^T staging copies; accumulation uses 8
   sub-bank [128,33] regions spaced 64 cols apart in ONE psum bank,
   single-buffered across heads (each head's epilogue completes before
   the next head's first accumulating matmul needs the regions).
 - biasT is built by TWO accumulating matmuls per i-block (relw-window
   x a q^T block, then relh-window x a stride-32 column view of q^T
   that realizes the (i,j)->(j,i) permutation for free), so no
   permuted-q replica and no separate bias-rhs tile exist; both rhs
   reads come straight from qaug rows 0:32.
 - The softmax exp is split between the Act engine (activation Exp)
   and the Pool engine: gpsimd supports elementwise pow, so
   exp(s*x) = pow(e^s, x) with a stride-0 broadcast base.  gpsimd
   cannot read PSUM, so DVE stages the offloaded logit tiles
   PSUM->SBUF; the offload count balances Act against DVE+Pool.
 - Per-head epilogue: DVE reciprocal over the 8 strided ones-sums,
   one broadcast tensor_tensor multiply normalizing all 8 regions into
   the n-major staging tile, one strided DMA per head.
 - PSUM map: banks 0-5 = one [128,8,512] region manually slotted in
   512-col halves: pair (0,1) serves the Pool-offloaded logit tiles,
   the startup/half-1 transposes, and (heads >= 3) the bias scratch;
   pairs (2,3)/(4,5) alternate (continuously across heads) for the
   Act-exp'd tiles.  Banks 6/7 double-buffer the attn regions across
   heads (each head's bank is reset by ONE full-width start=True
   matmul -- interleaved per-region start writes clobber each other on
   real HW -- and all attn@V matmuls accumulate with start=False);
   bank 7 also hosts heads 0-2's bias scratch and the warm-up dummies
   before head 1's accumulation begins.
 - Engine queue discipline: the Tile scheduler reorders engine streams
   with its internal timing model, so ALL PE/DVE/Pool instructions are
   pinned in emission order with ordering-only deps; logit matmuls are
   emitted two tiles ahead so they only WAR-wait the exp that frees
   their psum pair (~1us of margin); deferred attn@V and the epilogue
   of head h-1 are emitted inside head h's j1 iteration.
 - rel tables are DMA'd in natural [63,32] layout (a transposed DMA
   would cost ~2000 descriptors on the single shared DMA stream, ahead
   of the critical q/k row loads) and transposed on the PE instead.

Toolchain note: walrus codegen only fits ONE semaphore wait in most
TPB instruction structs; split_multiwaits() moves excess waits onto
same-engine InstNoOp carriers (same workaround as v5).
"""

import numpy as np

import concourse.bass as bass
import concourse.mybir as mybir
import concourse.tile as tile
from concourse import bass_utils
from concourse.masks import make_identity
from concourse.tile import add_dep_helper

F32 = mybir.dt.float32
F32R = mybir.dt.float32r
BF16 = mybir.dt.bfloat16
AF = mybir.ActivationFunctionType

NH = 8
N = 1024
DK = 32
SCALE = float(DK) ** -0.5
BASE = float(np.exp(SCALE))

# Per-head tuple of js whose exp runs on Pool (via DVE PSUM->SBUF stage).
# Must be a subset of {0, 3} (those js own psum slot pair (0,1)); their
# attn@V is deferred to head end.
OFFLOAD = {h: (0,) for h in range(NH)}

KMARKS = []   # (inst_name, label) for trace debugging


def split_multiwaits(nc, dma_limit=1):
    """Move excess semaphore waits onto same-engine nop carriers."""
    n_new = 0
    for f in nc.m.functions:
        for blk in f.blocks:
            newlist = []
            for inst in blk.instructions:
                si = getattr(inst, "sync_info", None)
                is_dma = isinstance(inst, mybir.InstDMACopy)
                limit = dma_limit if is_dma else 1
                if si is not None and len(si.on_wait) > limit:
                    waits = list(si.on_wait)
                    for w in waits[:-1]:
                        n_new += 1
                        newlist.append(mybir.InstNoOp(
                            name=f"I-wc{n_new}",
                            ins=[], outs=[],
                            sync_info=mybir.SyncInfo(on_wait=[w], on_update=[]),
                            bass_nofuse=True,
                            engine=inst.engine,
                        ))
                    inst.sync_info = mybir.SyncInfo(
                        on_wait=waits[-1:], on_update=si.on_update)
                newlist.append(inst)
            blk.instructions = newlist
    return n_new


def kernel_body(tc, outs, ins):
    nc = tc.nc
    x = ins["x"]          # [1024, 768] rows n=(i,j), cols q|k|v
    relh = ins["relh"]    # [63, 32]
    relw = ins["relw"]    # [63, 32]
    out = outs["out"]     # [1024, 256]

    with (
        tc.tile_pool(name="persist", bufs=1) as persist,
        tc.tile_pool(name="expw", bufs=10) as expwp,
        tc.tile_pool(name="qaug", bufs=3) as qaugp,
        tc.tile_pool(name="lstg", bufs=3) as lstgp,
        tc.tile_pool(name="psmain", bufs=1, space="PSUM") as psmain,
    ):
        # ---- PSUM map ----
        ps_all = psmain.tile([128, 8, 512], F32, tag="ps")
        ps_flat = ps_all.rearrange("p s c -> p (s c)")

        def slot(s, n=1):
            return ps_flat[:, s * 512:(s + n) * 512]
        def ps_att_of(h):
            return slot(6 + h % 2)
        ps_bias = slot(7)

        # ---------------- DMAs (the DMA stream is serial in practice:
        # critical q rows first, tiny rel loads sandwiched, v last) ----
        rowsR = persist.tile([128, 4, 8, 128], F32R)
        CB_COLS = (0, 256, 128, 384)   # q0, k0, q1, k1

        def emit_rows_dma(cb, eng, lo=0, hi=8):
            src = bass.AP(tensor=x.tensor,
                          offset=CB_COLS[cb] + lo * 128 * 768,
                          ap=[[768, 128], [128 * 768, hi - lo], [1, 128]],
                          ).bitcast(F32R)
            eng.dma_start(out=rowsR[:, cb, lo:hi, :], in_=src)

        rel_nat = persist.tile([64, 64], F32R)
        nc.vector.memset(rel_nat.bitcast(F32), 0.0)
        v_st = persist.tile([128, 8, 256], F32)

        def emit_v_dma(j):
            nc.sync.dma_start(
                out=v_st[:, j, :], in_=x[j * 128:(j + 1) * 128, 512:768])

        # ALL input DMAs ride the SP queue: a dma_start costs ~667ns of
        # SEQUENCER time on the issuing engine, and the Act sequencer must
        # stay free to issue exps; a single queue also makes the serial
        # DMA stream follow emission order exactly
        emit_rows_dma(0, nc.sync, 0, 4)          # q half0 lo
        nc.sync.dma_start(out=rel_nat[0:63, 0:32], in_=relw.bitcast(F32R))
        emit_rows_dma(0, nc.sync, 4, 8)          # q half0 hi
        nc.sync.dma_start(out=rel_nat[0:63, 32:64], in_=relh.bitcast(F32R))
        emit_rows_dma(1, nc.sync, 0, 4)          # k half0 lo
        emit_v_dma(0)
        emit_rows_dma(1, nc.sync, 4, 8)          # k half0 hi
        emit_v_dma(1)
        emit_v_dma(2)
        emit_rows_dma(2, nc.sync)                # q half1
        emit_rows_dma(3, nc.sync)                # k half1
        for j in range(3, 8):
            emit_v_dma(j)

        # ---------------- warm-up + constants ----------------
        _chain = {}
        dummy_sb = persist.tile([128, 64], F32)
        nc.vector.memset(dummy_sb, 0.0)
        for w in range(20):
            _dm = nc.tensor.matmul(ps_bias[64:96, 0:32],
                                   lhsT=dummy_sb[:, 0:32],
                                   rhs=dummy_sb[:, 0:32],
                                   start=True, stop=True)
            _chain.setdefault("pe", _dm)
            if _chain["pe"] is not _dm:
                add_dep_helper(_dm.ins, _chain["pe"].ins, sync=False,
                               reason="pin pe queue order")
            _chain["pe"] = _dm

        zeros_bf = persist.tile([128, 128], BF16)
        nc.vector.memset(zeros_bf, 0.0)
        # pre-warm the Act engine's Exp table (the first activation pays
        # a ~1.3us table load otherwise -- on the critical startup path)
        act_warm = persist.tile([128, 1], F32)
        nc.scalar.activation(act_warm, dummy_sb[:, 0:1], AF.Exp, scale=SCALE)

        ident = persist.tile([128, 128], F32)
        make_identity(nc, ident)
        identR = persist.tile([128, 128], F32R)
        nc.vector.tensor_copy(identR, ident)
        ident_marker = nc.gpsimd.tensor_copy(ident[0:1, 0:1], ident[0:1, 0:1])

        # pow base for the Pool exp share: pow(e^s, x) = exp(s*x)
        base_t = persist.tile([128, 1], F32)
        nc.vector.memset(base_t, BASE)

        def base_bcast(cols):
            return bass.AP(tensor=base_t.tensor, offset=base_t.offset,
                           ap=[list(base_t.ap[0]), [0, cols]])

        # Pin Pool and startup-DVE queue order (the Tile scheduler's
        # internal timing model reorders engine queues badly otherwise).
        _chain["pool"] = ident_marker

        def chained(engine_name, inst):
            prev = _chain.get(engine_name)
            if prev is not None:
                add_dep_helper(inst.ins, prev.ins, sync=False,
                               reason=f"pin {engine_name} queue order")
            _chain[engine_name] = inst
            return inst

        def pool(op, *args, **kwargs):
            return chained("pool", getattr(nc.gpsimd, op)(*args, **kwargs))

        def pe(inst):
            return chained("pe", inst)

        # onehot rows for K_aug (bf16) -- no input deps, head of Pool chain
        oh_st = persist.tile([32, 8, 128], F32)
        pool("memset", oh_st, 0.0)
        oh = oh_st.rearrange("t j (b m) -> t j b m", b=4)
        pool("affine_select",
             out=oh, in_=oh, compare_op=mybir.AluOpType.not_equal,
             fill=1.0, base=0, pattern=[[-4, 8], [-1, 4], [0, 32]],
             channel_multiplier=1)
        oh_bf = persist.tile([32, 8, 128], BF16)
        pool("tensor_copy", oh_bf, oh_st)

        # bf16 transposed replicas of q and k: [p=32*(h%4)+d, half, n]
        qT = persist.tile([128, 2, N], BF16)
        kT = persist.tile([128, 2, N], BF16)

        ka = [persist.tile([64, 8, 128], BF16, tag=f"ka{i}", name=f"ka{i}")
              for i in range(4)]

        # rel^T: [32, 2, 63] at partitions 0:32 (PE operands must share a
        # base partition): [:,0,:] = relw^T, [:,1,:] = relh^T
        rel_bf = persist.tile([32, 2, 63], BF16)

        def emit_transposes(cb, dst, half, lo, s, pin=False, pe_anchor=None):
            """4 transposes into one psum slot + one wide copy into the
            bf16 replica."""
            pt = slot(s).bitcast(F32R)
            for c in range(4):
                tr = pe(nc.tensor.transpose(pt[:, c * 128:(c + 1) * 128],
                                            rowsR[:, cb, lo + c, :], identR))
            cp = nc.vector.tensor_copy(
                dst[:, half, lo * 128:(lo + 4) * 128], slot(s))
            if pin:
                chained("dve", cp)

        # ---------------- per-head staging ----------------
        def emit_qstage(h, pin=False):
            """qaug rows 0:32 = q^T for head h (bf16 4x copy)."""
            lane = (h % 4) * 32
            qsT = qT[lane:lane + 32, h // 4, :]
            qaug = qaugp.tile([64, N], BF16, tag="qaug", name=f"qaug{h}")
            c1 = nc.vector.tensor_copy(qaug[0:32, :], qsT)
            if pin:
                chained("dve", c1)
            return qaug

        def emit_bias(h, qaug, halves=(0, 1), pin=False, pe_anchor=None):
            """biasT[t,(i,j)] = SWT[t+31-i,(i,j)]: per i-block, two
            accumulating matmuls with shifted rel windows; the relh term
            reads q^T through a stride-32 column view (the (i,j)->(j,i)
            permutation).  Scratch: heads 0-2 use bank 7 halves at
            partitions 0:32/32:64 (bank 7 becomes attn-B from head 1's
            accumulation on); later heads use the idle offload slots."""
            for half in halves:
                if h <= 2:
                    ps_b = ps_bias[half * 32:half * 32 + 32, :]
                else:
                    ps_b = slot(half)[0:32, :]
                for ib in range(16):
                    i = half * 16 + ib
                    pe(nc.tensor.matmul(
                        ps_b[:, ib * 32:(ib + 1) * 32],
                        lhsT=rel_bf[:, 0, 31 - i:63 - i],
                        rhs=qaug[0:32, i * 32:(i + 1) * 32],
                        start=True, stop=False))
                    perm_rhs = bass.AP(
                        tensor=qaug.tensor, offset=qaug.offset + i,
                        ap=[[qaug.ap[0][0], 32], [32, 32]])
                    pe(nc.tensor.matmul(
                        ps_b[:, ib * 32:(ib + 1) * 32],
                        lhsT=rel_bf[:, 1, 31 - i:63 - i],
                        rhs=perm_rhs,
                        start=False, stop=True))
                cp = nc.vector.tensor_copy(
                    qaug[32:64, half * 512:(half + 1) * 512], ps_b)
                if pin:
                    chained("dve", cp)

        def emit_kaug(h, pin=False, los=(0, 8)):
            lane = (h % 4) * 32
            lo, hi = los
            ksT = kT[lane:lane + 32, h // 4, lo * 128:hi * 128]
            cp = nc.vector.tensor_copy(
                ka[h % 4][0:32, lo:hi].rearrange("d j m -> d (j m)"), ksT)
            if pin:
                chained("dve", cp)

        def emit_kaug_oh(h, pin=False):
            if h < 4:
                co = nc.vector.tensor_copy(ka[h % 4][32:64], oh_bf)
                if pin:
                    chained("dve", co)

        # ---------------- v staging ----------------
        v_aug = persist.tile([128, NH, 8, 33], BF16)
        ones_st = persist.tile([128, 64], F32)
        nc.vector.memset(ones_st, 1.0)

        def emit_vconv(j):
            pool("tensor_copy",
                 v_aug[:, :, j, 0:32],
                 v_st[:, j, :].rearrange("p (h d) -> p h d", h=NH))

        nc.vector.tensor_copy(
            v_aug[:, :, :, 32:33].rearrange("p h j o -> p (h j o)"), ones_st)

        # ---------------- epilogue ----------------
        out_sb = persist.tile([128, 8, 256], F32)
        rec_t = persist.tile([128, NH, 8], F32)

        def emit_epilogue(h):
            """reciprocal of the 8 ones-sums + one broadcast normalize of
            the 8 [128,33] regions into out_sb; one strided DMA."""
            ps_att = ps_att_of(h)
            rec = rec_t[:, h, :]
            sums_ap = bass.AP(tensor=ps_att.tensor, offset=ps_att.offset + 32,
                              ap=[list(ps_att.ap[0]), [64, 8]])
            chained("dve", nc.vector.reciprocal(rec, sums_ap))
            in0 = bass.AP(tensor=ps_att.tensor, offset=ps_att.offset,
                          ap=[list(ps_att.ap[0]), [64, 8], [1, 32]])
            in1 = bass.AP(tensor=rec.tensor, offset=rec.offset,
                          ap=[list(rec.ap[0]), [1, 8], [0, 32]])
            out_ap = bass.AP(tensor=out_sb.tensor,
                             offset=out_sb.offset + h * 32,
                             ap=[list(out_sb.ap[0]), [256, 8], [1, 32]])
            chained("dve", nc.vector.tensor_tensor(
                out=out_ap, in0=in0, in1=in1, op=mybir.AluOpType.mult))
            groups = ((0, 8),)
            for glo, ghi in groups:
                dstap = bass.AP(
                    tensor=out.tensor,
                    offset=glo * 128 * 256 + h * 32,
                    ap=[[256, 128], [128 * 256, ghi - glo], [1, 32]])
                nc.sync.dma_start(out=dstap,
                                  in_=out_sb[:, glo:ghi, h * 32:(h + 1) * 32])

        # ---------------- startup ----------------
        # PE order: dummies, rel-w transpose, q transposes, rel-h
        # transpose, k transposes, bias mms, logits.  The DVE chain IS
        # the head-0 critical path: qaug0 and ka0 rows are copied
        # straight out of the transpose psum slots (lane 0); the qT
        # replica copies (for later heads) trail behind and delay only
        # head 0's j0, whose exp is Pool-offloaded and slack-tolerant.
        # Slot use: q-lo->0, q-hi->1, rel->4 (cols 0:127), k-lo->2,
        # k-hi->5; slot 3 stays free for j1's logits.
        relT = slot(4).bitcast(F32R)
        pe(nc.tensor.transpose(relT[0:32, 0:64],
                               rel_nat[:, 0:32], identR[0:64, 0:64]))

        def transp4(cb, lo, s):
            pt = slot(s).bitcast(F32R)
            for c in range(4):
                pe(nc.tensor.transpose(pt[:, c * 128:(c + 1) * 128],
                                       rowsR[:, cb, lo + c, :], identR))

        transp4(0, 0, 0)                                # q half0 lo
        pe(nc.tensor.transpose(relT[0:32, 64:128],
                               rel_nat[:, 32:64], identR[0:64, 0:64]))
        transp4(0, 4, 1)                                # q half0 hi
        transp4(1, 0, 2)                                # k half0 lo

        chained("dve", nc.vector.tensor_copy(
            rel_bf.rearrange("p a u -> p (a u)"),
            bass.AP(tensor=ps_flat.tensor,
                    offset=ps_flat.offset + 4 * 512,
                    ap=[[ps_flat.ap[0][0], 32], [64, 2], [1, 63]])))
        qaug_h = {0: qaugp.tile([64, N], BF16, tag="qaug", name="qaug0")}
        chained("dve", nc.vector.tensor_copy(
            qaug_h[0][0:32, 0:512], slot(0)[0:32, :]))
        chained("dve", nc.vector.tensor_copy(
            qaug_h[0][0:32, 512:1024], slot(1)[0:32, :]))
        chained("dve", nc.vector.tensor_copy(
            ka[0][0:32, 0:4].rearrange("d j m -> d (j m)"),
            slot(2)[0:32, :]))
        # ka[1] rows straight off the same transpose slots (lane 1 =
        # psum partitions 32:64; DVE copies may shift partitions)
        chained("dve", nc.vector.tensor_copy(
            ka[1][0:32, 0:4].rearrange("d j m -> d (j m)"),
            slot(2)[32:64, :]))
        emit_kaug_oh(0, pin=True)
        emit_bias(0, qaug_h[0], halves=(0,), pin=True)
        emit_bias(0, qaug_h[0], halves=(1,), pin=True)
        transp4(1, 4, 5)                                # k half0 hi
        chained("dve", nc.vector.tensor_copy(
            ka[0][0:32, 4:8].rearrange("d j m -> d (j m)"),
            slot(5)[0:32, :]))
        chained("dve", nc.vector.tensor_copy(
            ka[1][0:32, 4:8].rearrange("d j m -> d (j m)"),
            slot(5)[32:64, :]))
        pool("tensor_copy", ka[1][32:64], oh_bf)
        # trailing (delays only head 0's slack-tolerant j0): qT replica
        chained("dve", nc.vector.tensor_copy(qT[:, 0, 0:512], slot(0)))
        chained("dve", nc.vector.tensor_copy(qT[:, 0, 512:1024], slot(1)))
        for j in range(8):
            emit_vconv(j)

        # ---------------- main pipeline ----------------
        ACT_PAIRS = ((2, 3), (4, 5))
        act_rot = [0]   # continuous pair rotation across heads

        def emit_logits(h, j, lo_slot):
            qaug = qaug_h[h]
            for half in range(2):
                mm = pe(nc.tensor.matmul(
                    slot(lo_slot + half), lhsT=ka[h % 4][:, j, :],
                    rhs=qaug[:, half * 512:(half + 1) * 512],
                    start=True, stop=True))
                KMARKS.append((mm.ins.name, f"logits{h}_{j}_h{half}_s{lo_slot+half}"))

        def hook(h, j, a):
            if h == 0:
                if j == 2:
                    emit_transposes(2, qT, 1, 0, s=0, pin=True)
                elif j == 3:
                    emit_transposes(2, qT, 1, 4, s=1, pin=True)
                    qaug_h[1] = emit_qstage(1, pin=True)
                    emit_bias(1, qaug_h[1], halves=(0,), pin=True)
                elif j == 4:
                    # kT half0 replica: re-transpose (startup slots were
                    # drained into ka0/ka1 directly)
                    emit_transposes(1, kT, 0, 0, s=0, pin=True)
                    emit_bias(1, qaug_h[1], halves=(1,), pin=True)
                elif j == 5:
                    emit_transposes(1, kT, 0, 4, s=0, pin=True)
                elif j == 6:
                    qaug_h[2] = emit_qstage(2, pin=True)
                    emit_bias(2, qaug_h[2], halves=(0,), pin=True)
                elif j == 7:
                    emit_bias(2, qaug_h[2], halves=(1,), pin=True)
            elif h == 1:
                if j == 1:
                    emit_transposes(3, kT, 1, 0, s=0, pin=True)
                    emit_kaug(2, pin=True)
                    emit_kaug_oh(2, pin=True)
                elif j == 2:
                    qaug_h[3] = emit_qstage(3, pin=True)
                elif j == 3:
                    emit_bias(3, qaug_h[3], halves=(0,), pin=True)
                elif j == 4:
                    emit_bias(3, qaug_h[3], halves=(1,), pin=True)
                elif j == 5:
                    emit_transposes(3, kT, 1, 4, s=0, pin=True)
            else:
                if j == 1 and h + 1 < NH:
                    emit_kaug(h + 1, pin=True)
                    emit_kaug_oh(h + 1, pin=True)
                elif j == 2 and h + 2 < NH:
                    qaug_h[h + 2] = emit_qstage(h + 2, pin=True)
                elif j == 3 and h + 2 < NH:
                    emit_bias(h + 2, qaug_h[h + 2], halves=(0,), pin=True)
                elif j == 4 and h + 2 < NH:
                    emit_bias(h + 2, qaug_h[h + 2], halves=(1,), pin=True)

        def emit_offload(h, j):
            emit_logits_once((h, j))
            ls = lstgp.tile([128, N], F32, tag="ls", name=f"ls{h}_{j}")
            KMARKS.append((chained("dve", nc.vector.tensor_copy(
                ls, slot(0, 2))).ins.name, f"stage{h}_{j}"))
            ew = expwp.tile([128, N], BF16, tag="ew", name=f"ew{h}_{j}")
            chained("pool", nc.gpsimd.tensor_tensor(
                out=ew, in0=base_bcast(N), in1=ls,
                op=mybir.AluOpType.pow))
            return ew

        def emit_attnv_zero(h):
            # one full-width start=True matmul resets the attn bank;
            # interleaved per-region start writes clobber each other on HW
            pe(nc.tensor.matmul(ps_att_of(h), lhsT=zeros_bf,
                                rhs=qT[:, 0, 0:512],
                                start=True, stop=False))

        def emit_attnv(h, j, ew, start, stop):
            ps_att = ps_att_of(h)
            a_last = None
            for nt in range(8):
                a_last = pe(nc.tensor.matmul(
                    ps_att[:, nt * 64:nt * 64 + 33],
                    lhsT=ew[:, nt * 128:(nt + 1) * 128],
                    rhs=v_aug[:, h, j, :],
                    start=start, stop=stop))
            return a_last

        # pending deferred work from head h-1, emitted inside head h's j1
        # iteration (gives the slow offload pipeline extra time before its
        # attn@V could block the in-order PE queue):
        pending = None

        # pair assignment per tile, rotation continuous across heads
        pair_of = {}
        rot = 0
        for h in range(NH):
            for j in range(8):
                if j in OFFLOAD[h]:
                    pair_of[(h, j)] = (0, 1)
                else:
                    pair_of[(h, j)] = ACT_PAIRS[rot % 2]
                    rot += 1

        emitted_logits = set()

        def next_tile(h, j):
            if j < 7:
                return (h, j + 1)
            return (h + 1, 0) if h + 1 < NH else None

        def emit_logits_once(t):
            if t is not None and t not in emitted_logits:
                emitted_logits.add(t)
                emit_logits(t[0], t[1], pair_of[t][0])

        # head-0 priming: j1/j2 logits first (they gate Act); j0 last --
        # its slots are released only by the trailing qT replica copies,
        # and its Pool-exp pipeline has most of the head as slack
        for t in ((0, 1), (0, 2)):
            emit_logits_once(t)
        for h in range(NH):
            off_js = OFFLOAD[h]
            act_js = [j for j in range(8) if j not in off_js]
            last = NH - 1
            first_j = act_js[0]
            last_j = 7 if h == last else 0
            ews = {}
            for j in range(8):
                # this tile's logits were emitted one iteration ago; emit
                # the NEXT tile's logits before this tile's attn@V so the
                # pinned PE queue never waits an exp to issue logits
                if h == last and j == 7:
                    emit_attnv(h, 0, ews[0], False, False)
                if j in off_js:
                    if j == 0 and h != 1:
                        # h=1's bank (7) drains bias(2) late; its zero is
                        # deferred to (1, j1) so it can't block the chain
                        emit_attnv_zero(h)
                    ews[j] = emit_offload(h, j)
                    emit_logits_once(next_tile(h, j))
                    nt2 = next_tile(h, j)
                    if nt2 is not None:
                        emit_logits_once(next_tile(*nt2))
                else:
                    ew = expwp.tile([128, N], BF16, tag="ew",
                                    name=f"ew{h}_{j}")
                    if (h, j) in ((0, 1), (0, 2)):
                        # two half-exps: half0 unblocks on bias-half-a,
                        # pulling the whole Act stream ~1us earlier
                        for hf in range(2):
                            KMARKS.append((nc.scalar.activation(
                                ew[:, hf * 512:(hf + 1) * 512],
                                slot(pair_of[(h, j)][0] + hf),
                                AF.Exp, scale=SCALE).ins.name,
                                f"exp{h}_{j}h{hf}"))
                    else:
                        KMARKS.append((nc.scalar.activation(
                            ew, slot(pair_of[(h, j)][0], 2),
                            AF.Exp, scale=SCALE).ins.name, f"exp{h}_{j}"))
                    ews[j] = ew
                    # two tiles ahead: logits(t+2) only WAR-waits this
                    # exp's pair, giving the chain ~1us of margin
                    emit_logits_once(next_tile(h, j))
                    nt2 = next_tile(h, j)
                    if nt2 is not None:
                        emit_logits_once(next_tile(*nt2))
                    if j == 1 and pending is not None:
                        ph, defs = pending
                        for idx, (pj, pew) in enumerate(defs):
                            emit_attnv(ph, pj, pew, False,
                                       idx == len(defs) - 1)
                        emit_epilogue(ph)
                        pending = None
                    if h == last and j == 7 and 3 in off_js:
                        emit_attnv(h, 3, ews[3], False, False)
                    if h == 1 and j == 1:
                        emit_attnv_zero(1)
                    a_last = emit_attnv(h, j, ew, False, j == last_j)
                    hook(h, j, a_last)
            if h != last:
                # all deferred attn@V for Pool-exp'd js punts into head
                # h+1's j1 (so a pow still in flight can never block the
                # in-order PE queue at the head boundary)
                pending = (h, [(j, ews[j]) for j in off_js])

        emit_epilogue(NH - 1)


_NC_CACHE = {}


def _build():
    if "nc" in _NC_CACHE:
        return _NC_CACHE["nc"]
    nc = bass.Bass("TRN2", target_bir_lowering=False, debug=False,
                   enable_asserts=True, num_devices=8)
    ins = {
        "x": nc.dram_tensor("x", [N, 768], F32, kind="ExternalInput").ap(),
        "relh": nc.dram_tensor("relh", [63, 32], F32, kind="ExternalInput").ap(),
        "relw": nc.dram_tensor("relw", [63, 32], F32, kind="ExternalInput").ap(),
    }
    outs = {
        "out": nc.dram_tensor("out", [N, 256], F32, kind="ExternalOutput").ap(),
    }
    with tile.TileContext(nc) as tc:
        kernel_body(tc, outs, ins)
    split_multiwaits(nc)
    _NC_CACHE["nc"] = nc
    return nc


def kernel(inputs, key_rel_h, key_rel_w, _trace=False):
    nc = _build()
    x = np.ascontiguousarray(np.asarray(inputs, dtype=np.float32).reshape(8, N, 768))
    rh = np.ascontiguousarray(np.asarray(key_rel_h, dtype=np.float32))
    rw = np.ascontiguousarray(np.asarray(key_rel_w, dtype=np.float32))
    in_maps = [{"x": x[c], "relh": rh, "relw": rw} for c in range(8)]
    res = bass_utils.run_bass_kernel_spmd(
        nc, in_maps, core_ids=list(range(8)), trace=_trace)
    outp = np.stack([r["out"] for r in res.results])
    if _trace:
        kernel.last_results = res
    return outp.reshape(8, 32, 32, 256)


# revision 51
# speedup vs baseline: 1.1347x; 1.0136x over previous
"""AttentionAugmentation2D Trainium2 kernel (v6).

Shapes (hardcoded): B=8, H=W=32, N=1024, NH=8 heads, dk=dv=32 per head.
inputs [8,32,32,768] = q|k|v (256 each), key_rel_h/w [63,32].
Sharding: data-parallel over batch B across the 8 cores.

Math per (batch, head), n=(i,j), m=(i',j') (i = H index):
  logits[n,m] = q[n]@k[m] + q[(j,i)]@rel_h[i'-i+31] + q[(i,j)]@rel_w[i'-i+31]
Both rel terms depend on m only through i', so with
  SWT[u,n] = rel_w[u]@q[(i,j)] + rel_h[u]@q[(j,i)]        (u in [0,63))
  biasT[t,n] = SWT[t+31-i(n), n]                          (shifted windows)
we get  logits^T = K_aug^T.T @ Q_augT  with contraction 64:
  K_aug rows: 0:32 = k^T, 32:64 = onehot[t==i'(m)] ;  Q_aug: [q^T; biasT].

v6 redesign vs v5 (cost-model driven):
 - All PE operands are bf16 (error budget 2e-2 is ~10x away): packed
   bf16 SBUF-to-SBUF DVE copies run in 4x perf mode.
 - attn@V swaps operand roles: the exp-weight chunk ew[:, nt*128:...]
   is the *stationary* matmul operand and v (32 cols + a ones column
   for the softmax row-sums) streams, so each matmul charges only 33
   rows instead of 512.  The output lands n-major, which kills v5's 64
   flush transposes and attn^T staging copies; accumulation uses 8
   sub-bank [128,33] regions spaced 64 cols apart in ONE psum bank,
   single-buffered across heads (each head's epilogue completes before
   the next head's first accumulating matmul needs the regions).
 - biasT is built by TWO accumulating matmuls per i-block (relw-window
   x a q^T block, then relh-window x a stride-32 column view of q^T
   that realizes the (i,j)->(j,i) permutation for free), so no
   permuted-q replica and no separate bias-rhs tile exist; both rhs
   reads come straight from qaug rows 0:32.
 - The softmax exp is split between the Act engine (activation Exp)
   and the Pool engine: gpsimd supports elementwise pow, so
   exp(s*x) = pow(e^s, x) with a stride-0 broadcast base.  gpsimd
   cannot read PSUM, so DVE stages the offloaded logit tiles
   PSUM->SBUF; the offload count balances Act against DVE+Pool.
 - Per-head epilogue: DVE reciprocal over the 8 strided ones-sums,
   one broadcast tensor_tensor multiply normalizing all 8 regions into
   the n-major staging tile, one strided DMA per head.
 - PSUM map: banks 0-5 = one [128,8,512] region manually slotted in
   512-col halves: pair (0,1) serves the Pool-offloaded logit tiles,
   the startup/half-1 transposes, and (heads >= 3) the bias scratch;
   pairs (2,3)/(4,5) alternate (continuously across heads) for the
   Act-exp'd tiles.  Banks 6/7 double-buffer the attn regions across
   heads (each head's bank is reset by ONE full-width start=True
   matmul -- interleaved per-region start writes clobber each other on
   real HW -- and all attn@V matmuls accumulate with start=False);
   bank 7 also hosts heads 0-2's bias scratch and the warm-up dummies
   before head 1's accumulation begins.
 - Engine queue discipline: the Tile scheduler reorders engine streams
   with its internal timing model, so ALL PE/DVE/Pool instructions are
   pinned in emission order with ordering-only deps; logit matmuls are
   emitted two tiles ahead so they only WAR-wait the exp that frees
   their psum pair (~1us of margin); deferred attn@V and the epilogue
   of head h-1 are emitted inside head h's j1 iteration.
 - rel tables are DMA'd in natural [63,32] layout (a transposed DMA
   would cost ~2000 descriptors on the single shared DMA stream, ahead
   of the critical q/k row loads) and transposed on the PE instead.

Toolchain note: walrus codegen only fits ONE semaphore wait in most
TPB instruction structs; split_multiwaits() moves excess waits onto
same-engine InstNoOp carriers (same workaround as v5).
"""

import numpy as np

import concourse.bass as bass
import concourse.mybir as mybir
import concourse.tile as tile
from concourse import bass_utils
from concourse.masks import make_identity
from concourse.tile import add_dep_helper

F32 = mybir.dt.float32
F32R = mybir.dt.float32r
BF16 = mybir.dt.bfloat16
AF = mybir.ActivationFunctionType

NH = 8
N = 1024
DK = 32
SCALE = float(DK) ** -0.5
BASE = float(np.exp(SCALE))

# Per-head tuple of js whose exp runs on Pool (via DVE PSUM->SBUF stage).
# Must be a subset of {0, 3} (those js own psum slot pair (0,1)); their
# attn@V is deferred to head end.
OFFLOAD = {h: (0,) for h in range(NH)}

KMARKS = []   # (inst_name, label) for trace debugging


def split_multiwaits(nc, dma_limit=1):
    """Move excess semaphore waits onto same-engine nop carriers."""
    n_new = 0
    for f in nc.m.functions:
        for blk in f.blocks:
            newlist = []
            for inst in blk.instructions:
                si = getattr(inst, "sync_info", None)
                is_dma = isinstance(inst, mybir.InstDMACopy)
                limit = dma_limit if is_dma else 1
                if si is not None and len(si.on_wait) > limit:
                    waits = list(si.on_wait)
                    for w in waits[:-1]:
                        n_new += 1
                        newlist.append(mybir.InstNoOp(
                            name=f"I-wc{n_new}",
                            ins=[], outs=[],
                            sync_info=mybir.SyncInfo(on_wait=[w], on_update=[]),
                            bass_nofuse=True,
                            engine=inst.engine,
                        ))
                    inst.sync_info = mybir.SyncInfo(
                        on_wait=waits[-1:], on_update=si.on_update)
                newlist.append(inst)
            blk.instructions = newlist
    return n_new


def kernel_body(tc, outs, ins):
    nc = tc.nc
    x = ins["x"]          # [1024, 768] rows n=(i,j), cols q|k|v
    relh = ins["relh"]    # [63, 32]
    relw = ins["relw"]    # [63, 32]
    out = outs["out"]     # [1024, 256]

    with (
        tc.tile_pool(name="persist", bufs=1) as persist,
        tc.tile_pool(name="expw", bufs=10) as expwp,
        tc.tile_pool(name="qaug", bufs=3) as qaugp,
        tc.tile_pool(name="lstg", bufs=3) as lstgp,
        tc.tile_pool(name="psmain", bufs=1, space="PSUM") as psmain,
    ):
        # ---- PSUM map ----
        ps_all = psmain.tile([128, 8, 512], F32, tag="ps")
        ps_flat = ps_all.rearrange("p s c -> p (s c)")

        def slot(s, n=1):
            return ps_flat[:, s * 512:(s + n) * 512]
        def ps_att_of(h):
            return slot(6 + h % 2)
        ps_bias = slot(7)

        # ---------------- DMAs (the DMA stream is serial in practice:
        # critical q rows first, tiny rel loads sandwiched, v last) ----
        rowsR = persist.tile([128, 4, 8, 128], F32R)
        CB_COLS = (0, 256, 128, 384)   # q0, k0, q1, k1

        def emit_rows_dma(cb, eng, lo=0, hi=8):
            src = bass.AP(tensor=x.tensor,
                          offset=CB_COLS[cb] + lo * 128 * 768,
                          ap=[[768, 128], [128 * 768, hi - lo], [1, 128]],
                          ).bitcast(F32R)
            eng.dma_start(out=rowsR[:, cb, lo:hi, :], in_=src)

        rel_nat = persist.tile([64, 64], F32R)
        nc.vector.memset(rel_nat.bitcast(F32), 0.0)
        v_st = persist.tile([128, 8, 256], F32)

        def emit_v_dma(j):
            nc.sync.dma_start(
                out=v_st[:, j, :], in_=x[j * 128:(j + 1) * 128, 512:768])

        # ALL input DMAs ride the SP queue: a dma_start costs ~667ns of
        # SEQUENCER time on the issuing engine, and the Act sequencer must
        # stay free to issue exps; a single queue also makes the serial
        # DMA stream follow emission order exactly
        emit_rows_dma(0, nc.sync, 0, 4)          # q half0 lo
        nc.sync.dma_start(out=rel_nat[0:63, 0:32], in_=relw.bitcast(F32R))
        emit_rows_dma(0, nc.sync, 4, 8)          # q half0 hi
        nc.sync.dma_start(out=rel_nat[0:63, 32:64], in_=relh.bitcast(F32R))
        emit_rows_dma(1, nc.sync, 0, 4)          # k half0 lo
        emit_v_dma(0)
        emit_rows_dma(1, nc.sync, 4, 8)          # k half0 hi
        emit_v_dma(1)
        emit_v_dma(2)
        emit_rows_dma(2, nc.sync)                # q half1
        emit_rows_dma(3, nc.sync)                # k half1
        for j in range(3, 8):
            emit_v_dma(j)

        # ---------------- warm-up + constants ----------------
        _chain = {}
        dummy_sb = persist.tile([128, 64], F32)
        nc.vector.memset(dummy_sb, 0.0)
        for w in range(20):
            _dm = nc.tensor.matmul(ps_bias[64:96, 0:32],
                                   lhsT=dummy_sb[:, 0:32],
                                   rhs=dummy_sb[:, 0:32],
                                   start=True, stop=True)
            _chain.setdefault("pe", _dm)
            if _chain["pe"] is not _dm:
                add_dep_helper(_dm.ins, _chain["pe"].ins, sync=False,
                               reason="pin pe queue order")
            _chain["pe"] = _dm

        zeros_bf = persist.tile([128, 128], BF16)
        nc.vector.memset(zeros_bf, 0.0)
        # pre-warm the Act engine's Exp table (the first activation pays
        # a ~1.3us table load otherwise -- on the critical startup path)
        act_warm = persist.tile([128, 1], F32)
        nc.scalar.activation(act_warm, dummy_sb[:, 0:1], AF.Exp, scale=SCALE)

        ident = persist.tile([128, 128], F32)
        make_identity(nc, ident)
        identR = persist.tile([128, 128], F32R)
        nc.vector.tensor_copy(identR, ident)
        ident_marker = nc.gpsimd.tensor_copy(ident[0:1, 0:1], ident[0:1, 0:1])

        # pow base for the Pool exp share: pow(e^s, x) = exp(s*x)
        base_t = persist.tile([128, 1], F32)
        nc.vector.memset(base_t, BASE)

        def base_bcast(cols):
            return bass.AP(tensor=base_t.tensor, offset=base_t.offset,
                           ap=[list(base_t.ap[0]), [0, cols]])

        # Pin Pool and startup-DVE queue order (the Tile scheduler's
        # internal timing model reorders engine queues badly otherwise).
        _chain["pool"] = ident_marker

        def chained(engine_name, inst):
            prev = _chain.get(engine_name)
            if prev is not None:
                add_dep_helper(inst.ins, prev.ins, sync=False,
                               reason=f"pin {engine_name} queue order")
            _chain[engine_name] = inst
            return inst

        def pool(op, *args, **kwargs):
            return chained("pool", getattr(nc.gpsimd, op)(*args, **kwargs))

        def pe(inst):
            return chained("pe", inst)

        # onehot rows for K_aug (bf16) -- no input deps, head of Pool chain
        oh_st = persist.tile([32, 8, 128], F32)
        pool("memset", oh_st, 0.0)
        oh = oh_st.rearrange("t j (b m) -> t j b m", b=4)
        pool("affine_select",
             out=oh, in_=oh, compare_op=mybir.AluOpType.not_equal,
             fill=1.0, base=0, pattern=[[-4, 8], [-1, 4], [0, 32]],
             channel_multiplier=1)
        oh_bf = persist.tile([32, 8, 128], BF16)
        pool("tensor_copy", oh_bf, oh_st)

        # bf16 transposed replicas of q and k: [p=32*(h%4)+d, half, n]
        qT = persist.tile([128, 2, N], BF16)
        kT = persist.tile([128, 2, N], BF16)

        ka = [persist.tile([64, 8, 128], BF16, tag=f"ka{i}", name=f"ka{i}")
              for i in range(4)]

        # rel^T: [32, 2, 63] at partitions 0:32 (PE operands must share a
        # base partition): [:,0,:] = relw^T, [:,1,:] = relh^T
        rel_bf = persist.tile([32, 2, 63], BF16)

        def emit_transposes(cb, dst, half, lo, s, pin=False, pe_anchor=None):
            """4 transposes into one psum slot + one wide copy into the
            bf16 replica."""
            pt = slot(s).bitcast(F32R)
            for c in range(4):
                tr = pe(nc.tensor.transpose(pt[:, c * 128:(c + 1) * 128],
                                            rowsR[:, cb, lo + c, :], identR))
            cp = nc.vector.tensor_copy(
                dst[:, half, lo * 128:(lo + 4) * 128], slot(s))
            if pin:
                chained("dve", cp)

        # ---------------- per-head staging ----------------
        def emit_qstage(h, pin=False):
            """qaug rows 0:32 = q^T for head h (bf16 4x copy)."""
            lane = (h % 4) * 32
            qsT = qT[lane:lane + 32, h // 4, :]
            qaug = qaugp.tile([64, N], BF16, tag="qaug", name=f"qaug{h}")
            c1 = nc.vector.tensor_copy(qaug[0:32, :], qsT)
            if pin:
                chained("dve", c1)
            return qaug

        def emit_bias(h, qaug, halves=(0, 1), pin=False, pe_anchor=None):
            """biasT[t,(i,j)] = SWT[t+31-i,(i,j)]: per i-block, two
            accumulating matmuls with shifted rel windows; the relh term
            reads q^T through a stride-32 column view (the (i,j)->(j,i)
            permutation).  Scratch: heads 0-2 use bank 7 halves at
            partitions 0:32/32:64 (bank 7 becomes attn-B from head 1's
            accumulation on); later heads use the idle offload slots."""
            for half in halves:
                if h <= 2:
                    ps_b = ps_bias[half * 32:half * 32 + 32, :]
                else:
                    ps_b = slot(half)[0:32, :]
                for ib in range(16):
                    i = half * 16 + ib
                    pe(nc.tensor.matmul(
                        ps_b[:, ib * 32:(ib + 1) * 32],
                        lhsT=rel_bf[:, 0, 31 - i:63 - i],
                        rhs=qaug[0:32, i * 32:(i + 1) * 32],
                        start=True, stop=False))
                    perm_rhs = bass.AP(
                        tensor=qaug.tensor, offset=qaug.offset + i,
                        ap=[[qaug.ap[0][0], 32], [32, 32]])
                    pe(nc.tensor.matmul(
                        ps_b[:, ib * 32:(ib + 1) * 32],
                        lhsT=rel_bf[:, 1, 31 - i:63 - i],
                        rhs=perm_rhs,
                        start=False, stop=True))
                cp = nc.vector.tensor_copy(
                    qaug[32:64, half * 512:(half + 1) * 512], ps_b)
                if pin:
                    chained("dve", cp)

        def emit_kaug(h, pin=False, los=(0, 8)):
            lane = (h % 4) * 32
            lo, hi = los
            ksT = kT[lane:lane + 32, h // 4, lo * 128:hi * 128]
            cp = nc.vector.tensor_copy(
                ka[h % 4][0:32, lo:hi].rearrange("d j m -> d (j m)"), ksT)
            if pin:
                chained("dve", cp)

        def emit_kaug_oh(h, pin=False):
            if h < 4:
                co = nc.vector.tensor_copy(ka[h % 4][32:64], oh_bf)
                if pin:
                    chained("dve", co)

        # ---------------- v staging ----------------
        v_aug = persist.tile([128, NH, 8, 33], BF16)
        ones_st = persist.tile([128, 64], F32)
        nc.vector.memset(ones_st, 1.0)

        def emit_vconv(j):
            pool("tensor_copy",
                 v_aug[:, :, j, 0:32],
                 v_st[:, j, :].rearrange("p (h d) -> p h d", h=NH))

        nc.vector.tensor_copy(
            v_aug[:, :, :, 32:33].rearrange("p h j o -> p (h j o)"), ones_st)

        # ---------------- epilogue ----------------
        out_sb = persist.tile([128, 8, 256], F32)
        rec_t = persist.tile([128, NH, 8], F32)

        def emit_epilogue(h):
            """reciprocal of the 8 ones-sums + one broadcast normalize of
            the 8 [128,33] regions into out_sb; one strided DMA."""
            ps_att = ps_att_of(h)
            rec = rec_t[:, h, :]
            sums_ap = bass.AP(tensor=ps_att.tensor, offset=ps_att.offset + 32,
                              ap=[list(ps_att.ap[0]), [64, 8]])
            chained("dve", nc.vector.reciprocal(rec, sums_ap))
            in0 = bass.AP(tensor=ps_att.tensor, offset=ps_att.offset,
                          ap=[list(ps_att.ap[0]), [64, 8], [1, 32]])
            in1 = bass.AP(tensor=rec.tensor, offset=rec.offset,
                          ap=[list(rec.ap[0]), [1, 8], [0, 32]])
            out_ap = bass.AP(tensor=out_sb.tensor,
                             offset=out_sb.offset + h * 32,
                             ap=[list(out_sb.ap[0]), [256, 8], [1, 32]])
            chained("dve", nc.vector.tensor_tensor(
                out=out_ap, in0=in0, in1=in1, op=mybir.AluOpType.mult))
            groups = ((0, 8),)
            for glo, ghi in groups:
                dstap = bass.AP(
                    tensor=out.tensor,
                    offset=glo * 128 * 256 + h * 32,
                    ap=[[256, 128], [128 * 256, ghi - glo], [1, 32]])
                nc.sync.dma_start(out=dstap,
                                  in_=out_sb[:, glo:ghi, h * 32:(h + 1) * 32])

        # ---------------- startup ----------------
        # PE order: dummies, rel-w transpose, q transposes, rel-h
        # transpose, k transposes, bias mms, logits.  The DVE chain IS
        # the head-0 critical path: qaug0 and ka0 rows are copied
        # straight out of the transpose psum slots (lane 0); the qT
        # replica copies (for later heads) trail behind and delay only
        # head 0's j0, whose exp is Pool-offloaded and slack-tolerant.
        # Slot use: q-lo->0, q-hi->1, rel->4 (cols 0:127), k-lo->2,
        # k-hi->5; slot 3 stays free for j1's logits.
        relT = slot(4).bitcast(F32R)
        pe(nc.tensor.transpose(relT[0:32, 0:64],
                               rel_nat[:, 0:32], identR[0:64, 0:64]))

        def transp4(cb, lo, s):
            pt = slot(s).bitcast(F32R)
            for c in range(4):
                pe(nc.tensor.transpose(pt[:, c * 128:(c + 1) * 128],
                                       rowsR[:, cb, lo + c, :], identR))

        transp4(0, 0, 0)                                # q half0 lo
        pe(nc.tensor.transpose(relT[0:32, 64:128],
                               rel_nat[:, 32:64], identR[0:64, 0:64]))
        transp4(0, 4, 1)                                # q half0 hi
        transp4(1, 0, 2)                                # k half0 lo

        chained("dve", nc.vector.tensor_copy(
            rel_bf.rearrange("p a u -> p (a u)"),
            bass.AP(tensor=ps_flat.tensor,
                    offset=ps_flat.offset + 4 * 512,
                    ap=[[ps_flat.ap[0][0], 32], [64, 2], [1, 63]])))
        qaug_h = {0: qaugp.tile([64, N], BF16, tag="qaug", name="qaug0")}
        chained("dve", nc.vector.tensor_copy(
            qaug_h[0][0:32, 0:512], slot(0)[0:32, :]))
        chained("dve", nc.vector.tensor_copy(
            qaug_h[0][0:32, 512:1024], slot(1)[0:32, :]))
        chained("dve", nc.vector.tensor_copy(
            ka[0][0:32, 0:4].rearrange("d j m -> d (j m)"),
            slot(2)[0:32, :]))
        # ka[1] rows straight off the same transpose slots (lane 1 =
        # psum partitions 32:64; DVE copies may shift partitions)
        chained("dve", nc.vector.tensor_copy(
            ka[1][0:32, 0:4].rearrange("d j m -> d (j m)"),
            slot(2)[32:64, :]))
        emit_kaug_oh(0, pin=True)
        emit_bias(0, qaug_h[0], halves=(0,), pin=True)
        emit_bias(0, qaug_h[0], halves=(1,), pin=True)
        transp4(1, 4, 5)                                # k half0 hi
        chained("dve", nc.vector.tensor_copy(
            ka[0][0:32, 4:8].rearrange("d j m -> d (j m)"),
            slot(5)[0:32, :]))
        chained("dve", nc.vector.tensor_copy(
            ka[1][0:32, 4:8].rearrange("d j m -> d (j m)"),
            slot(5)[32:64, :]))
        pool("tensor_copy", ka[1][32:64], oh_bf)
        # trailing (delays only head 0's slack-tolerant j0): qT replica
        chained("dve", nc.vector.tensor_copy(qT[:, 0, 0:512], slot(0)))
        chained("dve", nc.vector.tensor_copy(qT[:, 0, 512:1024], slot(1)))
        for j in range(8):
            emit_vconv(j)

        # ---------------- main pipeline ----------------
        ACT_PAIRS = ((2, 3), (4, 5))
        act_rot = [0]   # continuous pair rotation across heads

        def emit_logits(h, j, lo_slot):
            qaug = qaug_h[h]
            for half in range(2):
                mm = pe(nc.tensor.matmul(
                    slot(lo_slot + half), lhsT=ka[h % 4][:, j, :],
                    rhs=qaug[:, half * 512:(half + 1) * 512],
                    start=True, stop=True))
                KMARKS.append((mm.ins.name, f"logits{h}_{j}_h{half}_s{lo_slot+half}"))

        def hook(h, j, a):
            if h == 0:
                if j == 2:
                    emit_transposes(2, qT, 1, 0, s=0, pin=True)
                elif j == 3:
                    emit_transposes(2, qT, 1, 4, s=1, pin=True)
                    qaug_h[1] = emit_qstage(1, pin=True)
                    emit_bias(1, qaug_h[1], halves=(0,), pin=True)
                elif j == 4:
                    # kT half0 replica: re-transpose (startup slots were
                    # drained into ka0/ka1 directly)
                    emit_transposes(1, kT, 0, 0, s=0, pin=True)
                    emit_bias(1, qaug_h[1], halves=(1,), pin=True)
                elif j == 5:
                    emit_transposes(1, kT, 0, 4, s=0, pin=True)
                elif j == 6:
                    qaug_h[2] = emit_qstage(2, pin=True)
                    emit_bias(2, qaug_h[2], halves=(0,), pin=True)
                elif j == 7:
                    emit_bias(2, qaug_h[2], halves=(1,), pin=True)
            elif h == 1:
                if j == 1:
                    emit_transposes(3, kT, 1, 0, s=0, pin=True)
                    emit_kaug(2, pin=True)
                    emit_kaug_oh(2, pin=True)
                elif j == 2:
                    qaug_h[3] = emit_qstage(3, pin=True)
                elif j == 3:
                    emit_bias(3, qaug_h[3], halves=(0,), pin=True)
                elif j == 4:
                    emit_bias(3, qaug_h[3], halves=(1,), pin=True)
                elif j == 5:
                    emit_transposes(3, kT, 1, 4, s=0, pin=True)
            else:
                if j == 1 and h + 1 < NH:
                    emit_kaug(h + 1, pin=True)
                    emit_kaug_oh(h + 1, pin=True)
                elif j == 2 and h + 2 < NH:
                    qaug_h[h + 2] = emit_qstage(h + 2, pin=True)
                elif j == 3 and h + 2 < NH:
                    emit_bias(h + 2, qaug_h[h + 2], halves=(0,), pin=True)
                elif j == 4 and h + 2 < NH:
                    emit_bias(h + 2, qaug_h[h + 2], halves=(1,), pin=True)

        def emit_offload(h, j):
            emit_logits_once((h, j))
            ls = lstgp.tile([128, N], F32, tag="ls", name=f"ls{h}_{j}")
            KMARKS.append((chained("dve", nc.vector.tensor_copy(
                ls, slot(0, 2))).ins.name, f"stage{h}_{j}"))
            ew = expwp.tile([128, N], BF16, tag="ew", name=f"ew{h}_{j}")
            chained("pool", nc.gpsimd.tensor_tensor(
                out=ew, in0=base_bcast(N), in1=ls,
                op=mybir.AluOpType.pow))
            return ew

        def emit_attnv_zero(h):
            # one full-width start=True matmul resets the attn bank;
            # interleaved per-region start writes clobber each other on HW
            pe(nc.tensor.matmul(ps_att_of(h), lhsT=zeros_bf,
                                rhs=qT[:, 0, 0:512],
                                start=True, stop=False))

        def emit_attnv(h, j, ew, start, stop):
            ps_att = ps_att_of(h)
            a_last = None
            for nt in range(8):
                a_last = pe(nc.tensor.matmul(
                    ps_att[:, nt * 64:nt * 64 + 33],
                    lhsT=ew[:, nt * 128:(nt + 1) * 128],
                    rhs=v_aug[:, h, j, :],
                    start=start, stop=stop))
            return a_last

        # pending deferred work from head h-1, emitted inside head h's j1
        # iteration (gives the slow offload pipeline extra time before its
        # attn@V could block the in-order PE queue):
        pending = None

        # pair assignment per tile, rotation continuous across heads
        pair_of = {}
        rot = 0
        for h in range(NH):
            for j in range(8):
                if j in OFFLOAD[h]:
                    pair_of[(h, j)] = (0, 1)
                else:
                    pair_of[(h, j)] = ACT_PAIRS[rot % 2]
                    rot += 1

        emitted_logits = set()

        def next_tile(h, j):
            if j < 7:
                return (h, j + 1)
            return (h + 1, 0) if h + 1 < NH else None

        def emit_logits_once(t):
            if t is not None and t not in emitted_logits:
                emitted_logits.add(t)
                emit_logits(t[0], t[1], pair_of[t][0])

        # head-0 priming: j1/j2 logits first (they gate Act); j0 last --
        # its slots are released only by the trailing qT replica copies,
        # and its Pool-exp pipeline has most of the head as slack
        for t in ((0, 1), (0, 2)):
            emit_logits_once(t)
        for h in range(NH):
            off_js = OFFLOAD[h]
            act_js = [j for j in range(8) if j not in off_js]
            last = NH - 1
            first_j = act_js[0]
            last_j = 7 if h == last else 0
            ews = {}
            for j in range(8):
                # this tile's logits were emitted one iteration ago; emit
                # the NEXT tile's logits before this tile's attn@V so the
                # pinned PE queue never waits an exp to issue logits
                if h == last and j == 7:
                    emit_attnv(h, 0, ews[0], False, False)
                if j in off_js:
                    if j == 0 and h != 1:
                        # h=1's bank (7) drains bias(2) late; its zero is
                        # deferred to (1, j1) so it can't block the chain
                        emit_attnv_zero(h)
                    ews[j] = emit_offload(h, j)
                    emit_logits_once(next_tile(h, j))
                    nt2 = next_tile(h, j)
                    if nt2 is not None:
                        emit_logits_once(next_tile(*nt2))
                else:
                    ew = expwp.tile([128, N], BF16, tag="ew",
                                    name=f"ew{h}_{j}")
                    if (h, j) in ((0, 1), (0, 2)):
                        # two half-exps: half0 unblocks on bias-half-a,
                        # pulling the whole Act stream ~1us earlier
                        for hf in range(2):
                            KMARKS.append((nc.scalar.activation(
                                ew[:, hf * 512:(hf + 1) * 512],
                                slot(pair_of[(h, j)][0] + hf),
                                AF.Exp, scale=SCALE).ins.name,
                                f"exp{h}_{j}h{hf}"))
                    else:
                        KMARKS.append((nc.scalar.activation(
                            ew, slot(pair_of[(h, j)][0], 2),
                            AF.Exp, scale=SCALE).ins.name, f"exp{h}_{j}"))
                    ews[j] = ew
                    # two tiles ahead: logits(t+2) only WAR-waits this
                    # exp's pair, giving the chain ~1us of margin.  At j6
                    # the two-ahead tile would be the next head's j0 --
                    # whose slots drain late but whose Pool-exp pipeline
                    # has slack -- so emit the next head's j1 instead and
                    # let j0 follow at its own iteration.
                    if j == 6:
                        emit_logits_once((h, 7))
                        if h + 1 < NH:
                            emit_logits_once((h + 1, 1))
                    else:
                        emit_logits_once(next_tile(h, j))
                        nt2 = next_tile(h, j)
                        if nt2 is not None:
                            emit_logits_once(next_tile(*nt2))
                    if j == 1 and pending is not None:
                        ph, defs = pending
                        for idx, (pj, pew) in enumerate(defs):
                            emit_attnv(ph, pj, pew, False,
                                       idx == len(defs) - 1)
                        emit_epilogue(ph)
                        pending = None
                    if h == last and j == 7 and 3 in off_js:
                        emit_attnv(h, 3, ews[3], False, False)
                    if h == 1 and j == 1:
                        emit_attnv_zero(1)
                    a_last = emit_attnv(h, j, ew, False, j == last_j)
                    hook(h, j, a_last)
            if h != last:
                # all deferred attn@V for Pool-exp'd js punts into head
                # h+1's j1 (so a pow still in flight can never block the
                # in-order PE queue at the head boundary)
                pending = (h, [(j, ews[j]) for j in off_js])

        emit_epilogue(NH - 1)


_NC_CACHE = {}


def _build():
    if "nc" in _NC_CACHE:
        return _NC_CACHE["nc"]
    nc = bass.Bass("TRN2", target_bir_lowering=False, debug=False,
                   enable_asserts=True, num_devices=8)
    ins = {
        "x": nc.dram_tensor("x", [N, 768], F32, kind="ExternalInput").ap(),
        "relh": nc.dram_tensor("relh", [63, 32], F32, kind="ExternalInput").ap(),
        "relw": nc.dram_tensor("relw", [63, 32], F32, kind="ExternalInput").ap(),
    }
    outs = {
        "out": nc.dram_tensor("out", [N, 256], F32, kind="ExternalOutput").ap(),
    }
    with tile.TileContext(nc) as tc:
        kernel_body(tc, outs, ins)
    split_multiwaits(nc)
    _NC_CACHE["nc"] = nc
    return nc


def kernel(inputs, key_rel_h, key_rel_w, _trace=False):
    nc = _build()
    x = np.ascontiguousarray(np.asarray(inputs, dtype=np.float32).reshape(8, N, 768))
    rh = np.ascontiguousarray(np.asarray(key_rel_h, dtype=np.float32))
    rw = np.ascontiguousarray(np.asarray(key_rel_w, dtype=np.float32))
    in_maps = [{"x": x[c], "relh": rh, "relw": rw} for c in range(8)]
    res = bass_utils.run_bass_kernel_spmd(
        nc, in_maps, core_ids=list(range(8)), trace=_trace)
    outp = np.stack([r["out"] for r in res.results])
    if _trace:
        kernel.last_results = res
    return outp.reshape(8, 32, 32, 256)


# revision 54
# speedup vs baseline: 1.1393x; 1.0040x over previous
"""AttentionAugmentation2D Trainium2 kernel (v6).

Shapes (hardcoded): B=8, H=W=32, N=1024, NH=8 heads, dk=dv=32 per head.
inputs [8,32,32,768] = q|k|v (256 each), key_rel_h/w [63,32].
Sharding: data-parallel over batch B across the 8 cores.

Math per (batch, head), n=(i,j), m=(i',j') (i = H index):
  logits[n,m] = q[n]@k[m] + q[(j,i)]@rel_h[i'-i+31] + q[(i,j)]@rel_w[i'-i+31]
Both rel terms depend on m only through i', so with
  SWT[u,n] = rel_w[u]@q[(i,j)] + rel_h[u]@q[(j,i)]        (u in [0,63))
  biasT[t,n] = SWT[t+31-i(n), n]                          (shifted windows)
we get  logits^T = K_aug^T.T @ Q_augT  with contraction 64:
  K_aug rows: 0:32 = k^T, 32:64 = onehot[t==i'(m)] ;  Q_aug: [q^T; biasT].

v6 redesign vs v5 (cost-model driven):
 - All PE operands are bf16 (error budget 2e-2 is ~10x away): packed
   bf16 SBUF-to-SBUF DVE copies run in 4x perf mode.
 - attn@V swaps operand roles: the exp-weight chunk ew[:, nt*128:...]
   is the *stationary* matmul operand and v (32 cols + a ones column
   for the softmax row-sums) streams, so each matmul charges only 33
   rows instead of 512.  The output lands n-major, which kills v5's 64
   flush transposes and attn^T staging copies; accumulation uses 8
   sub-bank [128,33] regions spaced 64 cols apart in ONE psum bank,
   single-buffered across heads (each head's epilogue completes before
   the next head's first accumulating matmul needs the regions).
 - biasT is built by TWO accumulating matmuls per i-block (relw-window
   x a q^T block, then relh-window x a stride-32 column view of q^T
   that realizes the (i,j)->(j,i) permutation for free), so no
   permuted-q replica and no separate bias-rhs tile exist; both rhs
   reads come straight from qaug rows 0:32.
 - The softmax exp is split between the Act engine (activation Exp)
   and the Pool engine: gpsimd supports elementwise pow, so
   exp(s*x) = pow(e^s, x) with a stride-0 broadcast base.  gpsimd
   cannot read PSUM, so DVE stages the offloaded logit tiles
   PSUM->SBUF; the offload count balances Act against DVE+Pool.
 - Per-head epilogue: DVE reciprocal over the 8 strided ones-sums,
   one broadcast tensor_tensor multiply normalizing all 8 regions into
   the n-major staging tile, one strided DMA per head.
 - PSUM map: banks 0-5 = one [128,8,512] region manually slotted in
   512-col halves: pair (0,1) serves the Pool-offloaded logit tiles,
   the startup/half-1 transposes, and (heads >= 3) the bias scratch;
   pairs (2,3)/(4,5) alternate (continuously across heads) for the
   Act-exp'd tiles.  Banks 6/7 double-buffer the attn regions across
   heads (each head's bank is reset by ONE full-width start=True
   matmul -- interleaved per-region start writes clobber each other on
   real HW -- and all attn@V matmuls accumulate with start=False);
   bank 7 also hosts heads 0-2's bias scratch and the warm-up dummies
   before head 1's accumulation begins.
 - Engine queue discipline: the Tile scheduler reorders engine streams
   with its internal timing model, so ALL PE/DVE/Pool instructions are
   pinned in emission order with ordering-only deps; logit matmuls are
   emitted two tiles ahead so they only WAR-wait the exp that frees
   their psum pair (~1us of margin); deferred attn@V and the epilogue
   of head h-1 are emitted inside head h's j1 iteration.
 - rel tables are DMA'd in natural [63,32] layout (a transposed DMA
   would cost ~2000 descriptors on the single shared DMA stream, ahead
   of the critical q/k row loads) and transposed on the PE instead.

Toolchain note: walrus codegen only fits ONE semaphore wait in most
TPB instruction structs; split_multiwaits() moves excess waits onto
same-engine InstNoOp carriers (same workaround as v5).
"""

import numpy as np

import concourse.bass as bass
import concourse.mybir as mybir
import concourse.tile as tile
from concourse import bass_utils
from concourse.masks import make_identity
from concourse.tile import add_dep_helper

F32 = mybir.dt.float32
F32R = mybir.dt.float32r
BF16 = mybir.dt.bfloat16
AF = mybir.ActivationFunctionType

NH = 8
N = 1024
DK = 32
SCALE = float(DK) ** -0.5
BASE = float(np.exp(SCALE))

# Per-head tuple of js whose exp runs on Pool (via DVE PSUM->SBUF stage).
# Must be a subset of {0, 3} (those js own psum slot pair (0,1)); their
# attn@V is deferred to head end.
OFFLOAD = {h: (0,) for h in range(NH)}

KMARKS = []   # (inst_name, label) for trace debugging


def split_multiwaits(nc, dma_limit=1):
    """Move excess semaphore waits onto same-engine nop carriers."""
    n_new = 0
    for f in nc.m.functions:
        for blk in f.blocks:
            newlist = []
            for inst in blk.instructions:
                si = getattr(inst, "sync_info", None)
                is_dma = isinstance(inst, mybir.InstDMACopy)
                limit = dma_limit if is_dma else 1
                if si is not None and len(si.on_wait) > limit:
                    waits = list(si.on_wait)
                    for w in waits[:-1]:
                        n_new += 1
                        newlist.append(mybir.InstNoOp(
                            name=f"I-wc{n_new}",
                            ins=[], outs=[],
                            sync_info=mybir.SyncInfo(on_wait=[w], on_update=[]),
                            bass_nofuse=True,
                            engine=inst.engine,
                        ))
                    inst.sync_info = mybir.SyncInfo(
                        on_wait=waits[-1:], on_update=si.on_update)
                newlist.append(inst)
            blk.instructions = newlist
    return n_new


def kernel_body(tc, outs, ins):
    nc = tc.nc
    x = ins["x"]          # [1024, 768] rows n=(i,j), cols q|k|v
    relh = ins["relh"]    # [63, 32]
    relw = ins["relw"]    # [63, 32]
    out = outs["out"]     # [1024, 256]

    with (
        tc.tile_pool(name="persist", bufs=1) as persist,
        tc.tile_pool(name="expw", bufs=10) as expwp,
        tc.tile_pool(name="qaug", bufs=3) as qaugp,
        tc.tile_pool(name="lstg", bufs=3) as lstgp,
        tc.tile_pool(name="psmain", bufs=1, space="PSUM") as psmain,
    ):
        # ---- PSUM map ----
        ps_all = psmain.tile([128, 8, 512], F32, tag="ps")
        ps_flat = ps_all.rearrange("p s c -> p (s c)")

        def slot(s, n=1):
            return ps_flat[:, s * 512:(s + n) * 512]
        def ps_att_of(h):
            return slot(6 + h % 2)
        ps_bias = slot(7)

        # ---------------- DMAs (the DMA stream is serial in practice:
        # critical q rows first, tiny rel loads sandwiched, v last) ----
        rowsR = persist.tile([128, 4, 8, 128], F32R)
        CB_COLS = (0, 256, 128, 384)   # q0, k0, q1, k1

        def emit_rows_dma(cb, eng, lo=0, hi=8):
            src = bass.AP(tensor=x.tensor,
                          offset=CB_COLS[cb] + lo * 128 * 768,
                          ap=[[768, 128], [128 * 768, hi - lo], [1, 128]],
                          ).bitcast(F32R)
            eng.dma_start(out=rowsR[:, cb, lo:hi, :], in_=src)

        rel_nat = persist.tile([64, 64], F32R)
        nc.vector.memset(rel_nat.bitcast(F32), 0.0)
        v_st = persist.tile([128, 8, 256], F32)

        def emit_v_dma(j):
            nc.sync.dma_start(
                out=v_st[:, j, :], in_=x[j * 128:(j + 1) * 128, 512:768])

        # ALL input DMAs ride the SP queue: a dma_start costs ~667ns of
        # SEQUENCER time on the issuing engine, and the Act sequencer must
        # stay free to issue exps; a single queue also makes the serial
        # DMA stream follow emission order exactly
        emit_rows_dma(0, nc.sync, 0, 4)          # q half0 lo
        nc.sync.dma_start(out=rel_nat[0:63, 0:32], in_=relw.bitcast(F32R))
        emit_rows_dma(0, nc.sync, 4, 8)          # q half0 hi
        nc.sync.dma_start(out=rel_nat[0:63, 32:64], in_=relh.bitcast(F32R))
        emit_rows_dma(1, nc.sync, 0, 4)          # k half0 lo
        emit_v_dma(0)
        emit_rows_dma(1, nc.sync, 4, 8)          # k half0 hi
        emit_v_dma(1)
        emit_v_dma(2)
        emit_rows_dma(2, nc.sync)                # q half1
        emit_rows_dma(3, nc.sync)                # k half1
        for j in range(3, 8):
            emit_v_dma(j)

        # ---------------- warm-up + constants ----------------
        _chain = {}
        dummy_sb = persist.tile([128, 64], F32)
        nc.vector.memset(dummy_sb, 0.0)
        for w in range(20):
            _dm = nc.tensor.matmul(ps_bias[64:96, 0:32],
                                   lhsT=dummy_sb[:, 0:32],
                                   rhs=dummy_sb[:, 0:32],
                                   start=True, stop=True)
            _chain.setdefault("pe", _dm)
            if _chain["pe"] is not _dm:
                add_dep_helper(_dm.ins, _chain["pe"].ins, sync=False,
                               reason="pin pe queue order")
            _chain["pe"] = _dm

        zeros_bf = persist.tile([128, 128], BF16)
        nc.vector.memset(zeros_bf, 0.0)
        # pre-warm the Act engine's Exp table (the first activation pays
        # a ~1.3us table load otherwise -- on the critical startup path)
        act_warm = persist.tile([128, 1], F32)
        nc.scalar.activation(act_warm, dummy_sb[:, 0:1], AF.Exp, scale=SCALE)

        ident = persist.tile([128, 128], F32)
        make_identity(nc, ident)
        identR = persist.tile([128, 128], F32R)
        nc.vector.tensor_copy(identR, ident)
        ident_marker = nc.gpsimd.tensor_copy(ident[0:1, 0:1], ident[0:1, 0:1])

        # pow base for the Pool exp share: pow(e^s, x) = exp(s*x)
        base_t = persist.tile([128, 1], F32)
        nc.vector.memset(base_t, BASE)

        def base_bcast(cols):
            return bass.AP(tensor=base_t.tensor, offset=base_t.offset,
                           ap=[list(base_t.ap[0]), [0, cols]])

        # Pin Pool and startup-DVE queue order (the Tile scheduler's
        # internal timing model reorders engine queues badly otherwise).
        _chain["pool"] = ident_marker

        def chained(engine_name, inst):
            prev = _chain.get(engine_name)
            if prev is not None:
                add_dep_helper(inst.ins, prev.ins, sync=False,
                               reason=f"pin {engine_name} queue order")
            _chain[engine_name] = inst
            return inst

        def pool(op, *args, **kwargs):
            return chained("pool", getattr(nc.gpsimd, op)(*args, **kwargs))

        def pe(inst):
            return chained("pe", inst)

        # onehot rows for K_aug (bf16) -- no input deps, head of Pool chain
        oh_st = persist.tile([32, 8, 128], F32)
        pool("memset", oh_st, 0.0)
        oh = oh_st.rearrange("t j (b m) -> t j b m", b=4)
        pool("affine_select",
             out=oh, in_=oh, compare_op=mybir.AluOpType.not_equal,
             fill=1.0, base=0, pattern=[[-4, 8], [-1, 4], [0, 32]],
             channel_multiplier=1)
        oh_bf = persist.tile([32, 8, 128], BF16)
        pool("tensor_copy", oh_bf, oh_st)

        # bf16 transposed replicas of q and k: [p=32*(h%4)+d, half, n]
        qT = persist.tile([128, 2, N], BF16)
        kT = persist.tile([128, 2, N], BF16)

        ka = [persist.tile([64, 8, 128], BF16, tag=f"ka{i}", name=f"ka{i}")
              for i in range(4)]

        # rel^T: [32, 2, 63] at partitions 0:32 (PE operands must share a
        # base partition): [:,0,:] = relw^T, [:,1,:] = relh^T
        rel_bf = persist.tile([32, 2, 63], BF16)

        def emit_transposes(cb, dst, half, lo, s, pin=False, pe_anchor=None):
            """4 transposes into one psum slot + one wide copy into the
            bf16 replica."""
            pt = slot(s).bitcast(F32R)
            for c in range(4):
                tr = pe(nc.tensor.transpose(pt[:, c * 128:(c + 1) * 128],
                                            rowsR[:, cb, lo + c, :], identR))
            cp = nc.vector.tensor_copy(
                dst[:, half, lo * 128:(lo + 4) * 128], slot(s))
            if pin:
                chained("dve", cp)

        # ---------------- per-head staging ----------------
        def emit_qstage(h, pin=False):
            """qaug rows 0:32 = q^T for head h (bf16 4x copy)."""
            lane = (h % 4) * 32
            qsT = qT[lane:lane + 32, h // 4, :]
            qaug = qaugp.tile([64, N], BF16, tag="qaug", name=f"qaug{h}")
            c1 = nc.vector.tensor_copy(qaug[0:32, :], qsT)
            if pin:
                chained("dve", c1)
            return qaug

        def emit_bias(h, qaug, halves=(0, 1), pin=False, pe_anchor=None):
            """biasT[t,(i,j)] = SWT[t+31-i,(i,j)]: per i-block, two
            accumulating matmuls with shifted rel windows; the relh term
            reads q^T through a stride-32 column view (the (i,j)->(j,i)
            permutation).  Scratch: heads 0-2 use bank 7 halves at
            partitions 0:32/32:64 (bank 7 becomes attn-B from head 1's
            accumulation on); later heads use the idle offload slots."""
            for half in halves:
                if h <= 2:
                    ps_b = ps_bias[half * 32:half * 32 + 32, :]
                else:
                    ps_b = slot(half)[0:32, :]
                for ib in range(16):
                    i = half * 16 + ib
                    pe(nc.tensor.matmul(
                        ps_b[:, ib * 32:(ib + 1) * 32],
                        lhsT=rel_bf[:, 0, 31 - i:63 - i],
                        rhs=qaug[0:32, i * 32:(i + 1) * 32],
                        start=True, stop=False))
                    perm_rhs = bass.AP(
                        tensor=qaug.tensor, offset=qaug.offset + i,
                        ap=[[qaug.ap[0][0], 32], [32, 32]])
                    pe(nc.tensor.matmul(
                        ps_b[:, ib * 32:(ib + 1) * 32],
                        lhsT=rel_bf[:, 1, 31 - i:63 - i],
                        rhs=perm_rhs,
                        start=False, stop=True))
                cp = nc.vector.tensor_copy(
                    qaug[32:64, half * 512:(half + 1) * 512], ps_b)
                if pin:
                    chained("dve", cp)

        def emit_kaug(h, pin=False, los=(0, 8)):
            lane = (h % 4) * 32
            lo, hi = los
            ksT = kT[lane:lane + 32, h // 4, lo * 128:hi * 128]
            cp = nc.vector.tensor_copy(
                ka[h % 4][0:32, lo:hi].rearrange("d j m -> d (j m)"), ksT)
            if pin:
                chained("dve", cp)

        def emit_kaug_oh(h, pin=False):
            if h < 4:
                co = nc.vector.tensor_copy(ka[h % 4][32:64], oh_bf)
                if pin:
                    chained("dve", co)

        # ---------------- v staging ----------------
        v_aug = persist.tile([128, NH, 8, 33], BF16)
        ones_st = persist.tile([128, 64], F32)
        nc.vector.memset(ones_st, 1.0)

        def emit_vconv(j):
            pool("tensor_copy",
                 v_aug[:, :, j, 0:32],
                 v_st[:, j, :].rearrange("p (h d) -> p h d", h=NH))

        nc.vector.tensor_copy(
            v_aug[:, :, :, 32:33].rearrange("p h j o -> p (h j o)"), ones_st)

        # ---------------- epilogue ----------------
        out_sb = persist.tile([128, 8, 256], F32)
        rec_t = persist.tile([128, NH, 8], F32)

        def emit_epilogue(h):
            """reciprocal of the 8 ones-sums + one broadcast normalize of
            the 8 [128,33] regions into out_sb; one strided DMA."""
            ps_att = ps_att_of(h)
            rec = rec_t[:, h, :]
            sums_ap = bass.AP(tensor=ps_att.tensor, offset=ps_att.offset + 32,
                              ap=[list(ps_att.ap[0]), [64, 8]])
            chained("dve", nc.vector.reciprocal(rec, sums_ap))
            in0 = bass.AP(tensor=ps_att.tensor, offset=ps_att.offset,
                          ap=[list(ps_att.ap[0]), [64, 8], [1, 32]])
            in1 = bass.AP(tensor=rec.tensor, offset=rec.offset,
                          ap=[list(rec.ap[0]), [1, 8], [0, 32]])
            out_ap = bass.AP(tensor=out_sb.tensor,
                             offset=out_sb.offset + h * 32,
                             ap=[list(out_sb.ap[0]), [256, 8], [1, 32]])
            chained("dve", nc.vector.tensor_tensor(
                out=out_ap, in0=in0, in1=in1, op=mybir.AluOpType.mult))
            groups = ((0, 8),)
            for glo, ghi in groups:
                dstap = bass.AP(
                    tensor=out.tensor,
                    offset=glo * 128 * 256 + h * 32,
                    ap=[[256, 128], [128 * 256, ghi - glo], [1, 32]])
                nc.sync.dma_start(out=dstap,
                                  in_=out_sb[:, glo:ghi, h * 32:(h + 1) * 32])

        # ---------------- startup ----------------
        # PE order: dummies, rel-w transpose, q transposes, rel-h
        # transpose, k transposes, bias mms, logits.  The DVE chain IS
        # the head-0 critical path: qaug0 and ka0 rows are copied
        # straight out of the transpose psum slots (lane 0); the qT
        # replica copies (for later heads) trail behind and delay only
        # head 0's j0, whose exp is Pool-offloaded and slack-tolerant.
        # Slot use: q-lo->0, q-hi->1, rel->4 (cols 0:127), k-lo->2,
        # k-hi->5; slot 3 stays free for j1's logits.
        relT = slot(4).bitcast(F32R)
        pe(nc.tensor.transpose(relT[0:32, 0:64],
                               rel_nat[:, 0:32], identR[0:64, 0:64]))

        def transp4(cb, lo, s):
            pt = slot(s).bitcast(F32R)
            for c in range(4):
                pe(nc.tensor.transpose(pt[:, c * 128:(c + 1) * 128],
                                       rowsR[:, cb, lo + c, :], identR))

        transp4(0, 0, 0)                                # q half0 lo
        pe(nc.tensor.transpose(relT[0:32, 64:128],
                               rel_nat[:, 32:64], identR[0:64, 0:64]))
        transp4(0, 4, 1)                                # q half0 hi
        transp4(1, 0, 2)                                # k half0 lo

        chained("dve", nc.vector.tensor_copy(
            rel_bf.rearrange("p a u -> p (a u)"),
            bass.AP(tensor=ps_flat.tensor,
                    offset=ps_flat.offset + 4 * 512,
                    ap=[[ps_flat.ap[0][0], 32], [64, 2], [1, 63]])))
        qaug_h = {0: qaugp.tile([64, N], BF16, tag="qaug", name="qaug0")}
        chained("dve", nc.vector.tensor_copy(
            qaug_h[0][0:32, 0:512], slot(0)[0:32, :]))
        chained("dve", nc.vector.tensor_copy(
            qaug_h[0][0:32, 512:1024], slot(1)[0:32, :]))
        chained("dve", nc.vector.tensor_copy(
            ka[0][0:32, 0:4].rearrange("d j m -> d (j m)"),
            slot(2)[0:32, :]))
        # ka[1] rows straight off the same transpose slots (lane 1 =
        # psum partitions 32:64; DVE copies may shift partitions)
        chained("dve", nc.vector.tensor_copy(
            ka[1][0:32, 0:4].rearrange("d j m -> d (j m)"),
            slot(2)[32:64, :]))
        emit_kaug_oh(0, pin=True)
        emit_bias(0, qaug_h[0], halves=(0,), pin=True)
        emit_bias(0, qaug_h[0], halves=(1,), pin=True)
        transp4(1, 4, 5)                                # k half0 hi
        chained("dve", nc.vector.tensor_copy(
            ka[0][0:32, 4:8].rearrange("d j m -> d (j m)"),
            slot(5)[0:32, :]))
        chained("dve", nc.vector.tensor_copy(
            ka[1][0:32, 4:8].rearrange("d j m -> d (j m)"),
            slot(5)[32:64, :]))
        pool("tensor_copy", ka[1][32:64], oh_bf)
        # trailing (delays only head 0's slack-tolerant j0): qT replica
        chained("dve", nc.vector.tensor_copy(qT[:, 0, 0:512], slot(0)))
        chained("dve", nc.vector.tensor_copy(qT[:, 0, 512:1024], slot(1)))
        for j in range(8):
            emit_vconv(j)

        # ---------------- main pipeline ----------------
        ACT_PAIRS = ((2, 3), (4, 5))
        act_rot = [0]   # continuous pair rotation across heads

        def emit_logits(h, j, lo_slot):
            qaug = qaug_h[h]
            for half in range(2):
                mm = pe(nc.tensor.matmul(
                    slot(lo_slot + half), lhsT=ka[h % 4][:, j, :],
                    rhs=qaug[:, half * 512:(half + 1) * 512],
                    start=True, stop=True))
                KMARKS.append((mm.ins.name, f"logits{h}_{j}_h{half}_s{lo_slot+half}"))

        def hook(h, j, a):
            if h == 0:
                if j == 2:
                    emit_transposes(2, qT, 1, 0, s=0, pin=True)
                elif j == 3:
                    emit_transposes(2, qT, 1, 4, s=1, pin=True)
                    qaug_h[1] = emit_qstage(1, pin=True)
                    emit_bias(1, qaug_h[1], halves=(0,), pin=True)
                elif j == 4:
                    # kT half0 replica: re-transpose (startup slots were
                    # drained into ka0/ka1 directly)
                    emit_transposes(1, kT, 0, 0, s=0, pin=True)
                    emit_bias(1, qaug_h[1], halves=(1,), pin=True)
                elif j == 5:
                    emit_transposes(1, kT, 0, 4, s=0, pin=True)
                elif j == 6:
                    qaug_h[2] = emit_qstage(2, pin=True)
                    emit_bias(2, qaug_h[2], halves=(0,), pin=True)
                elif j == 7:
                    emit_bias(2, qaug_h[2], halves=(1,), pin=True)
            elif h == 1:
                if j == 1:
                    emit_transposes(3, kT, 1, 0, s=0, pin=True)
                    emit_kaug(2, pin=True)
                    emit_kaug_oh(2, pin=True)
                elif j == 2:
                    qaug_h[3] = emit_qstage(3, pin=True)
                elif j == 3:
                    emit_bias(3, qaug_h[3], halves=(0,), pin=True)
                elif j == 4:
                    emit_bias(3, qaug_h[3], halves=(1,), pin=True)
                elif j == 5:
                    emit_transposes(3, kT, 1, 4, s=0, pin=True)
            else:
                if j == 1 and h + 1 < NH:
                    emit_kaug(h + 1, pin=True)
                    emit_kaug_oh(h + 1, pin=True)
                elif j == 2 and h + 2 < NH:
                    qaug_h[h + 2] = emit_qstage(h + 2, pin=True)
                elif j == 3 and h + 2 < NH:
                    emit_bias(h + 2, qaug_h[h + 2], halves=(0,), pin=True)
                elif j == 4 and h + 2 < NH:
                    emit_bias(h + 2, qaug_h[h + 2], halves=(1,), pin=True)

        def emit_offload(h, j):
            emit_logits_once((h, j))
            ls = lstgp.tile([128, N], F32, tag="ls", name=f"ls{h}_{j}")
            KMARKS.append((chained("dve", nc.vector.tensor_copy(
                ls, slot(0, 2))).ins.name, f"stage{h}_{j}"))
            ew = expwp.tile([128, N], BF16, tag="ew", name=f"ew{h}_{j}")
            chained("pool", nc.gpsimd.tensor_tensor(
                out=ew, in0=base_bcast(N), in1=ls,
                op=mybir.AluOpType.pow))
            return ew

        def emit_attnv_zero(h):
            # one full-width start=True matmul resets the attn bank;
            # interleaved per-region start writes clobber each other on HW
            pe(nc.tensor.matmul(ps_att_of(h), lhsT=zeros_bf,
                                rhs=qT[:, 0, 0:512],
                                start=True, stop=False))

        def emit_attnv(h, j, ew, start, stop):
            ps_att = ps_att_of(h)
            a_last = None
            for nt in range(8):
                a_last = pe(nc.tensor.matmul(
                    ps_att[:, nt * 64:nt * 64 + 33],
                    lhsT=ew[:, nt * 128:(nt + 1) * 128],
                    rhs=v_aug[:, h, j, :],
                    start=start, stop=stop))
            return a_last

        # pending deferred work from head h-1, emitted inside head h's j1
        # iteration (gives the slow offload pipeline extra time before its
        # attn@V could block the in-order PE queue):
        pending = None

        # pair assignment per tile, rotation continuous across heads
        pair_of = {}
        rot = 0
        for h in range(NH):
            for j in range(8):
                if j in OFFLOAD[h]:
                    pair_of[(h, j)] = (0, 1)
                else:
                    pair_of[(h, j)] = ACT_PAIRS[rot % 2]
                    rot += 1

        emitted_logits = set()

        def next_tile(h, j):
            if j < 7:
                return (h, j + 1)
            return (h + 1, 0) if h + 1 < NH else None

        def emit_logits_once(t):
            if t is not None and t not in emitted_logits:
                emitted_logits.add(t)
                emit_logits(t[0], t[1], pair_of[t][0])

        # head-0 priming: j1/j2 logits first (they gate Act); j0 last --
        # its slots are released only by the trailing qT replica copies,
        # and its Pool-exp pipeline has most of the head as slack
        for t in ((0, 1), (0, 2)):
            emit_logits_once(t)
        for h in range(NH):
            off_js = OFFLOAD[h]
            act_js = [j for j in range(8) if j not in off_js]
            last = NH - 1
            first_j = act_js[0]
            last_j = 7 if h == last else 0
            ews = {}
            for j in range(8):
                # this tile's logits were emitted one iteration ago; emit
                # the NEXT tile's logits before this tile's attn@V so the
                # pinned PE queue never waits an exp to issue logits
                if h == last and j == 7:
                    emit_attnv(h, 0, ews[0], False, False)
                if j in off_js:
                    if j == 0 and h != 1:
                        # h=1's bank (7) drains bias(2) late; its zero is
                        # deferred to (1, j1) so it can't block the chain
                        emit_attnv_zero(h)
                    ews[j] = emit_offload(h, j)
                    emit_logits_once(next_tile(h, j))
                    nt2 = next_tile(h, j)
                    if nt2 is not None:
                        emit_logits_once(next_tile(*nt2))
                else:
                    ew = expwp.tile([128, N], BF16, tag="ew",
                                    name=f"ew{h}_{j}")
                    if (h, j) in ((0, 1), (0, 2)):
                        # two half-exps: half0 unblocks on bias-half-a,
                        # pulling the whole Act stream ~1us earlier
                        for hf in range(2):
                            KMARKS.append((nc.scalar.activation(
                                ew[:, hf * 512:(hf + 1) * 512],
                                slot(pair_of[(h, j)][0] + hf),
                                AF.Exp, scale=SCALE).ins.name,
                                f"exp{h}_{j}h{hf}"))
                    else:
                        KMARKS.append((nc.scalar.activation(
                            ew, slot(pair_of[(h, j)][0], 2),
                            AF.Exp, scale=SCALE).ins.name, f"exp{h}_{j}"))
                    ews[j] = ew
                    # two tiles ahead: logits(t+2) only WAR-waits this
                    # exp's pair, giving the chain ~1us of margin.  At j6
                    # the two-ahead tile would be the next head's j0 --
                    # whose slots drain late but whose Pool-exp pipeline
                    # has slack -- so emit the next head's j1 instead and
                    # let j0 follow at its own iteration.
                    if j == 6:
                        emit_logits_once((h, 7))
                        if h + 1 < NH:
                            emit_logits_once((h + 1, 1))
                    else:
                        emit_logits_once(next_tile(h, j))
                        nt2 = next_tile(h, j)
                        if nt2 is not None:
                            emit_logits_once(next_tile(*nt2))
                        if j == 7 and h + 1 < NH:
                            # keep the two-ahead margin across the
                            # boundary: the next head's j2 as well
                            emit_logits_once((h + 1, 2))
                    if j == 1 and pending is not None:
                        ph, defs = pending
                        for idx, (pj, pew) in enumerate(defs):
                            emit_attnv(ph, pj, pew, False,
                                       idx == len(defs) - 1)
                        emit_epilogue(ph)
                        pending = None
                    if h == last and j == 7 and 3 in off_js:
                        emit_attnv(h, 3, ews[3], False, False)
                    if h == 1 and j == 1:
                        emit_attnv_zero(1)
                    a_last = emit_attnv(h, j, ew, False, j == last_j)
                    hook(h, j, a_last)
            if h != last:
                # all deferred attn@V for Pool-exp'd js punts into head
                # h+1's j1 (so a pow still in flight can never block the
                # in-order PE queue at the head boundary)
                pending = (h, [(j, ews[j]) for j in off_js])

        emit_epilogue(NH - 1)


_NC_CACHE = {}


def _build():
    if "nc" in _NC_CACHE:
        return _NC_CACHE["nc"]
    nc = bass.Bass("TRN2", target_bir_lowering=False, debug=False,
                   enable_asserts=True, num_devices=8)
    ins = {
        "x": nc.dram_tensor("x", [N, 768], F32, kind="ExternalInput").ap(),
        "relh": nc.dram_tensor("relh", [63, 32], F32, kind="ExternalInput").ap(),
        "relw": nc.dram_tensor("relw", [63, 32], F32, kind="ExternalInput").ap(),
    }
    outs = {
        "out": nc.dram_tensor("out", [N, 256], F32, kind="ExternalOutput").ap(),
    }
    with tile.TileContext(nc) as tc:
        kernel_body(tc, outs, ins)
    split_multiwaits(nc)
    _NC_CACHE["nc"] = nc
    return nc


def kernel(inputs, key_rel_h, key_rel_w, _trace=False):
    nc = _build()
    x = np.ascontiguousarray(np.asarray(inputs, dtype=np.float32).reshape(8, N, 768))
    rh = np.ascontiguousarray(np.asarray(key_rel_h, dtype=np.float32))
    rw = np.ascontiguousarray(np.asarray(key_rel_w, dtype=np.float32))
    in_maps = [{"x": x[c], "relh": rh, "relw": rw} for c in range(8)]
    res = bass_utils.run_bass_kernel_spmd(
        nc, in_maps, core_ids=list(range(8)), trace=_trace)
    outp = np.stack([r["out"] for r in res.results])
    if _trace:
        kernel.last_results = res
    return outp.reshape(8, 32, 32, 256)
